# revision 29
# baseline (speedup 1.0000x reference)
"""GCN block (GraphConv + BatchNorm1d + ReLU) on 8 Trainium2 NeuronCores.

Strategy (per sharding hint): partition nodes (and incident edges) across the
8 cores; replicate W/b/gamma/beta; all-reduce BN batch statistics.

Per core k (owns dst nodes [k*NPC, (k+1)*NPC)):
  1. h_k = (x_k @ W) * rsqrt(clip(deg_out_k,1))  (PE matmul; x columns are
     host-permuted so even nodes land on partitions 0:64, odd on 64:128).
  2. TWO AllGathers of h (bf16): even nodes, then odd nodes. Gathers that
     read even-sourced edges overlap the second collective. The 4
     int16-indexable "bank" tables are *interleaved strided views* of the
     AG outputs: bank b = nodes with (n % NPC) % 4 == b lives in half b%2
     at row 2*j + b//2, j = owner*(NPCP/4) + (n % NPC)//4 (elem_step=2
     rows). Interleaving makes bank-row -> table-row linear, so two big
     collectives (cheap) serve four int16-indexed gather tables.
  3. For each (bank, dst-group) run of edges (64-slot granular, shared
     run sizes = max over cores), gather h[src] rows (dma_gather, bf16,
     one batched gather per (phase, chunk-of-groups, bank)) and
     segment-sum them with one-hot matmuls M^T @ G accumulated in PSUM.
     Each dst group keeps ONE psum accumulation per phase; the phase-A
     partial spills to bf16 and is re-injected via an identity matmul.
     Runs straddling 128-slot block boundaries get one matmul per
     straddled block; out-of-segment slots carry doff 255 so their
     one-hot column is zero.
  4. relu(psum * rsqrt(clip(deg_in,1)) [+ b]) via ACT directly from PSUM
     (bf16 out); BN sums via ones-matmuls (single accumulation group);
     AllReduce sums; y = h*S + T with S = gamma*rsqrt(var+eps),
     T = beta - mu*S (broadcast-AP DVE ops); y cast bf16->f32 during the
     output DMA (SWDGE), pipelined per 14-group batch.

Host-side work is limited to integer index bookkeeping (bucketing edges by
(core, src-bank, dst-group), degree counting) and layout transforms (x^T
permutation/padding, int16 gather indices). All floating-point math runs on
device.

Run sizes are padded to a structure shared by all 8 cores so a single SPMD
NEFF serves every core; pad slots re-gather the run's last row (HBM page
hit) and carry a dst offset of 255 -> contribute exactly 0. Edges are
sorted by gather row within each bucket for HBM locality.
"""
import math
import os
import sys

sys.path.insert(0, "/opt/trn_rl_repo")

import numpy as np

import concourse.bacc as bacc
import concourse.bass as bass
import concourse.mybir as mybir
import concourse.tile as tile
from concourse import bass_utils

F32 = mybir.dt.float32
BF16 = mybir.dt.bfloat16
I16 = mybir.dt.int16

CFG = dict(
    N=100000,
    E=1600000,
    IN=256,
    OUT=128,
    NCORES=8,
    GRP=128,          # dst nodes per segment group (= psum partition dim)
    NBANKS=4,         # interleaved src banks (bank rows must be < 32768)
    GCHUNK=8,         # groups per chunk (gather batch granularity)
    EPS=1e-5,
    TRACE=False,
)

LAST_RESULTS = None  # set by kernel() for test harness introspection
LAST_NC = None
LAST_RUN_S = None


def _ceil_div(a, b):
    return (a + b - 1) // b


def _wrap16(idx, ncols):
    """int16 idx list -> [128, ncols] tile: idx i at [i%16, i//16], replicated
    8x across the 16-partition groups (one copy per GpSimd Q7 core)."""
    n = idx.shape[0]
    assert n == ncols * 16
    w = np.ascontiguousarray(idx.reshape(ncols, 16).T)
    return np.tile(w, (8, 1))


def _preprocess(cfg, src, dst):
    """Bucket edges by (owner core, interleaved src bank, dst group); build
    per-core gather-index / dst-offset arrays and the shared run structure."""
    N, E = cfg["N"], cfg["E"]
    C, NBANKS, GRP, GC = cfg["NCORES"], cfg["NBANKS"], cfg["GRP"], cfg["GCHUNK"]
    NPC = N // C
    NG = _ceil_div(NPC, GRP)
    NPCP = NG * GRP                # padded nodes per core (x cols zero-padded)
    assert NPCP % NBANKS == 0
    QB = NPCP // NBANKS            # gather rows per owner per bank view
    BANKROWS = QB * C              # rows per bank view of one AG-half output
    assert BANKROWS < 32768

    src = src.astype(np.int64)
    dst = dst.astype(np.int64)
    deg_out = np.bincount(src, minlength=N).astype(np.float32)
    deg_in = np.bincount(dst, minlength=N).astype(np.float32)

    owner = dst // NPC
    loc = src % NPC
    bank = loc % NBANKS            # interleaved bank of the source
    grow = (src // NPC) * QB + loc // NBANKS   # gather row within bank view
    assert grow.max() < 32768
    grp = (dst % NPC) // GRP
    key = (owner * NBANKS + bank) * NG + grp
    # sort by bucket, then by gather row inside the bucket (HBM locality)
    order = np.lexsort((grow, key))
    s_grow = grow[order]
    s_dst = dst[order]
    s_key = key[order]

    counts = np.bincount(key, minlength=C * NBANKS * NG).reshape(C, NBANKS, NG)
    P = counts.max(axis=0)  # [NBANKS, NG] shared run sizes (64-granular)
    P = ((P + 63) // 64) * 64
    P = np.maximum(P, 64)   # every (b,g) run structurally exists

    # two phases: banks {0,2} (even AG half), then {1,3} (odd half)
    phases = [(0, 2), (1, 3)]
    chunks = [list(range(c, min(c + GC, NG))) for c in range(0, NG, GC)]
    run_seq = [
        (b, g) for ph in phases for ch in chunks for b in ph for g in ch
    ]
    # lay out runs; pad each (phase, chunk, bank) unit to a 128 multiple
    run_off = np.zeros((NBANKS, NG), np.int64)
    units = []  # (bank, first_block, n_blocks) in stream order
    pos = 0
    for ph in phases:
        for ch in chunks:
            for b in ph:
                u0 = pos
                for g in ch:
                    run_off[b, g] = pos
                    pos += P[b, g]
                pos = ((pos + 127) // 128) * 128  # unit pad
                units.append((b, u0 // 128, (pos - u0) // 128))
    nidx_tot = int(pos)
    nb_tot = nidx_tot // 128

    # segments: a run may straddle block boundaries; each (run, block)
    # intersection is one segment = one doff column + one full matmul
    # (out-of-segment slots carry doff 255 -> zero one-hot column).
    run_segs = {}  # (b, g) -> list of (block_t, doff_col, slot_lo, slot_hi)
    nseg = 0
    for b, g in run_seq:
        off = int(run_off[b, g])
        end = off + int(P[b, g])
        segs = []
        t = off // 128
        while t * 128 < end:
            lo = max(off, t * 128)
            hi = min(end, (t + 1) * 128)
            segs.append((t, nseg, lo, hi))
            nseg += 1
            t += 1
        run_segs[(b, g)] = segs

    # boundaries of each (k, b, g) bucket in the sorted edge stream
    bkeys = (
        np.arange(C)[:, None, None] * NBANKS + np.arange(NBANKS)[None, :, None]
    ) * NG + np.arange(NG)[None, None, :]
    starts = np.searchsorted(s_key, bkeys.ravel()).reshape(C, NBANKS, NG)
    ends = np.searchsorted(s_key, bkeys.ravel(), side="right").reshape(C, NBANKS, NG)

    gidx_cores = []
    dstoff_cores = []
    for k in range(C):
        gidx = np.zeros(nidx_tot, np.int16)
        doff_cols = np.full((nseg, 128), 255.0, np.float32)
        for b in range(NBANKS):
            for g in range(NG):
                s, e = starts[k, b, g], ends[k, b, g]
                cnt = e - s
                p0 = int(run_off[b, g])
                if cnt:
                    gidx[p0 : p0 + cnt] = s_grow[s:e].astype(np.int16)
                    # pad slots re-gather the last row (HBM page hit)
                    gidx[p0 + cnt : p0 + int(P[b, g])] = gidx[p0 + cnt - 1]
                    offs = ((s_dst[s:e] % NPC) - g * GRP).astype(np.float32)
                    for t, col, lo, hi in run_segs[(b, g)]:
                        a = max(lo, p0)
                        z = min(hi, p0 + cnt)
                        if z > a:
                            doff_cols[col, a - t * 128 : z - t * 128] = offs[
                                a - p0 : z - p0
                            ]
        # unit-pad slots gather row 0 (gidx stays 0) and have no segment
        gidx_cores.append(_wrap16(gidx, nidx_tot // 16))
        dstoff_cores.append(np.ascontiguousarray(doff_cols.T))

    meta = dict(
        NPC=NPC,
        NPCP=NPCP,
        NG=NG,
        QB=QB,
        BANKROWS=BANKROWS,
        nidx_tot=nidx_tot,
        nb_tot=nb_tot,
        nseg=nseg,
        run_segs=run_segs,
        units=units,
        chunks=chunks,
        run_seq=run_seq,
        deg_out=deg_out,
        deg_in=deg_in,
    )
    return meta, gidx_cores, dstoff_cores


def _tile_major(vec, NG, GRP, pad_val):
    """[NPC] -> [GRP, NG]: entry (p, m) = vec[m*GRP + p], padded."""
    out = np.full((NG * GRP,), pad_val, vec.dtype)
    out[: vec.shape[0]] = vec
    return np.ascontiguousarray(out.reshape(NG, GRP).T)


def _build_nc(cfg, meta, b_nonzero=False):
    N, IN, OUT, C = cfg["N"], cfg["IN"], cfg["OUT"], cfg["NCORES"]
    GRP, NBANKS = cfg["GRP"], cfg["NBANKS"]
    NPC, NPCP, NG = meta["NPC"], meta["NPCP"], meta["NG"]
    nidx_tot, nb_tot = meta["nidx_tot"], meta["nb_tot"]
    units = meta["units"]
    XK = _ceil_div(IN, 128)
    assert OUT == 128 and GRP == 128
    last_w = NPC - (NG - 1) * GRP  # valid rows in the last group
    HALF = NPCP // 2               # rows per AG-half input

    nc = bacc.Bacc(
        "TRN2", target_bir_lowering=False, debug=False, num_devices=C
    )

    # ---- external inputs ----
    NXQ = 4  # x DMA split for earlier stage-B start
    xq = NPCP // NXQ
    assert NPCP % NXQ == 0
    xt = [
        nc.dram_tensor(f"xt{j}", [128, NPCP], BF16, kind="ExternalInput")
        for j in range(XK)
    ]
    wt = [
        nc.dram_tensor(f"wt{j}", [128, OUT], BF16, kind="ExternalInput")
        for j in range(XK)
    ]
    gidx_d = nc.dram_tensor("gidx", [128, nidx_tot // 16], I16, kind="ExternalInput")
    doff_d = nc.dram_tensor("doff", [128, meta["nseg"]], F32, kind="ExternalInput")
    dego_d = nc.dram_tensor("dego", [128, NG], F32, kind="ExternalInput")
    degi_d = nc.dram_tensor("degi", [128, NG], F32, kind="ExternalInput")
    iota_d = nc.dram_tensor("iota", [128, GRP], BF16, kind="ExternalInput")
    gm_d = nc.dram_tensor("gm", [1, OUT], F32, kind="ExternalInput")
    bb_d = nc.dram_tensor("bb", [1, OUT], F32, kind="ExternalInput")
    onesc_d = nc.dram_tensor("onesc", [128, 1], BF16, kind="ExternalInput")
    onest_d = nc.dram_tensor("onest", [128, 1], BF16, kind="ExternalInput")
    onesr_d = nc.dram_tensor("onesr", [1, 128], F32, kind="ExternalInput")
    ident_d = nc.dram_tensor("ident", [128, 128], BF16, kind="ExternalInput")
    if b_nonzero:
        bt_d = nc.dram_tensor("bt", [1, OUT], F32, kind="ExternalInput")

    ypad_d = nc.dram_tensor("ypad", [NG * GRP, OUT], F32, kind="ExternalOutput")

    with tile.TileContext(nc) as tc:
        with (
            tc.tile_pool(name="const", bufs=1) as cpool,
            tc.tile_pool(name="dram", bufs=1, space="DRAM") as dpool,
            tc.tile_pool(name="agg", bufs=1) as apool,
            tc.tile_pool(name="mpool", bufs=8) as mpool,
            tc.tile_pool(name="etmp", bufs=4) as epool,
            tc.tile_pool(name="psg", bufs=4, space="PSUM") as pgpool,
            tc.tile_pool(name="psb", bufs=2, space="PSUM") as pbpool,
            tc.tile_pool(name="pstat", bufs=1, space="PSUM") as pspool,
        ):
            # ---- constants / small tiles ----
            iota_t = cpool.tile([128, GRP], BF16)
            dego_t = cpool.tile([128, NG], F32)
            degi_t = cpool.tile([128, NG], F32)
            nsrc_t = cpool.tile([128, NG], F32)
            ndst_t = cpool.tile([128, NG], F32)
            gm_t = cpool.tile([1, OUT], F32)
            bb_t = cpool.tile([1, OUT], F32)
            onesc_t = cpool.tile([128, 1], BF16)
            onest_t = cpool.tile([128, 1], BF16)
            onesr_t = cpool.tile([1, 128], F32)
            gidx_t = cpool.tile([128, nidx_tot // 16], I16)
            doff_t = cpool.tile([128, meta["nseg"]], F32)
            ident_t = cpool.tile([128, 128], BF16)
            nc.sync.dma_start(ident_t[:], ident_d[:])

            nc.sync.dma_start(iota_t[:], iota_d[:])
            nc.sync.dma_start(dego_t[:], dego_d[:])
            nc.sync.dma_start(degi_t[:], degi_d[:])
            nc.sync.dma_start(gm_t[:], gm_d[:])
            nc.sync.dma_start(bb_t[:], bb_d[:])
            nc.sync.dma_start(onesc_t[:], onesc_d[:])
            nc.sync.dma_start(onest_t[:], onest_d[:])
            nc.sync.dma_start(onesr_t[:], onesr_d[:])
            nc.sync.dma_start(gidx_t[:], gidx_d[:])
            nc.sync.dma_start(doff_t[:], doff_d[:])
            if b_nonzero:
                bt_t = cpool.tile([1, OUT], F32)
                nc.sync.dma_start(bt_t[:], bt_d[:])

            # norms: rsqrt(max(deg, 1))
            for deg_t, norm_t in ((dego_t, nsrc_t), (degi_t, ndst_t)):
                nc.vector.tensor_scalar(
                    norm_t[:], deg_t[:], 1.0, None, op0=mybir.AluOpType.max
                )
                nc.vector.reciprocal(norm_t[:], norm_t[:])
                nc.scalar.activation(
                    norm_t[:], norm_t[:], mybir.ActivationFunctionType.Sqrt
                )

            # internal DRAM for collectives (even/odd node halves)
            _aspace = "Local" if cfg.get("NOCC") else "Shared"
            h_my_e = dpool.tile([HALF, OUT], BF16, name="h_my_e")
            h_my_o = dpool.tile([HALF, OUT], BF16, name="h_my_o")
            h_all_e = dpool.tile(
                [C * HALF, OUT], BF16, addr_space=_aspace, name="h_all_e"
            )
            h_all_o = dpool.tile(
                [C * HALF, OUT], BF16, addr_space=_aspace, name="h_all_o"
            )
            stats_in = dpool.tile([1, 2 * OUT], F32)
            stats_out = dpool.tile([1, 2 * OUT], F32, addr_space=_aspace)

            # relu(norm*agg) output, bf16, [128, NG, OUT]
            agg_t = apool.tile([128, NG, OUT], BF16)

            # ---- stage B: h = (x @ W) * norm_src, cast bf16, store to HBM
            # (staged in SBUF; 2 large DMAs instead of 98 small ones)
            with tc.tile_pool(name="xw", bufs=1) as xwp:
                xts = []
                wts = []
                for j in range(XK):
                    xtile = xwp.tile([128, NPCP], BF16, name=f"xt_s{j}")
                    wtile = xwp.tile([128, OUT], BF16, name=f"wt_s{j}")
                    for q in range(NXQ):
                        nc.sync.dma_start(
                            xtile[:, q * xq : (q + 1) * xq],
                            xt[j][:, q * xq : (q + 1) * xq],
                        )
                    nc.sync.dma_start(wtile[:], wt[j][:])
                    xts.append(xtile)
                    wts.append(wtile)
                hstage = xwp.tile([128, NG, OUT], BF16, name="hstage")
                for m in range(NG):
                    ps = pbpool.tile([128, OUT], F32, tag="hps")
                    for j in range(XK):
                        nc.tensor.matmul(
                            ps[:, :],
                            xts[j][:, m * GRP : (m + 1) * GRP],
                            wts[j][:, :],
                            start=(j == 0),
                            stop=(j == XK - 1),
                        )
                    nc.scalar.activation(
                        hstage[:, m, :],
                        ps[:, :],
                        mybir.ActivationFunctionType.Copy,
                        scale=nsrc_t[:, m : m + 1],
                    )
                # partitions 0:64 = even nodes of each group (loc = g*128+2q),
                # 64:128 = odd (x columns are host-permuted to match) ->
                # h_my_e row g*64+q = node loc 2r exactly
                hq = NG // 4
                for q in range(4):
                    a = q * hq
                    z = (q + 1) * hq if q < 3 else NG
                    nc.sync.dma_start(
                        h_my_e[a * 64 : z * 64, :].rearrange(
                            "(g p) f -> p g f", p=64
                        ),
                        hstage[0:64, a:z, :],
                    )
                    nc.sync.dma_start(
                        h_my_o[a * 64 : z * 64, :].rearrange(
                            "(g p) f -> p g f", p=64
                        ),
                        hstage[64:128, a:z, :],
                    )

            # ---- stage C: two AllGathers (even half, then odd half) ----
            for h_my_h, h_all_h in ((h_my_e, h_all_e), (h_my_o, h_all_o)):
                if cfg.get("NOCC"):
                    rep = (
                        h_my_h[:]
                        .rearrange("(o r) f -> o r f", o=1)
                        .to_broadcast((C, HALF, OUT))
                    )
                    nc.sync.dma_start(
                        h_all_h[:].rearrange("(o r) f -> o r f", o=C), rep
                    )
                else:
                    nc.gpsimd.collective_compute(
                        "AllGather",
                        mybir.AluOpType.bypass,
                        replica_groups=[list(range(C))],
                        ins=[h_my_h[:]],
                        outs=[h_all_h[:]],
                    )

            # interleaved bank views: bank b -> half b%2, row 2j + b//2
            h_banks = [
                (h_all_e if b % 2 == 0 else h_all_o)[:]
                .rearrange("(j k) f -> j (k f)", k=2)[
                    :, (b // 2) * OUT : (b // 2 + 1) * OUT
                ]
                for b in range(NBANKS)
            ]

            # ---- stage D: gather + one-hot matmul segmented sum ----
            # ---- stage E (inline): relu(psum*ndst) + BN partial sums ----
            # Gathers are batched per (chunk, bank); groups are processed
            # sequentially (their 4 bank runs back-to-back) so each PSUM bank
            # holds at most one pending accumulation group.
            ps_stat = pspool.tile([1, 2 * OUT], F32, name="ps_stat")
            ps_sum = ps_stat[:, 0:OUT]
            ps_sq = ps_stat[:, OUT : 2 * OUT]
            ndone = [0]  # groups completed (for BN-sum start/stop flags)

            def finish_group(g, ps_g):
                """relu + BN-sum accumulation for a completed group psum."""
                if b_nonzero:
                    tmp = epool.tile([128, OUT], F32, tag="etmp")
                    nc.vector.scalar_tensor_tensor(
                        tmp[:],
                        ps_g[:],
                        ndst_t[:, g : g + 1],
                        btile_t[:],
                        op0=mybir.AluOpType.mult,
                        op1=mybir.AluOpType.add,
                    )
                    nc.scalar.activation(
                        agg_t[:, g, :], tmp[:], mybir.ActivationFunctionType.Relu
                    )
                else:
                    nc.scalar.activation(
                        agg_t[:, g, :],
                        ps_g[:],
                        mybir.ActivationFunctionType.Relu,
                        scale=ndst_t[:, g : g + 1],
                    )
                ones = onesc_t if g < NG - 1 else onest_t
                i0 = ndone[0]
                # ps_sum/ps_sq share one bank = ONE accumulation group:
                # start only on the very first matmul, stop on the very last.
                nc.tensor.matmul(
                    ps_sum,
                    ones[:],
                    agg_t[:, g, :],
                    start=(i0 == 0),
                    stop=False,
                )
                sq = epool.tile([128, OUT], BF16, tag="esq")
                nc.scalar.activation(
                    sq[:], agg_t[:, g, :], mybir.ActivationFunctionType.Square
                )
                nc.tensor.matmul(
                    ps_sq,
                    ones[:],
                    sq[:],
                    start=False,
                    stop=(i0 == NG - 1),
                )
                ndone[0] += 1

            if b_nonzero:
                # replicate b across partitions once (PE broadcast)
                ps_b = pbpool.tile([128, OUT], F32, tag="hps", name="ps_b")
                btile_t = cpool.tile([128, OUT], F32)
                nc.tensor.matmul(ps_b[:], onesr_t[:], bt_t[:], start=True, stop=True)
                nc.scalar.activation(
                    btile_t[:], ps_b[:], mybir.ActivationFunctionType.Copy
                )

            run_segs = meta["run_segs"]
            chunks = meta["chunks"]
            nbmax = max(nb for _, _, nb in units)
            dstack = tc.tile_pool(name="gath", bufs=8)
            gpool = dstack.__enter__()
            phases = [(0, 2), (1, 3)]
            ui = 0
            for pi, ph in enumerate(phases):
                for ci, ch in enumerate(chunks):
                    gts = {}
                    for b in ph:
                        bank, t0, nblk = units[ui]
                        ui += 1
                        assert bank == b
                        Gt = gpool.tile(
                            [128, nbmax, OUT], BF16, tag="G", name=f"G{pi}_{ci}_{b}"
                        )
                        nc.gpsimd.dma_gather(
                            Gt[:, :nblk, :],
                            h_banks[b],
                            gidx_t[:, t0 * 8 : (t0 + nblk) * 8],
                            nblk * 128,
                            nblk * 128,
                            OUT,
                            elem_step=2 * OUT,
                            single_packet=False,
                        )
                        gts[b] = (Gt, t0)
                    for g in ch:
                        ps_g = pgpool.tile(
                            [128, OUT], F32, tag="aggps", name=f"ps{pi}_{g}"
                        )
                        if pi == 1:
                            # re-inject phase-A partial (spilled bf16)
                            nc.tensor.matmul(
                                ps_g[:],
                                ident_t[:],
                                agg_t[:, g, :],
                                start=True,
                                stop=False,
                            )
                        for bi, b in enumerate(ph):
                            Gt, t0 = gts[b]
                            segs = run_segs[(b, g)]
                            for si, (t, col, lo, hi) in enumerate(segs):
                                Mt = mpool.tile([128, GRP], BF16, tag="M")
                                nc.vector.tensor_scalar(
                                    Mt[:],
                                    iota_t[:],
                                    doff_t[:, col : col + 1],
                                    None,
                                    op0=mybir.AluOpType.is_equal,
                                )
                                nc.tensor.matmul(
                                    ps_g[:],
                                    Mt[:],
                                    Gt[:, t - t0, :],
                                    start=(pi == 0 and bi == 0 and si == 0),
                                    stop=(bi == 1 and si == len(segs) - 1),
                                )
                        if pi == 0:
                            # spill partial sum to agg_t (bf16), no relu yet
                            nc.scalar.activation(
                                agg_t[:, g, :],
                                ps_g[:],
                                mybir.ActivationFunctionType.Copy,
                            )
                        else:
                            finish_group(g, ps_g)
            dstack.__exit__(None, None, None)
            assert ndone[0] == NG

            # ---- stage F: AllReduce BN stats; build affine S/T tiles ----
            st_sb = cpool.tile([1, 2 * OUT], F32)
            nc.scalar.activation(
                st_sb[:, 0:OUT], ps_sum, mybir.ActivationFunctionType.Copy
            )
            nc.scalar.activation(
                st_sb[:, OUT : 2 * OUT], ps_sq, mybir.ActivationFunctionType.Copy
            )
            nc.sync.dma_start(stats_in[:], st_sb[:])
            if cfg.get("NOCC"):
                nc.sync.dma_start(stats_out[:], stats_in[:])
            else:
                nc.gpsimd.collective_compute(
                    "AllReduce",
                    mybir.AluOpType.add,
                    replica_groups=[list(range(C))],
                    ins=[stats_in[:]],
                    outs=[stats_out[:]],
                )
            st_rb = cpool.tile([1, 2 * OUT], F32)
            nc.sync.dma_start(st_rb[:], stats_out[:])

            mu = cpool.tile([1, OUT], F32)
            ex2 = cpool.tile([1, OUT], F32)
            var = cpool.tile([1, OUT], F32)
            srow = cpool.tile([1, OUT], F32)
            trow = cpool.tile([1, OUT], F32)
            inv_n = 1.0 / float(N)
            nc.scalar.activation(
                mu[:], st_rb[:, 0:OUT], mybir.ActivationFunctionType.Copy, scale=inv_n
            )
            nc.scalar.activation(
                ex2[:],
                st_rb[:, OUT : 2 * OUT],
                mybir.ActivationFunctionType.Copy,
                scale=inv_n,
            )
            nc.scalar.activation(var[:], mu[:], mybir.ActivationFunctionType.Square)
            nc.vector.tensor_sub(var[:], ex2[:], var[:])
            # var <- rsqrt(var + eps) (ACT Rsqrt is banned for accuracy)
            nc.scalar.activation(
                var[:],
                var[:],
                mybir.ActivationFunctionType.Copy,
                bias=float(cfg["EPS"]),
            )
            nc.vector.reciprocal(var[:], var[:])
            nc.scalar.activation(var[:], var[:], mybir.ActivationFunctionType.Sqrt)
            nc.vector.tensor_mul(srow[:], gm_t[:], var[:])
            nc.vector.tensor_mul(trow[:], mu[:], srow[:])
            nc.vector.tensor_sub(trow[:], bb_t[:], trow[:])

            S_t = cpool.tile([128, OUT], BF16)
            T_t = cpool.tile([128, OUT], BF16)
            ps_S = pgpool.tile([128, OUT], F32, tag="aggps", name="ps_S")
            ps_T = pgpool.tile([128, OUT], F32, tag="aggps", name="ps_T")
            nc.tensor.matmul(ps_S[:], onesr_t[:], srow[:], start=True, stop=True)
            nc.tensor.matmul(ps_T[:], onesr_t[:], trow[:], start=True, stop=True)
            nc.scalar.activation(S_t[:], ps_S[:], mybir.ActivationFunctionType.Copy)
            nc.scalar.activation(T_t[:], ps_T[:], mybir.ActivationFunctionType.Copy)

            # ---- stage G: y = hrelu * S + T (bf16), cast f32 on DMA out ----
            with tc.tile_pool(name="gtmp", bufs=2) as gpool2:
                GB = 14  # groups per batched op
                S_bc = (
                    S_t[:]
                    .rearrange("p (o f) -> p o f", o=1)
                    .to_broadcast((128, GB, OUT))
                )
                T_bc = (
                    T_t[:]
                    .rearrange("p (o f) -> p o f", o=1)
                    .to_broadcast((128, GB, OUT))
                )
                ypad_view = ypad_d[:].rearrange("(g p) f -> p g f", p=128)
                for g0 in range(0, NG, GB):
                    gw = min(GB, NG - g0)
                    tmp = gpool2.tile([128, GB, OUT], BF16, tag="gtmp")
                    nc.vector.tensor_mul(
                        tmp[:, :gw, :],
                        agg_t[:, g0 : g0 + gw, :],
                        S_bc if gw == GB else S_t[:]
                        .rearrange("p (o f) -> p o f", o=1)
                        .to_broadcast((128, gw, OUT)),
                    )
                    nc.vector.tensor_add(
                        agg_t[:, g0 : g0 + gw, :],
                        tmp[:, :gw, :],
                        T_bc if gw == GB else T_t[:]
                        .rearrange("p (o f) -> p o f", o=1)
                        .to_broadcast((128, gw, OUT)),
                    )
                    nc.gpsimd.dma_start(
                        ypad_view[:, g0 : g0 + gw, :],
                        agg_t[:, g0 : g0 + gw, :],
                    )

    nc.compile()
    return nc


def kernel(x, src, dst, W, b, gamma, beta):
    global LAST_RESULTS
    cfg = CFG
    N, E, IN, OUT, C = cfg["N"], cfg["E"], cfg["IN"], cfg["OUT"], cfg["NCORES"]
    GRP = cfg["GRP"]
    assert x.shape == (N, IN) and W.shape == (IN, OUT)
    assert src.shape == (E,) and dst.shape == (E,)

    b = np.asarray(b, np.float32)
    b_nonzero = bool(np.any(b != 0.0))
    meta, gidx_cores, dstoff_cores = _preprocess(cfg, src, dst)
    NPC, NPCP, NG = meta["NPC"], meta["NPCP"], meta["NG"]
    XK = _ceil_div(IN, 128)
    last_w = NPC - (NG - 1) * GRP
    # node permutation: within each 128-node group, evens first then odds
    perm = np.concatenate([np.arange(0, 128, 2), np.arange(1, 128, 2)])
    g_ = np.arange(NPCP) // 128
    p_ = np.arange(NPCP) % 128
    permn = g_ * 128 + perm[p_]          # source node (local) per padded col
    valid = permn < NPC

    nc = _build_nc(cfg, meta, b_nonzero=b_nonzero)

    xT = np.ascontiguousarray(np.asarray(x, np.float32).T)  # [IN, N]
    Wn = np.asarray(W, np.float32)
    import ml_dtypes

    iota = np.tile(np.arange(GRP, dtype=np.float32)[None, :], (128, 1)).astype(
        ml_dtypes.bfloat16
    )
    onesc = np.ones((128, 1), np.float32)
    onest = np.zeros((128, 1), np.float32)
    onest[:last_w] = 1.0
    onesr = np.ones((1, 128), np.float32)
    gm = np.asarray(gamma, np.float32)[None, :]
    bb = np.asarray(beta, np.float32)[None, :]

    in_maps = []
    for k in range(C):
        im = {
            "gidx": gidx_cores[k],
            "doff": dstoff_cores[k],
            "dego": _tile_major(
                np.where(
                    valid,
                    meta["deg_out"][k * NPC + np.minimum(permn, NPC - 1)],
                    np.float32(1.0),
                ).astype(np.float32),
                NG,
                GRP,
                np.float32(1.0),
            ),
            "degi": _tile_major(
                meta["deg_in"][k * NPC : (k + 1) * NPC], NG, GRP, np.float32(1.0)
            ),
            "iota": iota,
            "gm": gm,
            "bb": bb,
            "onesc": onesc.astype(ml_dtypes.bfloat16),
            "onest": onest.astype(ml_dtypes.bfloat16),
            "onesr": onesr,
            "ident": np.eye(128, dtype=np.float32).astype(ml_dtypes.bfloat16),
        }
        if b_nonzero:
            im["bt"] = b[None, :]
        for j in range(XK):
            xcols = np.zeros((128, NPCP), np.float32)
            xcols[:, valid] = xT[
                j * 128 : (j + 1) * 128, k * NPC + permn[valid]
            ]
            im[f"xt{j}"] = xcols.astype(ml_dtypes.bfloat16)
            im[f"wt{j}"] = np.ascontiguousarray(
                Wn[j * 128 : (j + 1) * 128, :]
            ).astype(ml_dtypes.bfloat16)
        in_maps.append(im)

    if cfg.get("SIM"):
        from concourse.bass_interp import MultiCoreSim

        sim = MultiCoreSim(nc, num_cores=C)
        for k, core_sim in sim.cores.items():
            for name, val in in_maps[k].items():
                core_sim.tensor(name)[:] = val
        sim.simulate()
        y = np.empty((N, OUT), np.float32)
        for k in range(C):
            y[k * NPC : (k + 1) * NPC] = sim.cores[k].tensor("ypad")[:NPC]
        return y

    global LAST_NC, LAST_RUN_S
    LAST_NC = nc
    import time as _time

    _t0 = _time.time()
    res = bass_utils.run_bass_kernel_spmd(
        nc,
        in_maps,
        core_ids=list(range(C)),
        trace=cfg.get("TRACE", False),
    )
    LAST_RUN_S = _time.time() - _t0
    LAST_RESULTS = res

    y = np.empty((N, OUT), np.float32)
    for k in range(C):
        y[k * NPC : (k + 1) * NPC] = res.results[k]["ypad"][:NPC]
    return y


# revision 33
# speedup vs baseline: 1.0390x; 1.0390x over previous
"""GCN block (GraphConv + BatchNorm1d + ReLU) on 8 Trainium2 NeuronCores.

Strategy (per sharding hint): partition nodes (and incident edges) across the
8 cores; replicate W/b/gamma/beta; all-reduce BN batch statistics.

Per core k (owns dst nodes [k*NPC, (k+1)*NPC)):
  1. h_k = (x_k @ W) * rsqrt(clip(deg_out_k,1))  (PE matmul; x columns are
     host-permuted so even nodes land on partitions 0:64, odd on 64:128).
  2. TWO AllGathers of h (bf16): even nodes, then odd nodes. Gathers that
     read even-sourced edges overlap the second collective. The 4
     int16-indexable "bank" tables are *interleaved strided views* of the
     AG outputs: bank b = nodes with (n % NPC) % 4 == b lives in half b%2
     at row 2*j + b//2, j = owner*(NPCP/4) + (n % NPC)//4 (elem_step=2
     rows). Interleaving makes bank-row -> table-row linear, so two big
     collectives (cheap) serve four int16-indexed gather tables.
  3. For each (bank, dst-group) run of edges (64-slot granular, shared
     run sizes = max over cores), gather h[src] rows (dma_gather, bf16,
     one batched gather per (phase, chunk-of-groups, bank)) and
     segment-sum them with one-hot matmuls M^T @ G accumulated in PSUM.
     Each dst group keeps ONE psum accumulation per phase; the phase-A
     partial spills to bf16 and is re-injected via an identity matmul.
     Runs straddling 128-slot block boundaries get one matmul per
     straddled block; out-of-segment slots carry doff 255 so their
     one-hot column is zero.
  4. relu(psum * rsqrt(clip(deg_in,1)) [+ b]) via ACT directly from PSUM
     (bf16 out); BN sums via ones-matmuls (single accumulation group);
     AllReduce sums; y = h*S + T with S = gamma*rsqrt(var+eps),
     T = beta - mu*S (broadcast-AP DVE ops); y cast bf16->f32 during the
     output DMA (SWDGE), pipelined per 14-group batch.

Host-side work is limited to integer index bookkeeping (bucketing edges by
(core, src-bank, dst-group), degree counting) and layout transforms (x^T
permutation/padding, int16 gather indices). All floating-point math runs on
device.

Run sizes are padded to a structure shared by all 8 cores so a single SPMD
NEFF serves every core; pad slots re-gather the run's last row (HBM page
hit) and carry a dst offset of 255 -> contribute exactly 0. Edges are
sorted by gather row within each bucket for HBM locality.
"""
import math
import os
import sys

sys.path.insert(0, "/opt/trn_rl_repo")

import numpy as np

import concourse.bacc as bacc
import concourse.bass as bass
import concourse.mybir as mybir
import concourse.tile as tile
from concourse import bass_utils

F32 = mybir.dt.float32
BF16 = mybir.dt.bfloat16
I16 = mybir.dt.int16

CFG = dict(
    N=100000,
    E=1600000,
    IN=256,
    OUT=128,
    NCORES=8,
    GRP=128,          # dst nodes per segment group (= psum partition dim)
    NBANKS=4,         # interleaved src banks (bank rows must be < 32768)
    GCHUNK=12,        # groups per chunk (gather batch granularity)
    EPS=1e-5,
    TRACE=False,
)

LAST_RESULTS = None  # set by kernel() for test harness introspection
LAST_NC = None
LAST_RUN_S = None


def _ceil_div(a, b):
    return (a + b - 1) // b


def _wrap16(idx, ncols):
    """int16 idx list -> [128, ncols] tile: idx i at [i%16, i//16], replicated
    8x across the 16-partition groups (one copy per GpSimd Q7 core)."""
    n = idx.shape[0]
    assert n == ncols * 16
    w = np.ascontiguousarray(idx.reshape(ncols, 16).T)
    return np.tile(w, (8, 1))


def _preprocess(cfg, src, dst):
    """Bucket edges by (owner core, interleaved src bank, dst group); build
    per-core gather-index / dst-offset arrays and the shared run structure."""
    N, E = cfg["N"], cfg["E"]
    C, NBANKS, GRP, GC = cfg["NCORES"], cfg["NBANKS"], cfg["GRP"], cfg["GCHUNK"]
    NPC = N // C
    NG = _ceil_div(NPC, GRP)
    NPCP = NG * GRP                # padded nodes per core (x cols zero-padded)
    assert NPCP % NBANKS == 0
    QB = NPCP // NBANKS            # gather rows per owner per bank view
    BANKROWS = QB * C              # rows per bank view of one AG-half output
    assert BANKROWS < 32768

    src = src.astype(np.int64)
    dst = dst.astype(np.int64)
    deg_out = np.bincount(src, minlength=N).astype(np.float32)
    deg_in = np.bincount(dst, minlength=N).astype(np.float32)

    owner = dst // NPC
    loc = src % NPC
    bank = loc % NBANKS            # interleaved bank of the source
    grow = (src // NPC) * QB + loc // NBANKS   # gather row within bank view
    assert grow.max() < 32768
    grp = (dst % NPC) // GRP
    key = (owner * NBANKS + bank) * NG + grp
    # sort by bucket, then by gather row inside the bucket (HBM locality)
    order = np.lexsort((grow, key))
    s_grow = grow[order]
    s_dst = dst[order]
    s_key = key[order]

    counts = np.bincount(key, minlength=C * NBANKS * NG).reshape(C, NBANKS, NG)
    P = counts.max(axis=0)  # [NBANKS, NG] shared run sizes (64-granular)
    P = ((P + 63) // 64) * 64
    P = np.maximum(P, 64)   # every (b,g) run structurally exists

    # two phases: banks {0,2} (even AG half), then {1,3} (odd half)
    phases = [(0, 2), (1, 3)]
    chunks = [list(range(c, min(c + GC, NG))) for c in range(0, NG, GC)]
    run_seq = [
        (b, g) for ph in phases for ch in chunks for b in ph for g in ch
    ]
    # lay out runs; pad each (phase, chunk, bank) unit to a 128 multiple
    run_off = np.zeros((NBANKS, NG), np.int64)
    units = []  # (bank, first_block, n_blocks) in stream order
    pos = 0
    for ph in phases:
        for ch in chunks:
            for b in ph:
                u0 = pos
                for g in ch:
                    run_off[b, g] = pos
                    pos += P[b, g]
                pos = ((pos + 127) // 128) * 128  # unit pad
                units.append((b, u0 // 128, (pos - u0) // 128))
    nidx_tot = int(pos)
    nb_tot = nidx_tot // 128

    # segments: a run may straddle block boundaries; each (run, block)
    # intersection is one segment = one doff column + one full matmul
    # (out-of-segment slots carry doff 255 -> zero one-hot column).
    run_segs = {}  # (b, g) -> list of (block_t, doff_col, slot_lo, slot_hi)
    nseg = 0
    for b, g in run_seq:
        off = int(run_off[b, g])
        end = off + int(P[b, g])
        segs = []
        t = off // 128
        while t * 128 < end:
            lo = max(off, t * 128)
            hi = min(end, (t + 1) * 128)
            segs.append((t, nseg, lo, hi))
            nseg += 1
            t += 1
        run_segs[(b, g)] = segs

    # boundaries of each (k, b, g) bucket in the sorted edge stream
    bkeys = (
        np.arange(C)[:, None, None] * NBANKS + np.arange(NBANKS)[None, :, None]
    ) * NG + np.arange(NG)[None, None, :]
    starts = np.searchsorted(s_key, bkeys.ravel()).reshape(C, NBANKS, NG)
    ends = np.searchsorted(s_key, bkeys.ravel(), side="right").reshape(C, NBANKS, NG)

    gidx_cores = []
    dstoff_cores = []
    for k in range(C):
        gidx = np.zeros(nidx_tot, np.int16)
        doff_cols = np.full((nseg, 128), 255.0, np.float32)
        for b in range(NBANKS):
            for g in range(NG):
                s, e = starts[k, b, g], ends[k, b, g]
                cnt = e - s
                p0 = int(run_off[b, g])
                if cnt:
                    gidx[p0 : p0 + cnt] = s_grow[s:e].astype(np.int16)
                    # pad slots re-gather the last row (HBM page hit)
                    gidx[p0 + cnt : p0 + int(P[b, g])] = gidx[p0 + cnt - 1]
                    offs = ((s_dst[s:e] % NPC) - g * GRP).astype(np.float32)
                    for t, col, lo, hi in run_segs[(b, g)]:
                        a = max(lo, p0)
                        z = min(hi, p0 + cnt)
                        if z > a:
                            doff_cols[col, a - t * 128 : z - t * 128] = offs[
                                a - p0 : z - p0
                            ]
        # unit-pad slots gather row 0 (gidx stays 0) and have no segment
        gidx_cores.append(_wrap16(gidx, nidx_tot // 16))
        dstoff_cores.append(np.ascontiguousarray(doff_cols.T))

    meta = dict(
        NPC=NPC,
        NPCP=NPCP,
        NG=NG,
        QB=QB,
        BANKROWS=BANKROWS,
        nidx_tot=nidx_tot,
        nb_tot=nb_tot,
        nseg=nseg,
        run_segs=run_segs,
        units=units,
        chunks=chunks,
        run_seq=run_seq,
        deg_out=deg_out,
        deg_in=deg_in,
    )
    return meta, gidx_cores, dstoff_cores


def _tile_major(vec, NG, GRP, pad_val):
    """[NPC] -> [GRP, NG]: entry (p, m) = vec[m*GRP + p], padded."""
    out = np.full((NG * GRP,), pad_val, vec.dtype)
    out[: vec.shape[0]] = vec
    return np.ascontiguousarray(out.reshape(NG, GRP).T)


def _build_nc(cfg, meta, b_nonzero=False):
    N, IN, OUT, C = cfg["N"], cfg["IN"], cfg["OUT"], cfg["NCORES"]
    GRP, NBANKS = cfg["GRP"], cfg["NBANKS"]
    NPC, NPCP, NG = meta["NPC"], meta["NPCP"], meta["NG"]
    nidx_tot, nb_tot = meta["nidx_tot"], meta["nb_tot"]
    units = meta["units"]
    XK = _ceil_div(IN, 128)
    assert OUT == 128 and GRP == 128
    last_w = NPC - (NG - 1) * GRP  # valid rows in the last group
    HALF = NPCP // 2               # rows per AG-half input

    nc = bacc.Bacc(
        "TRN2", target_bir_lowering=False, debug=False, num_devices=C
    )

    # ---- external inputs ----
    NXQ = 4  # x DMA split for earlier stage-B start
    xq = NPCP // NXQ
    assert NPCP % NXQ == 0
    xt = [
        nc.dram_tensor(f"xt{j}", [128, NPCP], BF16, kind="ExternalInput")
        for j in range(XK)
    ]
    wt = [
        nc.dram_tensor(f"wt{j}", [128, OUT], BF16, kind="ExternalInput")
        for j in range(XK)
    ]
    gidx_d = nc.dram_tensor("gidx", [128, nidx_tot // 16], I16, kind="ExternalInput")
    doff_d = nc.dram_tensor("doff", [128, meta["nseg"]], F32, kind="ExternalInput")
    dego_d = nc.dram_tensor("dego", [128, NG], F32, kind="ExternalInput")
    degi_d = nc.dram_tensor("degi", [128, NG], F32, kind="ExternalInput")
    iota_d = nc.dram_tensor("iota", [128, GRP], BF16, kind="ExternalInput")
    gm_d = nc.dram_tensor("gm", [1, OUT], F32, kind="ExternalInput")
    bb_d = nc.dram_tensor("bb", [1, OUT], F32, kind="ExternalInput")
    onesc_d = nc.dram_tensor("onesc", [128, 1], BF16, kind="ExternalInput")
    onest_d = nc.dram_tensor("onest", [128, 1], BF16, kind="ExternalInput")
    onesr_d = nc.dram_tensor("onesr", [1, 128], F32, kind="ExternalInput")
    ident_d = nc.dram_tensor("ident", [128, 128], BF16, kind="ExternalInput")
    if b_nonzero:
        bt_d = nc.dram_tensor("bt", [1, OUT], F32, kind="ExternalInput")

    ypad_d = nc.dram_tensor("ypad", [NG * GRP, OUT], F32, kind="ExternalOutput")

    with tile.TileContext(nc) as tc:
        with (
            tc.tile_pool(name="const", bufs=1) as cpool,
            tc.tile_pool(name="dram", bufs=1, space="DRAM") as dpool,
            tc.tile_pool(name="agg", bufs=1) as apool,
            tc.tile_pool(name="mpool", bufs=16) as mpool,
            tc.tile_pool(name="etmp", bufs=4) as epool,
            tc.tile_pool(name="psg", bufs=4, space="PSUM") as pgpool,
            tc.tile_pool(name="psb", bufs=2, space="PSUM") as pbpool,
            tc.tile_pool(name="pstat", bufs=1, space="PSUM") as pspool,
        ):
            # ---- constants / small tiles ----
            iota_t = cpool.tile([128, GRP], BF16)
            dego_t = cpool.tile([128, NG], F32)
            degi_t = cpool.tile([128, NG], F32)
            nsrc_t = cpool.tile([128, NG], F32)
            ndst_t = cpool.tile([128, NG], F32)
            gm_t = cpool.tile([1, OUT], F32)
            bb_t = cpool.tile([1, OUT], F32)
            onesc_t = cpool.tile([128, 1], BF16)
            onest_t = cpool.tile([128, 1], BF16)
            onesr_t = cpool.tile([1, 128], F32)
            gidx_t = cpool.tile([128, nidx_tot // 16], I16)
            doff_t = cpool.tile([128, meta["nseg"]], F32)
            ident_t = cpool.tile([128, 128], BF16)
            nc.sync.dma_start(ident_t[:], ident_d[:])

            nc.sync.dma_start(iota_t[:], iota_d[:])
            nc.sync.dma_start(dego_t[:], dego_d[:])
            nc.sync.dma_start(degi_t[:], degi_d[:])
            nc.sync.dma_start(gm_t[:], gm_d[:])
            nc.sync.dma_start(bb_t[:], bb_d[:])
            nc.sync.dma_start(onesc_t[:], onesc_d[:])
            nc.sync.dma_start(onest_t[:], onest_d[:])
            nc.sync.dma_start(onesr_t[:], onesr_d[:])
            if b_nonzero:
                bt_t = cpool.tile([1, OUT], F32)
                nc.sync.dma_start(bt_t[:], bt_d[:])

            # norms: rsqrt(max(deg, 1))
            for deg_t, norm_t in ((dego_t, nsrc_t), (degi_t, ndst_t)):
                nc.vector.tensor_scalar(
                    norm_t[:], deg_t[:], 1.0, None, op0=mybir.AluOpType.max
                )
                nc.vector.reciprocal(norm_t[:], norm_t[:])
                nc.scalar.activation(
                    norm_t[:], norm_t[:], mybir.ActivationFunctionType.Sqrt
                )

            # internal DRAM for collectives (even/odd node halves)
            _aspace = "Local" if cfg.get("NOCC") else "Shared"
            h_my_e = dpool.tile([HALF, OUT], BF16, name="h_my_e")
            h_my_o = dpool.tile([HALF, OUT], BF16, name="h_my_o")
            h_all_e = dpool.tile(
                [C * HALF, OUT], BF16, addr_space=_aspace, name="h_all_e"
            )
            h_all_o = dpool.tile(
                [C * HALF, OUT], BF16, addr_space=_aspace, name="h_all_o"
            )
            stats_in = dpool.tile([1, 2 * OUT], F32)
            stats_out = dpool.tile([C, 2 * OUT], F32, addr_space=_aspace)

            # relu(norm*agg) output, bf16, [128, NG, OUT]
            agg_t = apool.tile([128, NG, OUT], BF16)

            # ---- stage B: h = (x @ W) * norm_src, cast bf16, store to HBM
            # (staged in SBUF; 2 large DMAs instead of 98 small ones)
            with tc.tile_pool(name="xw", bufs=1) as xwp:
                xts = []
                wts = []
                for j in range(XK):
                    xts.append(xwp.tile([128, NPCP], BF16, name=f"xt_s{j}"))
                    wts.append(xwp.tile([128, OUT], BF16, name=f"wt_s{j}"))
                for j in range(XK):
                    nc.sync.dma_start(wts[j][:], wt[j][:])
                for q in range(NXQ):
                    for j in range(XK):
                        nc.sync.dma_start(
                            xts[j][:, q * xq : (q + 1) * xq],
                            xt[j][:, q * xq : (q + 1) * xq],
                        )
                hstage = xwp.tile([128, NG, OUT], BF16, name="hstage")
                for m in range(NG):
                    ps = pbpool.tile([128, OUT], F32, tag="hps")
                    for j in range(XK):
                        nc.tensor.matmul(
                            ps[:, :],
                            xts[j][:, m * GRP : (m + 1) * GRP],
                            wts[j][:, :],
                            start=(j == 0),
                            stop=(j == XK - 1),
                        )
                    if m % 2 == 0:
                        nc.scalar.activation(
                            hstage[:, m, :],
                            ps[:, :],
                            mybir.ActivationFunctionType.Copy,
                            scale=nsrc_t[:, m : m + 1],
                        )
                    else:
                        nc.vector.tensor_scalar(
                            hstage[:, m, :],
                            ps[:, :],
                            nsrc_t[:, m : m + 1],
                            None,
                            op0=mybir.AluOpType.mult,
                        )
                # partitions 0:64 = even nodes of each group (loc = g*128+2q),
                # 64:128 = odd (x columns are host-permuted to match) ->
                # h_my_e row g*64+q = node loc 2r exactly
                hq = NG // 8
                for q in range(8):
                    a = q * hq
                    z = (q + 1) * hq if q < 7 else NG
                    nc.sync.dma_start(
                        h_my_e[a * 64 : z * 64, :].rearrange(
                            "(g p) f -> p g f", p=64
                        ),
                        hstage[0:64, a:z, :],
                    )
                    nc.sync.dma_start(
                        h_my_o[a * 64 : z * 64, :].rearrange(
                            "(g p) f -> p g f", p=64
                        ),
                        hstage[64:128, a:z, :],
                    )

            # ---- stage C: two AllGathers (even half, then odd half) ----
            for h_my_h, h_all_h in ((h_my_e, h_all_e), (h_my_o, h_all_o)):
                if cfg.get("NOCC"):
                    rep = (
                        h_my_h[:]
                        .rearrange("(o r) f -> o r f", o=1)
                        .to_broadcast((C, HALF, OUT))
                    )
                    nc.sync.dma_start(
                        h_all_h[:].rearrange("(o r) f -> o r f", o=C), rep
                    )
                else:
                    nc.gpsimd.collective_compute(
                        "AllGather",
                        mybir.AluOpType.bypass,
                        replica_groups=[list(range(C))],
                        ins=[h_my_h[:]],
                        outs=[h_all_h[:]],
                    )

            # index tables are first needed by stage D's gathers - load
            # them after the x/B/AG chain is underway
            nc.sync.dma_start(gidx_t[:], gidx_d[:])
            nc.sync.dma_start(doff_t[:], doff_d[:])

            # interleaved bank views: bank b -> half b%2, row 2j + b//2
            h_banks = [
                (h_all_e if b % 2 == 0 else h_all_o)[:]
                .rearrange("(j k) f -> j (k f)", k=2)[
                    :, (b // 2) * OUT : (b // 2 + 1) * OUT
                ]
                for b in range(NBANKS)
            ]

            # ---- stage D: gather + one-hot matmul segmented sum ----
            # ---- stage E (inline): relu(psum*ndst) + BN partial sums ----
            # Gathers are batched per (chunk, bank); groups are processed
            # sequentially (their 4 bank runs back-to-back) so each PSUM bank
            # holds at most one pending accumulation group.
            ps_stat = pspool.tile([1, 2 * OUT], F32, name="ps_stat")
            ps_sum = ps_stat[:, 0:OUT]
            ps_sq = ps_stat[:, OUT : 2 * OUT]
            ndone = [0]  # groups completed (for BN-sum start/stop flags)

            def finish_group(g, ps_g):
                """relu + BN-sum accumulation for a completed group psum."""
                if b_nonzero:
                    tmp = epool.tile([128, OUT], F32, tag="etmp")
                    nc.vector.scalar_tensor_tensor(
                        tmp[:],
                        ps_g[:],
                        ndst_t[:, g : g + 1],
                        btile_t[:],
                        op0=mybir.AluOpType.mult,
                        op1=mybir.AluOpType.add,
                    )
                    nc.scalar.activation(
                        agg_t[:, g, :], tmp[:], mybir.ActivationFunctionType.Relu
                    )
                else:
                    nc.scalar.activation(
                        agg_t[:, g, :],
                        ps_g[:],
                        mybir.ActivationFunctionType.Relu,
                        scale=ndst_t[:, g : g + 1],
                    )
                ones = onesc_t if g < NG - 1 else onest_t
                i0 = ndone[0]
                # ps_sum/ps_sq share one bank = ONE accumulation group:
                # start only on the very first matmul, stop on the very last.
                nc.tensor.matmul(
                    ps_sum,
                    ones[:],
                    agg_t[:, g, :],
                    start=(i0 == 0),
                    stop=False,
                )
                sq = epool.tile([128, OUT], BF16, tag="esq")
                nc.scalar.activation(
                    sq[:], agg_t[:, g, :], mybir.ActivationFunctionType.Square
                )
                nc.tensor.matmul(
                    ps_sq,
                    ones[:],
                    sq[:],
                    start=False,
                    stop=(i0 == NG - 1),
                )
                ndone[0] += 1

            if b_nonzero:
                # replicate b across partitions once (PE broadcast)
                ps_b = pbpool.tile([128, OUT], F32, tag="hps", name="ps_b")
                btile_t = cpool.tile([128, OUT], F32)
                nc.tensor.matmul(ps_b[:], onesr_t[:], bt_t[:], start=True, stop=True)
                nc.scalar.activation(
                    btile_t[:], ps_b[:], mybir.ActivationFunctionType.Copy
                )

            run_segs = meta["run_segs"]
            chunks = meta["chunks"]
            nbmax = max(nb for _, _, nb in units)
            dstack = tc.tile_pool(name="gath", bufs=6)
            gpool = dstack.__enter__()
            phases = [(0, 2), (1, 3)]
            ui = 0
            for pi, ph in enumerate(phases):
                for ci, ch in enumerate(chunks):
                    gts = {}
                    for b in ph:
                        bank, t0, nblk = units[ui]
                        ui += 1
                        assert bank == b
                        Gt = gpool.tile(
                            [128, nbmax, OUT], BF16, tag="G", name=f"G{pi}_{ci}_{b}"
                        )
                        nc.gpsimd.dma_gather(
                            Gt[:, :nblk, :],
                            h_banks[b],
                            gidx_t[:, t0 * 8 : (t0 + nblk) * 8],
                            nblk * 128,
                            nblk * 128,
                            OUT,
                            elem_step=2 * OUT,
                            single_packet=False,
                        )
                        gts[b] = (Gt, t0)
                    for g in ch:
                        ps_g = pgpool.tile(
                            [128, OUT], F32, tag="aggps", name=f"ps{pi}_{g}"
                        )
                        if pi == 1:
                            # re-inject phase-A partial (spilled bf16)
                            nc.tensor.matmul(
                                ps_g[:],
                                ident_t[:],
                                agg_t[:, g, :],
                                start=True,
                                stop=False,
                            )
                        for bi, b in enumerate(ph):
                            Gt, t0 = gts[b]
                            segs = run_segs[(b, g)]
                            for si, (t, col, lo, hi) in enumerate(segs):
                                Mt = mpool.tile([128, GRP], BF16, tag="M")
                                nc.vector.tensor_scalar(
                                    Mt[:],
                                    iota_t[:],
                                    doff_t[:, col : col + 1],
                                    None,
                                    op0=mybir.AluOpType.is_equal,
                                )
                                nc.tensor.matmul(
                                    ps_g[:],
                                    Mt[:],
                                    Gt[:, t - t0, :],
                                    start=(pi == 0 and bi == 0 and si == 0),
                                    stop=(bi == 1 and si == len(segs) - 1),
                                )
                        if pi == 0:
                            # spill partial sum to agg_t (bf16), no relu yet
                            nc.scalar.activation(
                                agg_t[:, g, :],
                                ps_g[:],
                                mybir.ActivationFunctionType.Copy,
                            )
                        else:
                            finish_group(g, ps_g)
            dstack.__exit__(None, None, None)
            assert ndone[0] == NG

            # ---- stage F: AllReduce BN stats; build affine S/T tiles ----
            st_sb = cpool.tile([1, 2 * OUT], F32)
            nc.scalar.activation(
                st_sb[:, 0:OUT], ps_sum, mybir.ActivationFunctionType.Copy
            )
            nc.scalar.activation(
                st_sb[:, OUT : 2 * OUT], ps_sq, mybir.ActivationFunctionType.Copy
            )
            nc.sync.dma_start(stats_in[:], st_sb[:])
            if cfg.get("NOCC"):
                rep = (
                    stats_in[:]
                    .rearrange("(o r) f -> o r f", o=1)
                    .to_broadcast((C, 1, 2 * OUT))
                )
                nc.sync.dma_start(
                    stats_out[:].rearrange("(o r) f -> o r f", o=C), rep
                )
            else:
                nc.gpsimd.collective_compute(
                    "AllGather",
                    mybir.AluOpType.bypass,
                    replica_groups=[list(range(C))],
                    ins=[stats_in[:]],
                    outs=[stats_out[:]],
                )
            # per-core partials land row-major; sum the C rows locally
            st_all = cpool.tile([1, C * 2 * OUT], F32)
            nc.sync.dma_start(
                st_all[:], stats_out[:].rearrange("(o c) f -> o (c f)", o=1)
            )
            st_rb = cpool.tile([1, 2 * OUT], F32)
            nc.vector.tensor_add(
                st_rb[:], st_all[:, 0 : 2 * OUT], st_all[:, 2 * OUT : 4 * OUT]
            )
            for c_ in range(2, C):
                nc.vector.tensor_add(
                    st_rb[:],
                    st_rb[:],
                    st_all[:, c_ * 2 * OUT : (c_ + 1) * 2 * OUT],
                )

            mu = cpool.tile([1, OUT], F32)
            ex2 = cpool.tile([1, OUT], F32)
            var = cpool.tile([1, OUT], F32)
            srow = cpool.tile([1, OUT], F32)
            trow = cpool.tile([1, OUT], F32)
            inv_n = 1.0 / float(N)
            nc.scalar.activation(
                mu[:], st_rb[:, 0:OUT], mybir.ActivationFunctionType.Copy, scale=inv_n
            )
            nc.scalar.activation(
                ex2[:],
                st_rb[:, OUT : 2 * OUT],
                mybir.ActivationFunctionType.Copy,
                scale=inv_n,
            )
            nc.scalar.activation(var[:], mu[:], mybir.ActivationFunctionType.Square)
            nc.vector.tensor_sub(var[:], ex2[:], var[:])
            # var <- rsqrt(var + eps) (ACT Rsqrt is banned for accuracy)
            nc.scalar.activation(
                var[:],
                var[:],
                mybir.ActivationFunctionType.Copy,
                bias=float(cfg["EPS"]),
            )
            nc.vector.reciprocal(var[:], var[:])
            nc.scalar.activation(var[:], var[:], mybir.ActivationFunctionType.Sqrt)
            nc.vector.tensor_mul(srow[:], gm_t[:], var[:])
            nc.vector.tensor_mul(trow[:], mu[:], srow[:])
            nc.vector.tensor_sub(trow[:], bb_t[:], trow[:])

            S_t = cpool.tile([128, OUT], BF16)
            T_t = cpool.tile([128, OUT], BF16)
            ps_S = pgpool.tile([128, OUT], F32, tag="aggps", name="ps_S")
            ps_T = pgpool.tile([128, OUT], F32, tag="aggps", name="ps_T")
            nc.tensor.matmul(ps_S[:], onesr_t[:], srow[:], start=True, stop=True)
            nc.tensor.matmul(ps_T[:], onesr_t[:], trow[:], start=True, stop=True)
            nc.scalar.activation(S_t[:], ps_S[:], mybir.ActivationFunctionType.Copy)
            nc.scalar.activation(T_t[:], ps_T[:], mybir.ActivationFunctionType.Copy)

            # ---- stage G: y = hrelu * S + T (bf16), cast f32 on DMA out ----
            with tc.tile_pool(name="gtmp", bufs=2) as gpool2:
                GB = 14  # groups per batched op
                S_bc = (
                    S_t[:]
                    .rearrange("p (o f) -> p o f", o=1)
                    .to_broadcast((128, GB, OUT))
                )
                T_bc = (
                    T_t[:]
                    .rearrange("p (o f) -> p o f", o=1)
                    .to_broadcast((128, GB, OUT))
                )
                ypad_view = ypad_d[:].rearrange("(g p) f -> p g f", p=128)
                for g0 in range(0, NG, GB):
                    gw = min(GB, NG - g0)
                    tmp = gpool2.tile([128, GB, OUT], BF16, tag="gtmp")
                    nc.vector.tensor_mul(
                        tmp[:, :gw, :],
                        agg_t[:, g0 : g0 + gw, :],
                        S_bc if gw == GB else S_t[:]
                        .rearrange("p (o f) -> p o f", o=1)
                        .to_broadcast((128, gw, OUT)),
                    )
                    nc.vector.tensor_add(
                        agg_t[:, g0 : g0 + gw, :],
                        tmp[:, :gw, :],
                        T_bc if gw == GB else T_t[:]
                        .rearrange("p (o f) -> p o f", o=1)
                        .to_broadcast((128, gw, OUT)),
                    )
                    nc.gpsimd.dma_start(
                        ypad_view[:, g0 : g0 + gw, :],
                        agg_t[:, g0 : g0 + gw, :],
                    )

    nc.compile()
    return nc


def kernel(x, src, dst, W, b, gamma, beta):
    global LAST_RESULTS
    cfg = CFG
    N, E, IN, OUT, C = cfg["N"], cfg["E"], cfg["IN"], cfg["OUT"], cfg["NCORES"]
    GRP = cfg["GRP"]
    assert x.shape == (N, IN) and W.shape == (IN, OUT)
    assert src.shape == (E,) and dst.shape == (E,)

    b = np.asarray(b, np.float32)
    b_nonzero = bool(np.any(b != 0.0))
    meta, gidx_cores, dstoff_cores = _preprocess(cfg, src, dst)
    NPC, NPCP, NG = meta["NPC"], meta["NPCP"], meta["NG"]
    XK = _ceil_div(IN, 128)
    last_w = NPC - (NG - 1) * GRP
    # node permutation: within each 128-node group, evens first then odds
    perm = np.concatenate([np.arange(0, 128, 2), np.arange(1, 128, 2)])
    g_ = np.arange(NPCP) // 128
    p_ = np.arange(NPCP) % 128
    permn = g_ * 128 + perm[p_]          # source node (local) per padded col
    valid = permn < NPC

    nc = _build_nc(cfg, meta, b_nonzero=b_nonzero)

    xT = np.ascontiguousarray(np.asarray(x, np.float32).T)  # [IN, N]
    Wn = np.asarray(W, np.float32)
    import ml_dtypes

    iota = np.tile(np.arange(GRP, dtype=np.float32)[None, :], (128, 1)).astype(
        ml_dtypes.bfloat16
    )
    onesc = np.ones((128, 1), np.float32)
    onest = np.zeros((128, 1), np.float32)
    onest[:last_w] = 1.0
    onesr = np.ones((1, 128), np.float32)
    gm = np.asarray(gamma, np.float32)[None, :]
    bb = np.asarray(beta, np.float32)[None, :]

    in_maps = []
    for k in range(C):
        im = {
            "gidx": gidx_cores[k],
            "doff": dstoff_cores[k],
            "dego": _tile_major(
                np.where(
                    valid,
                    meta["deg_out"][k * NPC + np.minimum(permn, NPC - 1)],
                    np.float32(1.0),
                ).astype(np.float32),
                NG,
                GRP,
                np.float32(1.0),
            ),
            "degi": _tile_major(
                meta["deg_in"][k * NPC : (k + 1) * NPC], NG, GRP, np.float32(1.0)
            ),
            "iota": iota,
            "gm": gm,
            "bb": bb,
            "onesc": onesc.astype(ml_dtypes.bfloat16),
            "onest": onest.astype(ml_dtypes.bfloat16),
            "onesr": onesr,
            "ident": np.eye(128, dtype=np.float32).astype(ml_dtypes.bfloat16),
        }
        if b_nonzero:
            im["bt"] = b[None, :]
        for j in range(XK):
            xcols = np.zeros((128, NPCP), np.float32)
            xcols[:, valid] = xT[
                j * 128 : (j + 1) * 128, k * NPC + permn[valid]
            ]
            im[f"xt{j}"] = xcols.astype(ml_dtypes.bfloat16)
            im[f"wt{j}"] = np.ascontiguousarray(
                Wn[j * 128 : (j + 1) * 128, :]
            ).astype(ml_dtypes.bfloat16)
        in_maps.append(im)

    if cfg.get("SIM"):
        from concourse.bass_interp import MultiCoreSim

        sim = MultiCoreSim(nc, num_cores=C)
        for k, core_sim in sim.cores.items():
            for name, val in in_maps[k].items():
                core_sim.tensor(name)[:] = val
        sim.simulate()
        y = np.empty((N, OUT), np.float32)
        for k in range(C):
            y[k * NPC : (k + 1) * NPC] = sim.cores[k].tensor("ypad")[:NPC]
        return y

    global LAST_NC, LAST_RUN_S
    LAST_NC = nc
    import time as _time

    _t0 = _time.time()
    res = bass_utils.run_bass_kernel_spmd(
        nc,
        in_maps,
        core_ids=list(range(C)),
        trace=cfg.get("TRACE", False),
    )
    LAST_RUN_S = _time.time() - _t0
    LAST_RESULTS = res

    y = np.empty((N, OUT), np.float32)
    for k in range(C):
        y[k * NPC : (k + 1) * NPC] = res.results[k]["ypad"][:NPC]
    return y


# revision 35
# speedup vs baseline: 1.1175x; 1.0755x over previous
"""GCN block (GraphConv + BatchNorm1d + ReLU) on 8 Trainium2 NeuronCores.

Strategy (per sharding hint): partition nodes (and incident edges) across the
8 cores; replicate W/b/gamma/beta; all-reduce BN batch statistics.

Per core k (owns dst nodes [k*NPC, (k+1)*NPC)):
  1. h_k = (x_k @ W) * rsqrt(clip(deg_out_k,1))  (PE matmul; x columns are
     host-permuted so even nodes land on partitions 0:64, odd on 64:128).
  2. TWO AllGathers of h (bf16): even nodes, then odd nodes. Gathers that
     read even-sourced edges overlap the second collective. The 4
     int16-indexable "bank" tables are *interleaved strided views* of the
     AG outputs: bank b = nodes with (n % NPC) % 4 == b lives in half b%2
     at row 2*j + b//2, j = owner*(NPCP/4) + (n % NPC)//4 (elem_step=2
     rows). Interleaving makes bank-row -> table-row linear, so two big
     collectives (cheap) serve four int16-indexed gather tables.
  3. For each (bank, dst-group) run of edges (64-slot granular, shared
     run sizes = max over cores), gather h[src] rows (dma_gather, bf16,
     one batched gather per (phase, chunk-of-groups, bank)) and
     segment-sum them with one-hot matmuls M^T @ G accumulated in PSUM.
     Each dst group keeps ONE psum accumulation per phase; the phase-A
     partial spills to bf16 and is re-injected via an identity matmul.
     Runs straddling 128-slot block boundaries get one matmul per
     straddled block; out-of-segment slots carry doff 255 so their
     one-hot column is zero.
  4. relu(psum * rsqrt(clip(deg_in,1)) [+ b]) via ACT directly from PSUM
     (bf16 out); BN sums via ones-matmuls (single accumulation group);
     AllReduce sums; y = h*S + T with S = gamma*rsqrt(var+eps),
     T = beta - mu*S (broadcast-AP DVE ops); y cast bf16->f32 during the
     output DMA (SWDGE), pipelined per 14-group batch.

Host-side work is limited to integer index bookkeeping (bucketing edges by
(core, src-bank, dst-group), degree counting) and layout transforms (x^T
permutation/padding, int16 gather indices). All floating-point math runs on
device.

Run sizes are padded to a structure shared by all 8 cores so a single SPMD
NEFF serves every core; pad slots re-gather the run's last row (HBM page
hit) and carry a dst offset of 255 -> contribute exactly 0. Edges are
sorted by gather row within each bucket for HBM locality.
"""
import math
import os
import sys

sys.path.insert(0, "/opt/trn_rl_repo")

import numpy as np

import concourse.bacc as bacc
import concourse.bass as bass
import concourse.mybir as mybir
import concourse.tile as tile
from concourse import bass_utils

F32 = mybir.dt.float32
BF16 = mybir.dt.bfloat16
I16 = mybir.dt.int16

CFG = dict(
    N=100000,
    E=1600000,
    IN=256,
    OUT=128,
    NCORES=8,
    GRP=128,          # dst nodes per segment group (= psum partition dim)
    NBANKS=4,         # interleaved src banks (bank rows must be < 32768)
    GCHUNK=12,        # groups per chunk (gather batch granularity)
    EPS=1e-5,
    TRACE=False,
)

LAST_RESULTS = None  # set by kernel() for test harness introspection
LAST_NC = None
LAST_RUN_S = None


def _ceil_div(a, b):
    return (a + b - 1) // b


def _wrap16(idx, ncols):
    """int16 idx list -> [128, ncols] tile: idx i at [i%16, i//16], replicated
    8x across the 16-partition groups (one copy per GpSimd Q7 core)."""
    n = idx.shape[0]
    assert n == ncols * 16
    w = np.ascontiguousarray(idx.reshape(ncols, 16).T)
    return np.tile(w, (8, 1))


def _preprocess(cfg, src, dst):
    """Bucket edges by (owner core, interleaved src bank, dst group); build
    per-core gather-index / dst-offset arrays and the shared run structure."""
    N, E = cfg["N"], cfg["E"]
    C, NBANKS, GRP, GC = cfg["NCORES"], cfg["NBANKS"], cfg["GRP"], cfg["GCHUNK"]
    NPC = N // C
    NG = _ceil_div(NPC, GRP)
    NPCP = NG * GRP                # padded nodes per core (x cols zero-padded)
    assert NPCP % NBANKS == 0
    QB = NPCP // NBANKS            # gather rows per owner per bank view
    BANKROWS = QB * C              # rows per bank view of one AG-half output
    assert BANKROWS < 32768

    src = src.astype(np.int64)
    dst = dst.astype(np.int64)
    deg_out = np.bincount(src, minlength=N).astype(np.float32)
    deg_in = np.bincount(dst, minlength=N).astype(np.float32)

    owner = dst // NPC
    loc = src % NPC
    bank = loc % NBANKS            # interleaved bank of the source
    grow = (src // NPC) * QB + loc // NBANKS   # gather row within bank view
    assert grow.max() < 32768
    grp = (dst % NPC) // GRP
    key = (owner * NBANKS + bank) * NG + grp
    # sort by bucket, then by gather row inside the bucket (HBM locality)
    order = np.lexsort((grow, key))
    s_grow = grow[order]
    s_dst = dst[order]
    s_key = key[order]

    counts = np.bincount(key, minlength=C * NBANKS * NG).reshape(C, NBANKS, NG)
    P = counts.max(axis=0)  # [NBANKS, NG] shared run sizes (64-granular)
    P = ((P + 63) // 64) * 64
    P = np.maximum(P, 64)   # every (b,g) run structurally exists

    # two phases: banks {0,1,2} (3/4 AG part), then {3} (1/4 part)
    phases = [(0, 1, 2), (3,)]
    chunks = [list(range(c, min(c + GC, NG))) for c in range(0, NG, GC)]
    run_seq = [
        (b, g) for ph in phases for ch in chunks for b in ph for g in ch
    ]
    # lay out runs; pad each (phase, chunk, bank) unit to a 128 multiple
    run_off = np.zeros((NBANKS, NG), np.int64)
    units = []  # (bank, first_block, n_blocks) in stream order
    pos = 0
    for ph in phases:
        for ch in chunks:
            for b in ph:
                u0 = pos
                for g in ch:
                    run_off[b, g] = pos
                    pos += P[b, g]
                pos = ((pos + 127) // 128) * 128  # unit pad
                units.append((b, u0 // 128, (pos - u0) // 128))
    nidx_tot = int(pos)
    nb_tot = nidx_tot // 128

    # segments: a run may straddle block boundaries; each (run, block)
    # intersection is one segment = one doff column + one full matmul
    # (out-of-segment slots carry doff 255 -> zero one-hot column).
    run_segs = {}  # (b, g) -> list of (block_t, doff_col, slot_lo, slot_hi)
    nseg = 0
    for b, g in run_seq:
        off = int(run_off[b, g])
        end = off + int(P[b, g])
        segs = []
        t = off // 128
        while t * 128 < end:
            lo = max(off, t * 128)
            hi = min(end, (t + 1) * 128)
            segs.append((t, nseg, lo, hi))
            nseg += 1
            t += 1
        run_segs[(b, g)] = segs

    # boundaries of each (k, b, g) bucket in the sorted edge stream
    bkeys = (
        np.arange(C)[:, None, None] * NBANKS + np.arange(NBANKS)[None, :, None]
    ) * NG + np.arange(NG)[None, None, :]
    starts = np.searchsorted(s_key, bkeys.ravel()).reshape(C, NBANKS, NG)
    ends = np.searchsorted(s_key, bkeys.ravel(), side="right").reshape(C, NBANKS, NG)

    gidx_cores = []
    dstoff_cores = []
    for k in range(C):
        gidx = np.zeros(nidx_tot, np.int16)
        doff_cols = np.full((nseg, 128), 255.0, np.float32)
        for b in range(NBANKS):
            for g in range(NG):
                s, e = starts[k, b, g], ends[k, b, g]
                cnt = e - s
                p0 = int(run_off[b, g])
                if cnt:
                    gidx[p0 : p0 + cnt] = s_grow[s:e].astype(np.int16)
                    # pad slots re-gather the last row (HBM page hit)
                    gidx[p0 + cnt : p0 + int(P[b, g])] = gidx[p0 + cnt - 1]
                    offs = ((s_dst[s:e] % NPC) - g * GRP).astype(np.float32)
                    for t, col, lo, hi in run_segs[(b, g)]:
                        a = max(lo, p0)
                        z = min(hi, p0 + cnt)
                        if z > a:
                            doff_cols[col, a - t * 128 : z - t * 128] = offs[
                                a - p0 : z - p0
                            ]
        # unit-pad slots gather row 0 (gidx stays 0) and have no segment
        gidx_cores.append(_wrap16(gidx, nidx_tot // 16))
        dstoff_cores.append(np.ascontiguousarray(doff_cols.T))

    meta = dict(
        NPC=NPC,
        NPCP=NPCP,
        NG=NG,
        QB=QB,
        BANKROWS=BANKROWS,
        nidx_tot=nidx_tot,
        nb_tot=nb_tot,
        nseg=nseg,
        run_segs=run_segs,
        units=units,
        chunks=chunks,
        run_seq=run_seq,
        deg_out=deg_out,
        deg_in=deg_in,
    )
    return meta, gidx_cores, dstoff_cores


def _tile_major(vec, NG, GRP, pad_val):
    """[NPC] -> [GRP, NG]: entry (p, m) = vec[m*GRP + p], padded."""
    out = np.full((NG * GRP,), pad_val, vec.dtype)
    out[: vec.shape[0]] = vec
    return np.ascontiguousarray(out.reshape(NG, GRP).T)


def _build_nc(cfg, meta, b_nonzero=False):
    N, IN, OUT, C = cfg["N"], cfg["IN"], cfg["OUT"], cfg["NCORES"]
    GRP, NBANKS = cfg["GRP"], cfg["NBANKS"]
    NPC, NPCP, NG = meta["NPC"], meta["NPCP"], meta["NG"]
    nidx_tot, nb_tot = meta["nidx_tot"], meta["nb_tot"]
    units = meta["units"]
    XK = _ceil_div(IN, 128)
    assert OUT == 128 and GRP == 128
    last_w = NPC - (NG - 1) * GRP  # valid rows in the last group
    HALF = NPCP // 2               # rows per AG-half input

    nc = bacc.Bacc(
        "TRN2", target_bir_lowering=False, debug=False, num_devices=C
    )

    # ---- external inputs ----
    NXQ = 4  # x DMA split for earlier stage-B start
    xq = NPCP // NXQ
    assert NPCP % NXQ == 0
    xt = [
        nc.dram_tensor(f"xt{j}", [128, NPCP], BF16, kind="ExternalInput")
        for j in range(XK)
    ]
    wt = [
        nc.dram_tensor(f"wt{j}", [128, OUT], BF16, kind="ExternalInput")
        for j in range(XK)
    ]
    gidx_d = nc.dram_tensor("gidx", [128, nidx_tot // 16], I16, kind="ExternalInput")
    doff_d = nc.dram_tensor("doff", [128, meta["nseg"]], F32, kind="ExternalInput")
    dego_d = nc.dram_tensor("dego", [128, NG], F32, kind="ExternalInput")
    degi_d = nc.dram_tensor("degi", [128, NG], F32, kind="ExternalInput")
    iota_d = nc.dram_tensor("iota", [128, GRP], BF16, kind="ExternalInput")
    gm_d = nc.dram_tensor("gm", [1, OUT], F32, kind="ExternalInput")
    bb_d = nc.dram_tensor("bb", [1, OUT], F32, kind="ExternalInput")
    onesc_d = nc.dram_tensor("onesc", [128, 1], BF16, kind="ExternalInput")
    onest_d = nc.dram_tensor("onest", [128, 1], BF16, kind="ExternalInput")
    onesr_d = nc.dram_tensor("onesr", [1, 128], F32, kind="ExternalInput")
    ident_d = nc.dram_tensor("ident", [128, 128], BF16, kind="ExternalInput")
    if b_nonzero:
        bt_d = nc.dram_tensor("bt", [1, OUT], F32, kind="ExternalInput")

    ypad_d = nc.dram_tensor("ypad", [NG * GRP, OUT], F32, kind="ExternalOutput")

    with tile.TileContext(nc) as tc:
        with (
            tc.tile_pool(name="const", bufs=1) as cpool,
            tc.tile_pool(name="dram", bufs=1, space="DRAM") as dpool,
            tc.tile_pool(name="agg", bufs=1) as apool,
            tc.tile_pool(name="mpool", bufs=16) as mpool,
            tc.tile_pool(name="etmp", bufs=4) as epool,
            tc.tile_pool(name="psg", bufs=4, space="PSUM") as pgpool,
            tc.tile_pool(name="psb", bufs=2, space="PSUM") as pbpool,
            tc.tile_pool(name="pstat", bufs=1, space="PSUM") as pspool,
        ):
            # ---- constants / small tiles ----
            iota_t = cpool.tile([128, GRP], BF16)
            dego_t = cpool.tile([128, NG], F32)
            degi_t = cpool.tile([128, NG], F32)
            nsrc_t = cpool.tile([128, NG], F32)
            ndst_t = cpool.tile([128, NG], F32)
            gm_t = cpool.tile([1, OUT], F32)
            bb_t = cpool.tile([1, OUT], F32)
            onesc_t = cpool.tile([128, 1], BF16)
            onest_t = cpool.tile([128, 1], BF16)
            onesr_t = cpool.tile([1, 128], F32)
            gidx_t = cpool.tile([128, nidx_tot // 16], I16)
            doff_t = cpool.tile([128, meta["nseg"]], F32)
            ident_t = cpool.tile([128, 128], BF16)
            nc.sync.dma_start(ident_t[:], ident_d[:])

            nc.sync.dma_start(iota_t[:], iota_d[:])
            nc.sync.dma_start(dego_t[:], dego_d[:])
            nc.sync.dma_start(degi_t[:], degi_d[:])
            nc.sync.dma_start(gm_t[:], gm_d[:])
            nc.sync.dma_start(bb_t[:], bb_d[:])
            nc.sync.dma_start(onesc_t[:], onesc_d[:])
            nc.sync.dma_start(onest_t[:], onest_d[:])
            nc.sync.dma_start(onesr_t[:], onesr_d[:])
            if b_nonzero:
                bt_t = cpool.tile([1, OUT], F32)
                nc.sync.dma_start(bt_t[:], bt_d[:])

            # norms: rsqrt(max(deg, 1))
            for deg_t, norm_t in ((dego_t, nsrc_t), (degi_t, ndst_t)):
                nc.vector.tensor_scalar(
                    norm_t[:], deg_t[:], 1.0, None, op0=mybir.AluOpType.max
                )
                nc.vector.reciprocal(norm_t[:], norm_t[:])
                nc.scalar.activation(
                    norm_t[:], norm_t[:], mybir.ActivationFunctionType.Sqrt
                )

            # internal DRAM for collectives (3:1 interleaved node split)
            _aspace = "Local" if cfg.get("NOCC") else "Shared"
            HA = 3 * NPCP // 4     # nodes with loc%4 in {0,1,2}
            HB = NPCP // 4         # nodes with loc%4 == 3
            h_my_a = dpool.tile([HA, OUT], BF16, name="h_my_a")
            h_my_b = dpool.tile([HB, OUT], BF16, name="h_my_b")
            h_all_a = dpool.tile(
                [C * HA, OUT], BF16, addr_space=_aspace, name="h_all_a"
            )
            h_all_b = dpool.tile(
                [C * HB, OUT], BF16, addr_space=_aspace, name="h_all_b"
            )
            stats_in = dpool.tile([1, 2 * OUT], F32)
            stats_out = dpool.tile([C, 2 * OUT], F32, addr_space=_aspace)

            # relu(norm*agg) output, bf16, [128, NG, OUT]
            agg_t = apool.tile([128, NG, OUT], BF16)

            # ---- stage B: h = (x @ W) * norm_src, cast bf16, store to HBM
            # (staged in SBUF; 2 large DMAs instead of 98 small ones)
            with tc.tile_pool(name="xw", bufs=1) as xwp:
                xts = []
                wts = []
                for j in range(XK):
                    xts.append(xwp.tile([128, NPCP], BF16, name=f"xt_s{j}"))
                    wts.append(xwp.tile([128, OUT], BF16, name=f"wt_s{j}"))
                for j in range(XK):
                    nc.sync.dma_start(wts[j][:], wt[j][:])
                for q in range(NXQ):
                    for j in range(XK):
                        nc.sync.dma_start(
                            xts[j][:, q * xq : (q + 1) * xq],
                            xt[j][:, q * xq : (q + 1) * xq],
                        )
                hstage = xwp.tile([128, NG, OUT], BF16, name="hstage")
                for m in range(NG):
                    ps = pbpool.tile([128, OUT], F32, tag="hps")
                    for j in range(XK):
                        nc.tensor.matmul(
                            ps[:, :],
                            xts[j][:, m * GRP : (m + 1) * GRP],
                            wts[j][:, :],
                            start=(j == 0),
                            stop=(j == XK - 1),
                        )
                    if m % 2 == 0:
                        nc.scalar.activation(
                            hstage[:, m, :],
                            ps[:, :],
                            mybir.ActivationFunctionType.Copy,
                            scale=nsrc_t[:, m : m + 1],
                        )
                    else:
                        nc.vector.tensor_scalar(
                            hstage[:, m, :],
                            ps[:, :],
                            nsrc_t[:, m : m + 1],
                            None,
                            op0=mybir.AluOpType.mult,
                        )
                # partitions c*32+q hold node loc = g*128 + 4q + c (x columns
                # host-permuted): h_my_a row g*96 + 3q + c, h_my_b row g*32+q
                hq = NG // 8
                for q8 in range(8):
                    a = q8 * hq
                    z = (q8 + 1) * hq if q8 < 7 else NG
                    va = h_my_a[a * 96 : z * 96, :].rearrange(
                        "(g q c) f -> q g c f", q=32, c=3
                    )
                    for c_ in range(3):
                        nc.sync.dma_start(
                            va[:, :, c_, :],
                            hstage[c_ * 32 : (c_ + 1) * 32, a:z, :],
                        )
                    nc.sync.dma_start(
                        h_my_b[a * 32 : z * 32, :].rearrange(
                            "(g p) f -> p g f", p=32
                        ),
                        hstage[96:128, a:z, :],
                    )

            # ---- stage C: two AllGathers (3/4 part, then 1/4 part) ----
            for h_my_h, h_all_h, hr in (
                (h_my_a, h_all_a, HA),
                (h_my_b, h_all_b, HB),
            ):
                if cfg.get("NOCC"):
                    rep = (
                        h_my_h[:]
                        .rearrange("(o r) f -> o r f", o=1)
                        .to_broadcast((C, hr, OUT))
                    )
                    nc.sync.dma_start(
                        h_all_h[:].rearrange("(o r) f -> o r f", o=C), rep
                    )
                else:
                    nc.gpsimd.collective_compute(
                        "AllGather",
                        mybir.AluOpType.bypass,
                        replica_groups=[list(range(C))],
                        ins=[h_my_h[:]],
                        outs=[h_all_h[:]],
                    )

            # index tables are first needed by stage D's gathers - load
            # them after the x/B/AG chain is underway
            nc.sync.dma_start(gidx_t[:], gidx_d[:])
            nc.sync.dma_start(doff_t[:], doff_d[:])

            # interleaved bank views: banks 0..2 -> row 3j + b of part A,
            # bank 3 -> row j of part B
            h_banks = [
                h_all_a[:].rearrange("(j k) f -> j (k f)", k=3)[
                    :, b * OUT : (b + 1) * OUT
                ]
                for b in range(3)
            ] + [h_all_b[:]]
            h_esteps = [3 * OUT, 3 * OUT, 3 * OUT, OUT]

            # ---- stage D: gather + one-hot matmul segmented sum ----
            # ---- stage E (inline): relu(psum*ndst) + BN partial sums ----
            # Gathers are batched per (chunk, bank); groups are processed
            # sequentially (their 4 bank runs back-to-back) so each PSUM bank
            # holds at most one pending accumulation group.
            ps_stat = pspool.tile([1, 2 * OUT], F32, name="ps_stat")
            ps_sum = ps_stat[:, 0:OUT]
            ps_sq = ps_stat[:, OUT : 2 * OUT]
            ndone = [0]  # groups completed (for BN-sum start/stop flags)

            def finish_group(g, ps_g):
                """relu + BN-sum accumulation for a completed group psum."""
                if b_nonzero:
                    tmp = epool.tile([128, OUT], F32, tag="etmp")
                    nc.vector.scalar_tensor_tensor(
                        tmp[:],
                        ps_g[:],
                        ndst_t[:, g : g + 1],
                        btile_t[:],
                        op0=mybir.AluOpType.mult,
                        op1=mybir.AluOpType.add,
                    )
                    nc.scalar.activation(
                        agg_t[:, g, :], tmp[:], mybir.ActivationFunctionType.Relu
                    )
                else:
                    nc.scalar.activation(
                        agg_t[:, g, :],
                        ps_g[:],
                        mybir.ActivationFunctionType.Relu,
                        scale=ndst_t[:, g : g + 1],
                    )
                ones = onesc_t if g < NG - 1 else onest_t
                i0 = ndone[0]
                # ps_sum/ps_sq share one bank = ONE accumulation group:
                # start only on the very first matmul, stop on the very last.
                nc.tensor.matmul(
                    ps_sum,
                    ones[:],
                    agg_t[:, g, :],
                    start=(i0 == 0),
                    stop=False,
                )
                sq = epool.tile([128, OUT], BF16, tag="esq")
                nc.scalar.activation(
                    sq[:], agg_t[:, g, :], mybir.ActivationFunctionType.Square
                )
                nc.tensor.matmul(
                    ps_sq,
                    ones[:],
                    sq[:],
                    start=False,
                    stop=(i0 == NG - 1),
                )
                ndone[0] += 1

            if b_nonzero:
                # replicate b across partitions once (PE broadcast)
                ps_b = pbpool.tile([128, OUT], F32, tag="hps", name="ps_b")
                btile_t = cpool.tile([128, OUT], F32)
                nc.tensor.matmul(ps_b[:], onesr_t[:], bt_t[:], start=True, stop=True)
                nc.scalar.activation(
                    btile_t[:], ps_b[:], mybir.ActivationFunctionType.Copy
                )

            run_segs = meta["run_segs"]
            chunks = meta["chunks"]
            nbmax = max(nb for _, _, nb in units)
            dstack = tc.tile_pool(name="gath", bufs=6)
            gpool = dstack.__enter__()
            phases = [(0, 1, 2), (3,)]
            ui = 0
            for pi, ph in enumerate(phases):
                for ci, ch in enumerate(chunks):
                    gts = {}
                    for b in ph:
                        bank, t0, nblk = units[ui]
                        ui += 1
                        assert bank == b
                        Gt = gpool.tile(
                            [128, nbmax, OUT], BF16, tag="G", name=f"G{pi}_{ci}_{b}"
                        )
                        nc.gpsimd.dma_gather(
                            Gt[:, :nblk, :],
                            h_banks[b],
                            gidx_t[:, t0 * 8 : (t0 + nblk) * 8],
                            nblk * 128,
                            nblk * 128,
                            OUT,
                            elem_step=h_esteps[b],
                            single_packet=False,
                        )
                        gts[b] = (Gt, t0)
                    for g in ch:
                        ps_g = pgpool.tile(
                            [128, OUT], F32, tag="aggps", name=f"ps{pi}_{g}"
                        )
                        if pi == 1:
                            # re-inject phase-A partial (spilled bf16)
                            nc.tensor.matmul(
                                ps_g[:],
                                ident_t[:],
                                agg_t[:, g, :],
                                start=True,
                                stop=False,
                            )
                        for bi, b in enumerate(ph):
                            Gt, t0 = gts[b]
                            segs = run_segs[(b, g)]
                            for si, (t, col, lo, hi) in enumerate(segs):
                                Mt = mpool.tile([128, GRP], BF16, tag="M")
                                nc.vector.tensor_scalar(
                                    Mt[:],
                                    iota_t[:],
                                    doff_t[:, col : col + 1],
                                    None,
                                    op0=mybir.AluOpType.is_equal,
                                )
                                nc.tensor.matmul(
                                    ps_g[:],
                                    Mt[:],
                                    Gt[:, t - t0, :],
                                    start=(pi == 0 and bi == 0 and si == 0),
                                    stop=(
                                        bi == len(ph) - 1
                                        and si == len(segs) - 1
                                    ),
                                )
                        if pi == 0:
                            # spill partial sum to agg_t (bf16), no relu yet
                            nc.scalar.activation(
                                agg_t[:, g, :],
                                ps_g[:],
                                mybir.ActivationFunctionType.Copy,
                            )
                        else:
                            finish_group(g, ps_g)
            dstack.__exit__(None, None, None)
            assert ndone[0] == NG

            # ---- stage F: AllReduce BN stats; build affine S/T tiles ----
            st_sb = cpool.tile([1, 2 * OUT], F32)
            nc.scalar.activation(
                st_sb[:, 0:OUT], ps_sum, mybir.ActivationFunctionType.Copy
            )
            nc.scalar.activation(
                st_sb[:, OUT : 2 * OUT], ps_sq, mybir.ActivationFunctionType.Copy
            )
            nc.sync.dma_start(stats_in[:], st_sb[:])
            if cfg.get("NOCC"):
                rep = (
                    stats_in[:]
                    .rearrange("(o r) f -> o r f", o=1)
                    .to_broadcast((C, 1, 2 * OUT))
                )
                nc.sync.dma_start(
                    stats_out[:].rearrange("(o r) f -> o r f", o=C), rep
                )
            else:
                nc.gpsimd.collective_compute(
                    "AllGather",
                    mybir.AluOpType.bypass,
                    replica_groups=[list(range(C))],
                    ins=[stats_in[:]],
                    outs=[stats_out[:]],
                )
            # per-core partials land row-major; sum the C rows locally
            st_all = cpool.tile([1, C * 2 * OUT], F32)
            nc.sync.dma_start(
                st_all[:], stats_out[:].rearrange("(o c) f -> o (c f)", o=1)
            )
            st_rb = cpool.tile([1, 2 * OUT], F32)
            nc.vector.tensor_add(
                st_rb[:], st_all[:, 0 : 2 * OUT], st_all[:, 2 * OUT : 4 * OUT]
            )
            for c_ in range(2, C):
                nc.vector.tensor_add(
                    st_rb[:],
                    st_rb[:],
                    st_all[:, c_ * 2 * OUT : (c_ + 1) * 2 * OUT],
                )

            mu = cpool.tile([1, OUT], F32)
            ex2 = cpool.tile([1, OUT], F32)
            var = cpool.tile([1, OUT], F32)
            srow = cpool.tile([1, OUT], F32)
            trow = cpool.tile([1, OUT], F32)
            inv_n = 1.0 / float(N)
            nc.scalar.activation(
                mu[:], st_rb[:, 0:OUT], mybir.ActivationFunctionType.Copy, scale=inv_n
            )
            nc.scalar.activation(
                ex2[:],
                st_rb[:, OUT : 2 * OUT],
                mybir.ActivationFunctionType.Copy,
                scale=inv_n,
            )
            nc.scalar.activation(var[:], mu[:], mybir.ActivationFunctionType.Square)
            nc.vector.tensor_sub(var[:], ex2[:], var[:])
            # var <- rsqrt(var + eps) (ACT Rsqrt is banned for accuracy)
            nc.scalar.activation(
                var[:],
                var[:],
                mybir.ActivationFunctionType.Copy,
                bias=float(cfg["EPS"]),
            )
            nc.vector.reciprocal(var[:], var[:])
            nc.scalar.activation(var[:], var[:], mybir.ActivationFunctionType.Sqrt)
            nc.vector.tensor_mul(srow[:], gm_t[:], var[:])
            nc.vector.tensor_mul(trow[:], mu[:], srow[:])
            nc.vector.tensor_sub(trow[:], bb_t[:], trow[:])

            S_t = cpool.tile([128, OUT], BF16)
            T_t = cpool.tile([128, OUT], BF16)
            ps_S = pgpool.tile([128, OUT], F32, tag="aggps", name="ps_S")
            ps_T = pgpool.tile([128, OUT], F32, tag="aggps", name="ps_T")
            nc.tensor.matmul(ps_S[:], onesr_t[:], srow[:], start=True, stop=True)
            nc.tensor.matmul(ps_T[:], onesr_t[:], trow[:], start=True, stop=True)
            nc.scalar.activation(S_t[:], ps_S[:], mybir.ActivationFunctionType.Copy)
            nc.scalar.activation(T_t[:], ps_T[:], mybir.ActivationFunctionType.Copy)

            # ---- stage G: y = hrelu * S + T (bf16), cast f32 on DMA out ----
            with tc.tile_pool(name="gtmp", bufs=2) as gpool2:
                GB = 14  # groups per batched op
                S_bc = (
                    S_t[:]
                    .rearrange("p (o f) -> p o f", o=1)
                    .to_broadcast((128, GB, OUT))
                )
                T_bc = (
                    T_t[:]
                    .rearrange("p (o f) -> p o f", o=1)
                    .to_broadcast((128, GB, OUT))
                )
                ypad_view = ypad_d[:].rearrange("(g p) f -> p g f", p=128)
                for g0 in range(0, NG, GB):
                    gw = min(GB, NG - g0)
                    tmp = gpool2.tile([128, GB, OUT], BF16, tag="gtmp")
                    nc.vector.tensor_mul(
                        tmp[:, :gw, :],
                        agg_t[:, g0 : g0 + gw, :],
                        S_bc if gw == GB else S_t[:]
                        .rearrange("p (o f) -> p o f", o=1)
                        .to_broadcast((128, gw, OUT)),
                    )
                    nc.vector.tensor_add(
                        agg_t[:, g0 : g0 + gw, :],
                        tmp[:, :gw, :],
                        T_bc if gw == GB else T_t[:]
                        .rearrange("p (o f) -> p o f", o=1)
                        .to_broadcast((128, gw, OUT)),
                    )
                    nc.gpsimd.dma_start(
                        ypad_view[:, g0 : g0 + gw, :],
                        agg_t[:, g0 : g0 + gw, :],
                    )

    nc.compile()
    return nc


def kernel(x, src, dst, W, b, gamma, beta):
    global LAST_RESULTS
    cfg = CFG
    N, E, IN, OUT, C = cfg["N"], cfg["E"], cfg["IN"], cfg["OUT"], cfg["NCORES"]
    GRP = cfg["GRP"]
    assert x.shape == (N, IN) and W.shape == (IN, OUT)
    assert src.shape == (E,) and dst.shape == (E,)

    b = np.asarray(b, np.float32)
    b_nonzero = bool(np.any(b != 0.0))
    meta, gidx_cores, dstoff_cores = _preprocess(cfg, src, dst)
    NPC, NPCP, NG = meta["NPC"], meta["NPCP"], meta["NG"]
    XK = _ceil_div(IN, 128)
    last_w = NPC - (NG - 1) * GRP
    # node permutation: within each 128-node group, order by loc%4 class
    perm = np.concatenate([np.arange(c, 128, 4) for c in range(4)])
    g_ = np.arange(NPCP) // 128
    p_ = np.arange(NPCP) % 128
    permn = g_ * 128 + perm[p_]          # source node (local) per padded col
    valid = permn < NPC

    nc = _build_nc(cfg, meta, b_nonzero=b_nonzero)

    xT = np.ascontiguousarray(np.asarray(x, np.float32).T)  # [IN, N]
    Wn = np.asarray(W, np.float32)
    import ml_dtypes

    iota = np.tile(np.arange(GRP, dtype=np.float32)[None, :], (128, 1)).astype(
        ml_dtypes.bfloat16
    )
    onesc = np.ones((128, 1), np.float32)
    onest = np.zeros((128, 1), np.float32)
    onest[:last_w] = 1.0
    onesr = np.ones((1, 128), np.float32)
    gm = np.asarray(gamma, np.float32)[None, :]
    bb = np.asarray(beta, np.float32)[None, :]

    in_maps = []
    for k in range(C):
        im = {
            "gidx": gidx_cores[k],
            "doff": dstoff_cores[k],
            "dego": _tile_major(
                np.where(
                    valid,
                    meta["deg_out"][k * NPC + np.minimum(permn, NPC - 1)],
                    np.float32(1.0),
                ).astype(np.float32),
                NG,
                GRP,
                np.float32(1.0),
            ),
            "degi": _tile_major(
                meta["deg_in"][k * NPC : (k + 1) * NPC], NG, GRP, np.float32(1.0)
            ),
            "iota": iota,
            "gm": gm,
            "bb": bb,
            "onesc": onesc.astype(ml_dtypes.bfloat16),
            "onest": onest.astype(ml_dtypes.bfloat16),
            "onesr": onesr,
            "ident": np.eye(128, dtype=np.float32).astype(ml_dtypes.bfloat16),
        }
        if b_nonzero:
            im["bt"] = b[None, :]
        for j in range(XK):
            xcols = np.zeros((128, NPCP), np.float32)
            xcols[:, valid] = xT[
                j * 128 : (j + 1) * 128, k * NPC + permn[valid]
            ]
            im[f"xt{j}"] = xcols.astype(ml_dtypes.bfloat16)
            im[f"wt{j}"] = np.ascontiguousarray(
                Wn[j * 128 : (j + 1) * 128, :]
            ).astype(ml_dtypes.bfloat16)
        in_maps.append(im)

    if cfg.get("SIM"):
        from concourse.bass_interp import MultiCoreSim

        sim = MultiCoreSim(nc, num_cores=C)
        for k, core_sim in sim.cores.items():
            for name, val in in_maps[k].items():
                core_sim.tensor(name)[:] = val
        sim.simulate()
        y = np.empty((N, OUT), np.float32)
        for k in range(C):
            y[k * NPC : (k + 1) * NPC] = sim.cores[k].tensor("ypad")[:NPC]
        return y

    global LAST_NC, LAST_RUN_S
    LAST_NC = nc
    import time as _time

    _t0 = _time.time()
    res = bass_utils.run_bass_kernel_spmd(
        nc,
        in_maps,
        core_ids=list(range(C)),
        trace=cfg.get("TRACE", False),
    )
    LAST_RUN_S = _time.time() - _t0
    LAST_RESULTS = res

    y = np.empty((N, OUT), np.float32)
    for k in range(C):
        y[k * NPC : (k + 1) * NPC] = res.results[k]["ypad"][:NPC]
    return y


# revision 37
# speedup vs baseline: 1.1393x; 1.0195x over previous
"""GCN block (GraphConv + BatchNorm1d + ReLU) on 8 Trainium2 NeuronCores.

Strategy (per sharding hint): partition nodes (and incident edges) across the
8 cores; replicate W/b/gamma/beta; all-reduce BN batch statistics.

Per core k (owns dst nodes [k*NPC, (k+1)*NPC)):
  1. h_k = (x_k @ W) * rsqrt(clip(deg_out_k,1))  (PE matmul; x columns are
     host-permuted so even nodes land on partitions 0:64, odd on 64:128).
  2. TWO AllGathers of h (bf16): even nodes, then odd nodes. Gathers that
     read even-sourced edges overlap the second collective. The 4
     int16-indexable "bank" tables are *interleaved strided views* of the
     AG outputs: bank b = nodes with (n % NPC) % 4 == b lives in half b%2
     at row 2*j + b//2, j = owner*(NPCP/4) + (n % NPC)//4 (elem_step=2
     rows). Interleaving makes bank-row -> table-row linear, so two big
     collectives (cheap) serve four int16-indexed gather tables.
  3. For each (bank, dst-group) run of edges (64-slot granular, shared
     run sizes = max over cores), gather h[src] rows (dma_gather, bf16,
     one batched gather per (phase, chunk-of-groups, bank)) and
     segment-sum them with one-hot matmuls M^T @ G accumulated in PSUM.
     Each dst group keeps ONE psum accumulation per phase; the phase-A
     partial spills to bf16 and is re-injected via an identity matmul.
     Runs straddling 128-slot block boundaries get one matmul per
     straddled block; out-of-segment slots carry doff 255 so their
     one-hot column is zero.
  4. relu(psum * rsqrt(clip(deg_in,1)) [+ b]) via ACT directly from PSUM
     (bf16 out); BN sums via ones-matmuls (single accumulation group);
     AllReduce sums; y = h*S + T with S = gamma*rsqrt(var+eps),
     T = beta - mu*S (broadcast-AP DVE ops); y cast bf16->f32 during the
     output DMA (SWDGE), pipelined per 14-group batch.

Host-side work is limited to integer index bookkeeping (bucketing edges by
(core, src-bank, dst-group), degree counting) and layout transforms (x^T
permutation/padding, int16 gather indices). All floating-point math runs on
device.

Run sizes are padded to a structure shared by all 8 cores so a single SPMD
NEFF serves every core; pad slots re-gather the run's last row (HBM page
hit) and carry a dst offset of 255 -> contribute exactly 0. Edges are
sorted by gather row within each bucket for HBM locality.
"""
import math
import os
import sys

sys.path.insert(0, "/opt/trn_rl_repo")

import numpy as np

import concourse.bacc as bacc
import concourse.bass as bass
import concourse.mybir as mybir
import concourse.tile as tile
from concourse import bass_utils

F32 = mybir.dt.float32
BF16 = mybir.dt.bfloat16
I16 = mybir.dt.int16

CFG = dict(
    N=100000,
    E=1600000,
    IN=256,
    OUT=128,
    NCORES=8,
    GRP=128,          # dst nodes per segment group (= psum partition dim)
    NBANKS=4,         # interleaved src banks (bank rows must be < 32768)
    GCHUNK=12,        # groups per chunk (gather batch granularity)
    EPS=1e-5,
    TRACE=False,
)

LAST_RESULTS = None  # set by kernel() for test harness introspection
LAST_NC = None
LAST_RUN_S = None


def _ceil_div(a, b):
    return (a + b - 1) // b


def _wrap16(idx, ncols):
    """int16 idx list -> [128, ncols] tile: idx i at [i%16, i//16], replicated
    8x across the 16-partition groups (one copy per GpSimd Q7 core)."""
    n = idx.shape[0]
    assert n == ncols * 16
    w = np.ascontiguousarray(idx.reshape(ncols, 16).T)
    return np.tile(w, (8, 1))


def _preprocess(cfg, src, dst):
    """Bucket edges by (owner core, interleaved src bank, dst group); build
    per-core gather-index / dst-offset arrays and the shared run structure."""
    N, E = cfg["N"], cfg["E"]
    C, NBANKS, GRP, GC = cfg["NCORES"], cfg["NBANKS"], cfg["GRP"], cfg["GCHUNK"]
    NPC = N // C
    NG = _ceil_div(NPC, GRP)
    NPCP = NG * GRP                # padded nodes per core (x cols zero-padded)
    assert NPCP % NBANKS == 0
    QB = NPCP // NBANKS            # gather rows per owner per bank view
    BANKROWS = QB * C              # rows per bank view of one AG-half output
    assert BANKROWS < 32768

    src = src.astype(np.int64)
    dst = dst.astype(np.int64)
    deg_out = np.bincount(src, minlength=N).astype(np.float32)
    deg_in = np.bincount(dst, minlength=N).astype(np.float32)

    owner = dst // NPC
    loc = src % NPC
    bank = loc % NBANKS            # interleaved bank of the source
    grow = (src // NPC) * QB + loc // NBANKS   # gather row within bank view
    assert grow.max() < 32768
    grp = (dst % NPC) // GRP
    key = (owner * NBANKS + bank) * NG + grp
    # sort by bucket, then by gather row inside the bucket (HBM locality)
    order = np.lexsort((grow, key))
    s_grow = grow[order]
    s_dst = dst[order]
    s_key = key[order]

    counts = np.bincount(key, minlength=C * NBANKS * NG).reshape(C, NBANKS, NG)
    P = counts.max(axis=0)  # [NBANKS, NG] shared run sizes (64-granular)
    P = ((P + 63) // 64) * 64
    P = np.maximum(P, 64)   # every (b,g) run structurally exists

    # two phases: banks {0,1,2} (3/4 AG part), then {3} (1/4 part)
    phases = [(0, 1, 2), (3,)]
    chunks = [list(range(c, min(c + GC, NG))) for c in range(0, NG, GC)]
    run_seq = [
        (b, g) for ph in phases for ch in chunks for b in ph for g in ch
    ]
    # lay out runs; pad each (phase, chunk, bank) unit to a 128 multiple
    run_off = np.zeros((NBANKS, NG), np.int64)
    units = []  # (bank, first_block, n_blocks) in stream order
    pos = 0
    for ph in phases:
        for ch in chunks:
            for b in ph:
                u0 = pos
                for g in ch:
                    run_off[b, g] = pos
                    pos += P[b, g]
                pos = ((pos + 127) // 128) * 128  # unit pad
                units.append((b, u0 // 128, (pos - u0) // 128))
    nidx_tot = int(pos)
    nb_tot = nidx_tot // 128

    # segments: a run may straddle block boundaries; each (run, block)
    # intersection is one segment = one doff column + one full matmul
    # (out-of-segment slots carry doff 255 -> zero one-hot column).
    run_segs = {}  # (b, g) -> list of (block_t, doff_col, slot_lo, slot_hi)
    nseg = 0
    for b, g in run_seq:
        off = int(run_off[b, g])
        end = off + int(P[b, g])
        segs = []
        t = off // 128
        while t * 128 < end:
            lo = max(off, t * 128)
            hi = min(end, (t + 1) * 128)
            segs.append((t, nseg, lo, hi))
            nseg += 1
            t += 1
        run_segs[(b, g)] = segs

    # boundaries of each (k, b, g) bucket in the sorted edge stream
    bkeys = (
        np.arange(C)[:, None, None] * NBANKS + np.arange(NBANKS)[None, :, None]
    ) * NG + np.arange(NG)[None, None, :]
    starts = np.searchsorted(s_key, bkeys.ravel()).reshape(C, NBANKS, NG)
    ends = np.searchsorted(s_key, bkeys.ravel(), side="right").reshape(C, NBANKS, NG)

    gidx_cores = []
    dstoff_cores = []
    for k in range(C):
        gidx = np.zeros(nidx_tot, np.int16)
        doff_cols = np.full((nseg, 128), 255.0, np.float32)
        for b in range(NBANKS):
            for g in range(NG):
                s, e = starts[k, b, g], ends[k, b, g]
                cnt = e - s
                p0 = int(run_off[b, g])
                if cnt:
                    gidx[p0 : p0 + cnt] = s_grow[s:e].astype(np.int16)
                    # pad slots re-gather the last row (HBM page hit)
                    gidx[p0 + cnt : p0 + int(P[b, g])] = gidx[p0 + cnt - 1]
                    offs = ((s_dst[s:e] % NPC) - g * GRP).astype(np.float32)
                    for t, col, lo, hi in run_segs[(b, g)]:
                        a = max(lo, p0)
                        z = min(hi, p0 + cnt)
                        if z > a:
                            doff_cols[col, a - t * 128 : z - t * 128] = offs[
                                a - p0 : z - p0
                            ]
        # unit-pad slots gather row 0 (gidx stays 0) and have no segment
        gidx_cores.append(_wrap16(gidx, nidx_tot // 16))
        dstoff_cores.append(np.ascontiguousarray(doff_cols.T))

    meta = dict(
        NPC=NPC,
        NPCP=NPCP,
        NG=NG,
        QB=QB,
        BANKROWS=BANKROWS,
        nidx_tot=nidx_tot,
        nb_tot=nb_tot,
        nseg=nseg,
        run_segs=run_segs,
        units=units,
        chunks=chunks,
        run_seq=run_seq,
        deg_out=deg_out,
        deg_in=deg_in,
    )
    return meta, gidx_cores, dstoff_cores


def _tile_major(vec, NG, GRP, pad_val):
    """[NPC] -> [GRP, NG]: entry (p, m) = vec[m*GRP + p], padded."""
    out = np.full((NG * GRP,), pad_val, vec.dtype)
    out[: vec.shape[0]] = vec
    return np.ascontiguousarray(out.reshape(NG, GRP).T)


def _build_nc(cfg, meta, b_nonzero=False):
    N, IN, OUT, C = cfg["N"], cfg["IN"], cfg["OUT"], cfg["NCORES"]
    GRP, NBANKS = cfg["GRP"], cfg["NBANKS"]
    NPC, NPCP, NG = meta["NPC"], meta["NPCP"], meta["NG"]
    nidx_tot, nb_tot = meta["nidx_tot"], meta["nb_tot"]
    units = meta["units"]
    XK = _ceil_div(IN, 128)
    assert OUT == 128 and GRP == 128
    last_w = NPC - (NG - 1) * GRP  # valid rows in the last group
    HALF = NPCP // 2               # rows per AG-half input

    nc = bacc.Bacc(
        "TRN2", target_bir_lowering=False, debug=False, num_devices=C
    )

    # ---- external inputs ----
    NXQ = 8  # x DMA split for earlier stage-B start
    xq = NPCP // NXQ
    assert NPCP % NXQ == 0
    xt = [
        nc.dram_tensor(f"xt{j}", [128, NPCP], BF16, kind="ExternalInput")
        for j in range(XK)
    ]
    wt = [
        nc.dram_tensor(f"wt{j}", [128, OUT], BF16, kind="ExternalInput")
        for j in range(XK)
    ]
    gidx_d = nc.dram_tensor("gidx", [128, nidx_tot // 16], I16, kind="ExternalInput")
    doff_d = nc.dram_tensor("doff", [128, meta["nseg"]], F32, kind="ExternalInput")
    dego_d = nc.dram_tensor("dego", [128, NG], F32, kind="ExternalInput")
    degi_d = nc.dram_tensor("degi", [128, NG], F32, kind="ExternalInput")
    iota_d = nc.dram_tensor("iota", [128, GRP], BF16, kind="ExternalInput")
    gm_d = nc.dram_tensor("gm", [1, OUT], F32, kind="ExternalInput")
    bb_d = nc.dram_tensor("bb", [1, OUT], F32, kind="ExternalInput")
    onesc_d = nc.dram_tensor("onesc", [128, 1], BF16, kind="ExternalInput")
    onest_d = nc.dram_tensor("onest", [128, 1], BF16, kind="ExternalInput")
    onesr_d = nc.dram_tensor("onesr", [1, 128], F32, kind="ExternalInput")
    ident_d = nc.dram_tensor("ident", [128, 128], BF16, kind="ExternalInput")
    if b_nonzero:
        bt_d = nc.dram_tensor("bt", [1, OUT], F32, kind="ExternalInput")

    ypad_d = nc.dram_tensor("ypad", [NG * GRP, OUT], F32, kind="ExternalOutput")

    with tile.TileContext(nc) as tc:
        with (
            tc.tile_pool(name="const", bufs=1) as cpool,
            tc.tile_pool(name="dram", bufs=1, space="DRAM") as dpool,
            tc.tile_pool(name="agg", bufs=1) as apool,
            tc.tile_pool(name="mpool", bufs=16) as mpool,
            tc.tile_pool(name="etmp", bufs=4) as epool,
            tc.tile_pool(name="psg", bufs=4, space="PSUM") as pgpool,
            tc.tile_pool(name="psb", bufs=2, space="PSUM") as pbpool,
            tc.tile_pool(name="pstat", bufs=1, space="PSUM") as pspool,
        ):
            # ---- constants / small tiles ----
            iota_t = cpool.tile([128, GRP], BF16)
            dego_t = cpool.tile([128, NG], F32)
            degi_t = cpool.tile([128, NG], F32)
            nsrc_t = cpool.tile([128, NG], F32)
            ndst_t = cpool.tile([128, NG], F32)
            gm_t = cpool.tile([1, OUT], F32)
            bb_t = cpool.tile([1, OUT], F32)
            onesc_t = cpool.tile([128, 1], BF16)
            onest_t = cpool.tile([128, 1], BF16)
            onesr_t = cpool.tile([1, 128], F32)
            gidx_t = cpool.tile([128, nidx_tot // 16], I16)
            doff_t = cpool.tile([128, meta["nseg"]], F32)
            ident_t = cpool.tile([128, 128], BF16)
            nc.sync.dma_start(ident_t[:], ident_d[:])

            nc.sync.dma_start(iota_t[:], iota_d[:])
            nc.sync.dma_start(dego_t[:], dego_d[:])
            nc.sync.dma_start(degi_t[:], degi_d[:])
            nc.sync.dma_start(gm_t[:], gm_d[:])
            nc.sync.dma_start(bb_t[:], bb_d[:])
            nc.sync.dma_start(onesc_t[:], onesc_d[:])
            nc.sync.dma_start(onest_t[:], onest_d[:])
            nc.sync.dma_start(onesr_t[:], onesr_d[:])
            if b_nonzero:
                bt_t = cpool.tile([1, OUT], F32)
                nc.sync.dma_start(bt_t[:], bt_d[:])

            # norms: rsqrt(max(deg, 1))
            for deg_t, norm_t in ((dego_t, nsrc_t), (degi_t, ndst_t)):
                nc.vector.tensor_scalar(
                    norm_t[:], deg_t[:], 1.0, None, op0=mybir.AluOpType.max
                )
                nc.vector.reciprocal(norm_t[:], norm_t[:])
                nc.scalar.activation(
                    norm_t[:], norm_t[:], mybir.ActivationFunctionType.Sqrt
                )

            # internal DRAM for collectives (3:1 interleaved node split)
            _aspace = "Local" if cfg.get("NOCC") else "Shared"
            HA = 3 * NPCP // 4     # nodes with loc%4 in {0,1,2}
            HB = NPCP // 4         # nodes with loc%4 == 3
            h_my_a = dpool.tile([HA, OUT], BF16, name="h_my_a")
            h_my_b = dpool.tile([HB, OUT], BF16, name="h_my_b")
            h_all_a = dpool.tile(
                [C * HA, OUT], BF16, addr_space=_aspace, name="h_all_a"
            )
            h_all_b = dpool.tile(
                [C * HB, OUT], BF16, addr_space=_aspace, name="h_all_b"
            )
            stats_in = dpool.tile([1, 2 * OUT], F32)
            stats_out = dpool.tile([C, 2 * OUT], F32, addr_space=_aspace)

            # relu(norm*agg) output, bf16, [128, NG, OUT]
            agg_t = apool.tile([128, NG, OUT], BF16)

            # ---- stage B: h = (x @ W) * norm_src, cast bf16, store to HBM
            # (staged in SBUF; 2 large DMAs instead of 98 small ones)
            with tc.tile_pool(name="xw", bufs=1) as xwp:
                xts = []
                wts = []
                for j in range(XK):
                    xts.append(xwp.tile([128, NPCP], BF16, name=f"xt_s{j}"))
                    wts.append(xwp.tile([128, OUT], BF16, name=f"wt_s{j}"))
                for j in range(XK):
                    nc.sync.dma_start(wts[j][:], wt[j][:])
                for q in range(NXQ):
                    for j in range(XK):
                        nc.sync.dma_start(
                            xts[j][:, q * xq : (q + 1) * xq],
                            xt[j][:, q * xq : (q + 1) * xq],
                        )
                hstage = xwp.tile([128, NG, OUT], BF16, name="hstage")
                for m in range(NG):
                    ps = pbpool.tile([128, OUT], F32, tag="hps")
                    for j in range(XK):
                        nc.tensor.matmul(
                            ps[:, :],
                            xts[j][:, m * GRP : (m + 1) * GRP],
                            wts[j][:, :],
                            start=(j == 0),
                            stop=(j == XK - 1),
                        )
                    if m % 2 == 0:
                        nc.scalar.activation(
                            hstage[:, m, :],
                            ps[:, :],
                            mybir.ActivationFunctionType.Copy,
                            scale=nsrc_t[:, m : m + 1],
                        )
                    else:
                        nc.vector.tensor_scalar(
                            hstage[:, m, :],
                            ps[:, :],
                            nsrc_t[:, m : m + 1],
                            None,
                            op0=mybir.AluOpType.mult,
                        )
                # partitions c*32+q hold node loc = g*128 + 4q + c (x columns
                # host-permuted): h_my_a row g*96 + 3q + c, h_my_b row g*32+q
                hq = NG // 8
                for q8 in range(8):
                    a = q8 * hq
                    z = (q8 + 1) * hq if q8 < 7 else NG
                    va = h_my_a[a * 96 : z * 96, :].rearrange(
                        "(g q c) f -> q g c f", q=32, c=3
                    )
                    for c_ in range(3):
                        nc.sync.dma_start(
                            va[:, :, c_, :],
                            hstage[c_ * 32 : (c_ + 1) * 32, a:z, :],
                        )
                    nc.sync.dma_start(
                        h_my_b[a * 32 : z * 32, :].rearrange(
                            "(g p) f -> p g f", p=32
                        ),
                        hstage[96:128, a:z, :],
                    )

            # ---- stage C: two AllGathers (3/4 part, then 1/4 part) ----
            for h_my_h, h_all_h, hr in (
                (h_my_a, h_all_a, HA),
                (h_my_b, h_all_b, HB),
            ):
                if cfg.get("NOCC"):
                    rep = (
                        h_my_h[:]
                        .rearrange("(o r) f -> o r f", o=1)
                        .to_broadcast((C, hr, OUT))
                    )
                    nc.sync.dma_start(
                        h_all_h[:].rearrange("(o r) f -> o r f", o=C), rep
                    )
                else:
                    nc.gpsimd.collective_compute(
                        "AllGather",
                        mybir.AluOpType.bypass,
                        replica_groups=[list(range(C))],
                        ins=[h_my_h[:]],
                        outs=[h_all_h[:]],
                    )

            # index tables are first needed by stage D's gathers - load
            # them after the x/B/AG chain is underway
            nc.sync.dma_start(gidx_t[:], gidx_d[:])
            nc.sync.dma_start(doff_t[:], doff_d[:])

            # interleaved bank views: banks 0..2 -> row 3j + b of part A,
            # bank 3 -> row j of part B
            h_banks = [
                h_all_a[:].rearrange("(j k) f -> j (k f)", k=3)[
                    :, b * OUT : (b + 1) * OUT
                ]
                for b in range(3)
            ] + [h_all_b[:]]
            h_esteps = [3 * OUT, 3 * OUT, 3 * OUT, OUT]

            # ---- stage D: gather + one-hot matmul segmented sum ----
            # ---- stage E (inline): relu(psum*ndst) + BN partial sums ----
            # Gathers are batched per (chunk, bank); groups are processed
            # sequentially (their 4 bank runs back-to-back) so each PSUM bank
            # holds at most one pending accumulation group.
            ps_stat = pspool.tile([1, 2 * OUT], F32, name="ps_stat")
            ps_sum = ps_stat[:, 0:OUT]
            ps_sq = ps_stat[:, OUT : 2 * OUT]
            ndone = [0]  # groups completed (for BN-sum start/stop flags)

            def finish_group(g, ps_g):
                """relu + BN-sum accumulation for a completed group psum."""
                if b_nonzero:
                    tmp = epool.tile([128, OUT], F32, tag="etmp")
                    nc.vector.scalar_tensor_tensor(
                        tmp[:],
                        ps_g[:],
                        ndst_t[:, g : g + 1],
                        btile_t[:],
                        op0=mybir.AluOpType.mult,
                        op1=mybir.AluOpType.add,
                    )
                    nc.scalar.activation(
                        agg_t[:, g, :], tmp[:], mybir.ActivationFunctionType.Relu
                    )
                else:
                    nc.scalar.activation(
                        agg_t[:, g, :],
                        ps_g[:],
                        mybir.ActivationFunctionType.Relu,
                        scale=ndst_t[:, g : g + 1],
                    )
                ones = onesc_t if g < NG - 1 else onest_t
                i0 = ndone[0]
                # ps_sum/ps_sq share one bank = ONE accumulation group:
                # start only on the very first matmul, stop on the very last.
                nc.tensor.matmul(
                    ps_sum,
                    ones[:],
                    agg_t[:, g, :],
                    start=(i0 == 0),
                    stop=False,
                )
                sq = epool.tile([128, OUT], BF16, tag="esq")
                nc.scalar.activation(
                    sq[:], agg_t[:, g, :], mybir.ActivationFunctionType.Square
                )
                nc.tensor.matmul(
                    ps_sq,
                    ones[:],
                    sq[:],
                    start=False,
                    stop=(i0 == NG - 1),
                )
                ndone[0] += 1

            if b_nonzero:
                # replicate b across partitions once (PE broadcast)
                ps_b = pbpool.tile([128, OUT], F32, tag="hps", name="ps_b")
                btile_t = cpool.tile([128, OUT], F32)
                nc.tensor.matmul(ps_b[:], onesr_t[:], bt_t[:], start=True, stop=True)
                nc.scalar.activation(
                    btile_t[:], ps_b[:], mybir.ActivationFunctionType.Copy
                )

            run_segs = meta["run_segs"]
            chunks = meta["chunks"]
            nbmax = max(nb for _, _, nb in units)
            dstack = tc.tile_pool(name="gath", bufs=8)
            gpool = dstack.__enter__()
            phases = [(0, 1, 2), (3,)]
            ui = 0
            for pi, ph in enumerate(phases):
                for ci, ch in enumerate(chunks):
                    gts = {}
                    for b in ph:
                        bank, t0, nblk = units[ui]
                        ui += 1
                        assert bank == b
                        Gt = gpool.tile(
                            [128, nbmax, OUT], BF16, tag="G", name=f"G{pi}_{ci}_{b}"
                        )
                        nc.gpsimd.dma_gather(
                            Gt[:, :nblk, :],
                            h_banks[b],
                            gidx_t[:, t0 * 8 : (t0 + nblk) * 8],
                            nblk * 128,
                            nblk * 128,
                            OUT,
                            elem_step=h_esteps[b],
                            single_packet=False,
                        )
                        gts[b] = (Gt, t0)
                    for g in ch:
                        ps_g = pgpool.tile(
                            [128, OUT], F32, tag="aggps", name=f"ps{pi}_{g}"
                        )
                        if pi == 1:
                            # re-inject phase-A partial (spilled bf16)
                            nc.tensor.matmul(
                                ps_g[:],
                                ident_t[:],
                                agg_t[:, g, :],
                                start=True,
                                stop=False,
                            )
                        for bi, b in enumerate(ph):
                            Gt, t0 = gts[b]
                            segs = run_segs[(b, g)]
                            for si, (t, col, lo, hi) in enumerate(segs):
                                Mt = mpool.tile([128, GRP], BF16, tag="M")
                                nc.vector.tensor_scalar(
                                    Mt[:],
                                    iota_t[:],
                                    doff_t[:, col : col + 1],
                                    None,
                                    op0=mybir.AluOpType.is_equal,
                                )
                                nc.tensor.matmul(
                                    ps_g[:],
                                    Mt[:],
                                    Gt[:, t - t0, :],
                                    start=(pi == 0 and bi == 0 and si == 0),
                                    stop=(
                                        bi == len(ph) - 1
                                        and si == len(segs) - 1
                                    ),
                                )
                        if pi == 0:
                            # spill partial sum to agg_t (bf16), no relu yet
                            nc.scalar.activation(
                                agg_t[:, g, :],
                                ps_g[:],
                                mybir.ActivationFunctionType.Copy,
                            )
                        else:
                            finish_group(g, ps_g)
            dstack.__exit__(None, None, None)
            assert ndone[0] == NG

            # ---- stage F: AllReduce BN stats; build affine S/T tiles ----
            st_sb = cpool.tile([1, 2 * OUT], F32)
            nc.scalar.activation(
                st_sb[:, 0:OUT], ps_sum, mybir.ActivationFunctionType.Copy
            )
            nc.scalar.activation(
                st_sb[:, OUT : 2 * OUT], ps_sq, mybir.ActivationFunctionType.Copy
            )
            nc.sync.dma_start(stats_in[:], st_sb[:])
            if cfg.get("NOCC"):
                rep = (
                    stats_in[:]
                    .rearrange("(o r) f -> o r f", o=1)
                    .to_broadcast((C, 1, 2 * OUT))
                )
                nc.sync.dma_start(
                    stats_out[:].rearrange("(o r) f -> o r f", o=C), rep
                )
            else:
                nc.gpsimd.collective_compute(
                    "AllGather",
                    mybir.AluOpType.bypass,
                    replica_groups=[list(range(C))],
                    ins=[stats_in[:]],
                    outs=[stats_out[:]],
                )
            # per-core partials land row-major; sum the C rows locally
            st_all = cpool.tile([1, C * 2 * OUT], F32)
            nc.sync.dma_start(
                st_all[:], stats_out[:].rearrange("(o c) f -> o (c f)", o=1)
            )
            st_rb = cpool.tile([1, 2 * OUT], F32)
            nc.vector.tensor_add(
                st_rb[:], st_all[:, 0 : 2 * OUT], st_all[:, 2 * OUT : 4 * OUT]
            )
            for c_ in range(2, C):
                nc.vector.tensor_add(
                    st_rb[:],
                    st_rb[:],
                    st_all[:, c_ * 2 * OUT : (c_ + 1) * 2 * OUT],
                )

            mu = cpool.tile([1, OUT], F32)
            ex2 = cpool.tile([1, OUT], F32)
            var = cpool.tile([1, OUT], F32)
            srow = cpool.tile([1, OUT], F32)
            trow = cpool.tile([1, OUT], F32)
            inv_n = 1.0 / float(N)
            nc.scalar.activation(
                mu[:], st_rb[:, 0:OUT], mybir.ActivationFunctionType.Copy, scale=inv_n
            )
            nc.scalar.activation(
                ex2[:],
                st_rb[:, OUT : 2 * OUT],
                mybir.ActivationFunctionType.Copy,
                scale=inv_n,
            )
            nc.scalar.activation(var[:], mu[:], mybir.ActivationFunctionType.Square)
            nc.vector.tensor_sub(var[:], ex2[:], var[:])
            # var <- rsqrt(var + eps) (ACT Rsqrt is banned for accuracy)
            nc.scalar.activation(
                var[:],
                var[:],
                mybir.ActivationFunctionType.Copy,
                bias=float(cfg["EPS"]),
            )
            nc.vector.reciprocal(var[:], var[:])
            nc.scalar.activation(var[:], var[:], mybir.ActivationFunctionType.Sqrt)
            nc.vector.tensor_mul(srow[:], gm_t[:], var[:])
            nc.vector.tensor_mul(trow[:], mu[:], srow[:])
            nc.vector.tensor_sub(trow[:], bb_t[:], trow[:])

            S_t = cpool.tile([128, OUT], BF16)
            T_t = cpool.tile([128, OUT], BF16)
            ps_S = pgpool.tile([128, OUT], F32, tag="aggps", name="ps_S")
            ps_T = pgpool.tile([128, OUT], F32, tag="aggps", name="ps_T")
            nc.tensor.matmul(ps_S[:], onesr_t[:], srow[:], start=True, stop=True)
            nc.tensor.matmul(ps_T[:], onesr_t[:], trow[:], start=True, stop=True)
            nc.scalar.activation(S_t[:], ps_S[:], mybir.ActivationFunctionType.Copy)
            nc.scalar.activation(T_t[:], ps_T[:], mybir.ActivationFunctionType.Copy)

            # ---- stage G: y = hrelu * S + T (bf16), cast f32 on DMA out ----
            with tc.tile_pool(name="gtmp", bufs=2) as gpool2:
                GB = 25  # groups per batched op
                S_bc = (
                    S_t[:]
                    .rearrange("p (o f) -> p o f", o=1)
                    .to_broadcast((128, GB, OUT))
                )
                T_bc = (
                    T_t[:]
                    .rearrange("p (o f) -> p o f", o=1)
                    .to_broadcast((128, GB, OUT))
                )
                ypad_view = ypad_d[:].rearrange("(g p) f -> p g f", p=128)
                for g0 in range(0, NG, GB):
                    gw = min(GB, NG - g0)
                    tmp = gpool2.tile([128, GB, OUT], BF16, tag="gtmp")
                    nc.vector.tensor_mul(
                        tmp[:, :gw, :],
                        agg_t[:, g0 : g0 + gw, :],
                        S_bc if gw == GB else S_t[:]
                        .rearrange("p (o f) -> p o f", o=1)
                        .to_broadcast((128, gw, OUT)),
                    )
                    nc.vector.tensor_add(
                        agg_t[:, g0 : g0 + gw, :],
                        tmp[:, :gw, :],
                        T_bc if gw == GB else T_t[:]
                        .rearrange("p (o f) -> p o f", o=1)
                        .to_broadcast((128, gw, OUT)),
                    )
                    nc.gpsimd.dma_start(
                        ypad_view[:, g0 : g0 + gw, :],
                        agg_t[:, g0 : g0 + gw, :],
                    )

    nc.compile()
    return nc


def kernel(x, src, dst, W, b, gamma, beta):
    global LAST_RESULTS
    cfg = CFG
    N, E, IN, OUT, C = cfg["N"], cfg["E"], cfg["IN"], cfg["OUT"], cfg["NCORES"]
    GRP = cfg["GRP"]
    assert x.shape == (N, IN) and W.shape == (IN, OUT)
    assert src.shape == (E,) and dst.shape == (E,)

    b = np.asarray(b, np.float32)
    b_nonzero = bool(np.any(b != 0.0))
    meta, gidx_cores, dstoff_cores = _preprocess(cfg, src, dst)
    NPC, NPCP, NG = meta["NPC"], meta["NPCP"], meta["NG"]
    XK = _ceil_div(IN, 128)
    last_w = NPC - (NG - 1) * GRP
    # node permutation: within each 128-node group, order by loc%4 class
    perm = np.concatenate([np.arange(c, 128, 4) for c in range(4)])
    g_ = np.arange(NPCP) // 128
    p_ = np.arange(NPCP) % 128
    permn = g_ * 128 + perm[p_]          # source node (local) per padded col
    valid = permn < NPC

    nc = _build_nc(cfg, meta, b_nonzero=b_nonzero)

    xT = np.ascontiguousarray(np.asarray(x, np.float32).T)  # [IN, N]
    Wn = np.asarray(W, np.float32)
    import ml_dtypes

    iota = np.tile(np.arange(GRP, dtype=np.float32)[None, :], (128, 1)).astype(
        ml_dtypes.bfloat16
    )
    onesc = np.ones((128, 1), np.float32)
    onest = np.zeros((128, 1), np.float32)
    onest[:last_w] = 1.0
    onesr = np.ones((1, 128), np.float32)
    gm = np.asarray(gamma, np.float32)[None, :]
    bb = np.asarray(beta, np.float32)[None, :]

    in_maps = []
    for k in range(C):
        im = {
            "gidx": gidx_cores[k],
            "doff": dstoff_cores[k],
            "dego": _tile_major(
                np.where(
                    valid,
                    meta["deg_out"][k * NPC + np.minimum(permn, NPC - 1)],
                    np.float32(1.0),
                ).astype(np.float32),
                NG,
                GRP,
                np.float32(1.0),
            ),
            "degi": _tile_major(
                meta["deg_in"][k * NPC : (k + 1) * NPC], NG, GRP, np.float32(1.0)
            ),
            "iota": iota,
            "gm": gm,
            "bb": bb,
            "onesc": onesc.astype(ml_dtypes.bfloat16),
            "onest": onest.astype(ml_dtypes.bfloat16),
            "onesr": onesr,
            "ident": np.eye(128, dtype=np.float32).astype(ml_dtypes.bfloat16),
        }
        if b_nonzero:
            im["bt"] = b[None, :]
        for j in range(XK):
            xcols = np.zeros((128, NPCP), np.float32)
            xcols[:, valid] = xT[
                j * 128 : (j + 1) * 128, k * NPC + permn[valid]
            ]
            im[f"xt{j}"] = xcols.astype(ml_dtypes.bfloat16)
            im[f"wt{j}"] = np.ascontiguousarray(
                Wn[j * 128 : (j + 1) * 128, :]
            ).astype(ml_dtypes.bfloat16)
        in_maps.append(im)

    if cfg.get("SIM"):
        from concourse.bass_interp import MultiCoreSim

        sim = MultiCoreSim(nc, num_cores=C)
        for k, core_sim in sim.cores.items():
            for name, val in in_maps[k].items():
                core_sim.tensor(name)[:] = val
        sim.simulate()
        y = np.empty((N, OUT), np.float32)
        for k in range(C):
            y[k * NPC : (k + 1) * NPC] = sim.cores[k].tensor("ypad")[:NPC]
        return y

    global LAST_NC, LAST_RUN_S
    LAST_NC = nc
    import time as _time

    _t0 = _time.time()
    res = bass_utils.run_bass_kernel_spmd(
        nc,
        in_maps,
        core_ids=list(range(C)),
        trace=cfg.get("TRACE", False),
    )
    LAST_RUN_S = _time.time() - _t0
    LAST_RESULTS = res

    y = np.empty((N, OUT), np.float32)
    for k in range(C):
        y[k * NPC : (k + 1) * NPC] = res.results[k]["ypad"][:NPC]
    return y


# revision 41
# speedup vs baseline: 1.1989x; 1.0523x over previous
"""GCN block (GraphConv + BatchNorm1d + ReLU) on 8 Trainium2 NeuronCores.

Strategy (per sharding hint): partition nodes (and incident edges) across the
8 cores; replicate W/b/gamma/beta; all-reduce BN batch statistics.

Per core k (owns dst nodes [k*NPC, (k+1)*NPC)):
  1. h_k = (x_k @ W) * rsqrt(clip(deg_out_k,1))  (PE matmul; x columns are
     host-permuted so even nodes land on partitions 0:64, odd on 64:128).
  2. TWO AllGathers of h (bf16): even nodes, then odd nodes. Gathers that
     read even-sourced edges overlap the second collective. The 4
     int16-indexable "bank" tables are *interleaved strided views* of the
     AG outputs: bank b = nodes with (n % NPC) % 4 == b lives in half b%2
     at row 2*j + b//2, j = owner*(NPCP/4) + (n % NPC)//4 (elem_step=2
     rows). Interleaving makes bank-row -> table-row linear, so two big
     collectives (cheap) serve four int16-indexed gather tables.
  3. For each (bank, dst-group) run of edges (64-slot granular, shared
     run sizes = max over cores), gather h[src] rows (dma_gather, bf16,
     one batched gather per (phase, chunk-of-groups, bank)) and
     segment-sum them with one-hot matmuls M^T @ G accumulated in PSUM.
     Each dst group keeps ONE psum accumulation per phase; the phase-A
     partial spills to bf16 and is re-injected via an identity matmul.
     Runs straddling 128-slot block boundaries get one matmul per
     straddled block; out-of-segment slots carry doff 255 so their
     one-hot column is zero.
  4. relu(psum * rsqrt(clip(deg_in,1)) [+ b]) via ACT directly from PSUM
     (bf16 out); BN sums via ones-matmuls (single accumulation group);
     AllReduce sums; y = h*S + T with S = gamma*rsqrt(var+eps),
     T = beta - mu*S (broadcast-AP DVE ops); y cast bf16->f32 during the
     output DMA (SWDGE), pipelined per 14-group batch.

Host-side work is limited to integer index bookkeeping (bucketing edges by
(core, src-bank, dst-group), degree counting) and layout transforms (x^T
permutation/padding, int16 gather indices). All floating-point math runs on
device.

Run sizes are padded to a structure shared by all 8 cores so a single SPMD
NEFF serves every core; pad slots re-gather the run's last row (HBM page
hit) and carry a dst offset of 255 -> contribute exactly 0. Edges are
sorted by gather row within each bucket for HBM locality.
"""
import math
import os
import sys

sys.path.insert(0, "/opt/trn_rl_repo")

import numpy as np

import concourse.bacc as bacc
import concourse.bass as bass
import concourse.mybir as mybir
import concourse.tile as tile
from concourse import bass_utils

F32 = mybir.dt.float32
BF16 = mybir.dt.bfloat16
I16 = mybir.dt.int16

CFG = dict(
    N=100000,
    E=1600000,
    IN=256,
    OUT=128,
    NCORES=8,
    GRP=128,          # dst nodes per segment group (= psum partition dim)
    NBANKS=4,         # interleaved src banks (bank rows must be < 32768)
    GCHUNK=12,        # groups per chunk (gather batch granularity)
    EPS=1e-5,
    TRACE=False,
)

LAST_RESULTS = None  # set by kernel() for test harness introspection
LAST_NC = None
LAST_RUN_S = None


def _ceil_div(a, b):
    return (a + b - 1) // b


def _wrap16(idx, ncols):
    """int16 idx list -> [128, ncols] tile: idx i at [i%16, i//16], replicated
    8x across the 16-partition groups (one copy per GpSimd Q7 core)."""
    n = idx.shape[0]
    assert n == ncols * 16
    w = np.ascontiguousarray(idx.reshape(ncols, 16).T)
    return np.tile(w, (8, 1))


def _preprocess(cfg, src, dst):
    """Bucket edges by (owner core, interleaved src bank, dst group); build
    per-core gather-index / dst-offset arrays and the shared run structure."""
    N, E = cfg["N"], cfg["E"]
    C, NBANKS, GRP, GC = cfg["NCORES"], cfg["NBANKS"], cfg["GRP"], cfg["GCHUNK"]
    NPC = N // C
    NG = _ceil_div(NPC, GRP)
    NPCP = NG * GRP                # padded nodes per core (x cols zero-padded)
    assert NPCP % NBANKS == 0
    QB = NPCP // NBANKS            # gather rows per owner per bank view
    BANKROWS = QB * C              # rows per bank view of one AG-half output
    assert BANKROWS < 32768

    src = src.astype(np.int64)
    dst = dst.astype(np.int64)
    deg_out = np.bincount(src, minlength=N).astype(np.float32)
    deg_in = np.bincount(dst, minlength=N).astype(np.float32)

    owner = dst // NPC
    loc = src % NPC
    src_owner = src // NPC
    is_local = src_owner == owner  # src row available before any collective
    # bank classes: 0-3 remote (gather from AG output), 4-7 local (from
    # the core's own h tables, no owner term in the row index)
    bank = loc % NBANKS + NBANKS * is_local
    grow = np.where(is_local, loc // NBANKS, src_owner * QB + loc // NBANKS)
    assert grow.max() < 32768
    grp = (dst % NPC) // GRP
    key = (owner * 2 * NBANKS + bank) * NG + grp
    # sort by bucket, then by gather row inside the bucket (HBM locality)
    order = np.lexsort((grow, key))
    s_grow = grow[order]
    s_dst = dst[order]
    s_key = key[order]

    counts = np.bincount(key, minlength=C * 2 * NBANKS * NG).reshape(
        C, 2 * NBANKS, NG
    )
    P = counts.max(axis=0)  # [NBANKS, NG] shared run sizes (32-granular)
    P = ((P + 31) // 32) * 32
    P = np.maximum(P, 32)   # every (b,g) run structurally exists

    # local banks first (overlap the big collective), then remote 3:1
    phases = [(4, 5, 6, 7), (0, 1, 2), (3,)]
    chunks = [list(range(c, min(c + GC, NG))) for c in range(0, NG, GC)]
    run_seq = [
        (b, g) for ph in phases for ch in chunks for b in ph for g in ch
    ]
    # lay out runs; pad each (phase, chunk, bank) unit to a 128 multiple
    run_off = np.zeros((2 * NBANKS, NG), np.int64)
    units = []  # (bank, first_block, n_blocks) in stream order
    pos = 0
    for ph in phases:
        for ch in chunks:
            for b in ph:
                u0 = pos
                for g in ch:
                    run_off[b, g] = pos
                    pos += P[b, g]
                pos = ((pos + 127) // 128) * 128  # unit pad
                units.append((b, u0 // 128, (pos - u0) // 128))
    nidx_tot = int(pos)
    nb_tot = nidx_tot // 128

    # segments: a run may straddle block boundaries; each (run, block)
    # intersection is one segment = one doff column + one full matmul
    # (out-of-segment slots carry doff 255 -> zero one-hot column).
    run_segs = {}  # (b, g) -> list of (block_t, doff_col, slot_lo, slot_hi)
    nseg = 0
    for b, g in run_seq:
        off = int(run_off[b, g])
        end = off + int(P[b, g])
        segs = []
        t = off // 128
        while t * 128 < end:
            lo = max(off, t * 128)
            hi = min(end, (t + 1) * 128)
            segs.append((t, nseg, lo, hi))
            nseg += 1
            t += 1
        run_segs[(b, g)] = segs

    # boundaries of each (k, b, g) bucket in the sorted edge stream
    bkeys = (
        np.arange(C)[:, None, None] * 2 * NBANKS
        + np.arange(2 * NBANKS)[None, :, None]
    ) * NG + np.arange(NG)[None, None, :]
    starts = np.searchsorted(s_key, bkeys.ravel()).reshape(C, 2 * NBANKS, NG)
    ends = np.searchsorted(s_key, bkeys.ravel(), side="right").reshape(
        C, 2 * NBANKS, NG
    )

    gidx_cores = []
    dstoff_cores = []
    for k in range(C):
        gidx = np.zeros(nidx_tot, np.int16)
        doff_cols = np.full((nseg, 128), 255.0, np.float32)
        for b in range(2 * NBANKS):
            for g in range(NG):
                s, e = starts[k, b, g], ends[k, b, g]
                cnt = e - s
                p0 = int(run_off[b, g])
                if cnt:
                    gidx[p0 : p0 + cnt] = s_grow[s:e].astype(np.int16)
                    # pad slots re-gather the last row (HBM page hit)
                    gidx[p0 + cnt : p0 + int(P[b, g])] = gidx[p0 + cnt - 1]
                    offs = ((s_dst[s:e] % NPC) - g * GRP).astype(np.float32)
                    for t, col, lo, hi in run_segs[(b, g)]:
                        a = max(lo, p0)
                        z = min(hi, p0 + cnt)
                        if z > a:
                            doff_cols[col, a - t * 128 : z - t * 128] = offs[
                                a - p0 : z - p0
                            ]
        # unit-pad slots gather row 0 (gidx stays 0) and have no segment
        gidx_cores.append(_wrap16(gidx, nidx_tot // 16))
        dstoff_cores.append(np.ascontiguousarray(doff_cols.T))

    meta = dict(
        NPC=NPC,
        NPCP=NPCP,
        NG=NG,
        QB=QB,
        BANKROWS=BANKROWS,
        nidx_tot=nidx_tot,
        nb_tot=nb_tot,
        nseg=nseg,
        run_segs=run_segs,
        units=units,
        chunks=chunks,
        run_seq=run_seq,
        deg_out=deg_out,
        deg_in=deg_in,
    )
    return meta, gidx_cores, dstoff_cores


def _tile_major(vec, NG, GRP, pad_val):
    """[NPC] -> [GRP, NG]: entry (p, m) = vec[m*GRP + p], padded."""
    out = np.full((NG * GRP,), pad_val, vec.dtype)
    out[: vec.shape[0]] = vec
    return np.ascontiguousarray(out.reshape(NG, GRP).T)


def _build_nc(cfg, meta, b_nonzero=False):
    N, IN, OUT, C = cfg["N"], cfg["IN"], cfg["OUT"], cfg["NCORES"]
    GRP, NBANKS = cfg["GRP"], cfg["NBANKS"]
    NPC, NPCP, NG = meta["NPC"], meta["NPCP"], meta["NG"]
    nidx_tot, nb_tot = meta["nidx_tot"], meta["nb_tot"]
    units = meta["units"]
    XK = _ceil_div(IN, 128)
    assert OUT == 128 and GRP == 128
    last_w = NPC - (NG - 1) * GRP  # valid rows in the last group
    HALF = NPCP // 2               # rows per AG-half input

    nc = bacc.Bacc(
        "TRN2", target_bir_lowering=False, debug=False, num_devices=C
    )

    # ---- external inputs ----
    NXQ = 8  # x DMA split for earlier stage-B start
    xq = NPCP // NXQ
    assert NPCP % NXQ == 0
    xt = [
        nc.dram_tensor(f"xt{j}", [128, NPCP], BF16, kind="ExternalInput")
        for j in range(XK)
    ]
    wt = [
        nc.dram_tensor(f"wt{j}", [128, OUT], BF16, kind="ExternalInput")
        for j in range(XK)
    ]
    gidx_d = nc.dram_tensor("gidx", [128, nidx_tot // 16], I16, kind="ExternalInput")
    doff_d = nc.dram_tensor("doff", [128, meta["nseg"]], F32, kind="ExternalInput")
    dego_d = nc.dram_tensor("dego", [128, NG], F32, kind="ExternalInput")
    degi_d = nc.dram_tensor("degi", [128, NG], F32, kind="ExternalInput")
    iota_d = nc.dram_tensor("iota", [128, GRP], BF16, kind="ExternalInput")
    gm_d = nc.dram_tensor("gm", [1, OUT], F32, kind="ExternalInput")
    bb_d = nc.dram_tensor("bb", [1, OUT], F32, kind="ExternalInput")
    onesc_d = nc.dram_tensor("onesc", [128, 1], BF16, kind="ExternalInput")
    onest_d = nc.dram_tensor("onest", [128, 1], BF16, kind="ExternalInput")
    onesr_d = nc.dram_tensor("onesr", [1, 128], F32, kind="ExternalInput")
    ident_d = nc.dram_tensor("ident", [128, 128], BF16, kind="ExternalInput")
    if b_nonzero:
        bt_d = nc.dram_tensor("bt", [1, OUT], F32, kind="ExternalInput")

    ypad_d = nc.dram_tensor("ypad", [NG * GRP, OUT], F32, kind="ExternalOutput")

    with tile.TileContext(nc) as tc:
        with (
            tc.tile_pool(name="const", bufs=1) as cpool,
            tc.tile_pool(name="dram", bufs=1, space="DRAM") as dpool,
            tc.tile_pool(name="agg", bufs=1) as apool,
            tc.tile_pool(name="mpool", bufs=16) as mpool,
            tc.tile_pool(name="etmp", bufs=4) as epool,
            tc.tile_pool(name="psg", bufs=4, space="PSUM") as pgpool,
            tc.tile_pool(name="psb", bufs=2, space="PSUM") as pbpool,
            tc.tile_pool(name="pstat", bufs=1, space="PSUM") as pspool,
        ):
            # ---- constants / small tiles ----
            iota_t = cpool.tile([128, GRP], BF16)
            dego_t = cpool.tile([128, NG], F32)
            degi_t = cpool.tile([128, NG], F32)
            nsrc_t = cpool.tile([128, NG], F32)
            ndst_t = cpool.tile([128, NG], F32)
            gm_t = cpool.tile([1, OUT], F32)
            bb_t = cpool.tile([1, OUT], F32)
            onesc_t = cpool.tile([128, 1], BF16)
            onest_t = cpool.tile([128, 1], BF16)
            onesr_t = cpool.tile([1, 128], F32)
            gidx_t = cpool.tile([128, nidx_tot // 16], I16)
            doff_t = cpool.tile([128, meta["nseg"]], F32)
            ident_t = cpool.tile([128, 128], BF16)
            nc.sync.dma_start(ident_t[:], ident_d[:])

            nc.sync.dma_start(iota_t[:], iota_d[:])
            nc.sync.dma_start(dego_t[:], dego_d[:])
            nc.sync.dma_start(degi_t[:], degi_d[:])
            nc.sync.dma_start(gm_t[:], gm_d[:])
            nc.sync.dma_start(bb_t[:], bb_d[:])
            nc.sync.dma_start(onesc_t[:], onesc_d[:])
            nc.sync.dma_start(onest_t[:], onest_d[:])
            nc.sync.dma_start(onesr_t[:], onesr_d[:])
            if b_nonzero:
                bt_t = cpool.tile([1, OUT], F32)
                nc.sync.dma_start(bt_t[:], bt_d[:])

            # norms: rsqrt(max(deg, 1))
            for deg_t, norm_t in ((dego_t, nsrc_t), (degi_t, ndst_t)):
                nc.vector.tensor_scalar(
                    norm_t[:], deg_t[:], 1.0, None, op0=mybir.AluOpType.max
                )
                nc.vector.reciprocal(norm_t[:], norm_t[:])
                nc.scalar.activation(
                    norm_t[:], norm_t[:], mybir.ActivationFunctionType.Sqrt
                )

            # internal DRAM for collectives (3:1 interleaved node split)
            _aspace = "Local" if cfg.get("NOCC") else "Shared"
            HA = 3 * NPCP // 4     # nodes with loc%4 in {0,1,2}
            HB = NPCP // 4         # nodes with loc%4 == 3
            h_my_a = dpool.tile([HA, OUT], BF16, name="h_my_a")
            h_my_b = dpool.tile([HB, OUT], BF16, name="h_my_b")
            h_all_a = dpool.tile(
                [C * HA, OUT], BF16, addr_space=_aspace, name="h_all_a"
            )
            h_all_b = dpool.tile(
                [C * HB, OUT], BF16, addr_space=_aspace, name="h_all_b"
            )
            stats_in = dpool.tile([1, 2 * OUT], F32)
            stats_out = dpool.tile([C, 2 * OUT], F32, addr_space=_aspace)

            # relu(norm*agg) output, bf16, [128, NG, OUT]
            agg_t = apool.tile([128, NG, OUT], BF16)

            # ---- stage B: h = (x @ W) * norm_src, cast bf16, store to HBM
            # (staged in SBUF; 2 large DMAs instead of 98 small ones)
            with tc.tile_pool(name="xw", bufs=1) as xwp:
                xts = []
                wts = []
                for j in range(XK):
                    xts.append(xwp.tile([128, NPCP], BF16, name=f"xt_s{j}"))
                    wts.append(xwp.tile([128, OUT], BF16, name=f"wt_s{j}"))
                for j in range(XK):
                    nc.sync.dma_start(wts[j][:], wt[j][:])
                for q in range(NXQ):
                    for j in range(XK):
                        nc.sync.dma_start(
                            xts[j][:, q * xq : (q + 1) * xq],
                            xt[j][:, q * xq : (q + 1) * xq],
                        )
                hstage = xwp.tile([128, NG, OUT], BF16, name="hstage")
                for m in range(NG):
                    ps = pbpool.tile([128, OUT], F32, tag="hps")
                    for j in range(XK):
                        nc.tensor.matmul(
                            ps[:, :],
                            xts[j][:, m * GRP : (m + 1) * GRP],
                            wts[j][:, :],
                            start=(j == 0),
                            stop=(j == XK - 1),
                        )
                    if m % 2 == 0:
                        nc.scalar.activation(
                            hstage[:, m, :],
                            ps[:, :],
                            mybir.ActivationFunctionType.Copy,
                            scale=nsrc_t[:, m : m + 1],
                        )
                    else:
                        nc.vector.tensor_scalar(
                            hstage[:, m, :],
                            ps[:, :],
                            nsrc_t[:, m : m + 1],
                            None,
                            op0=mybir.AluOpType.mult,
                        )
                # partitions c*32+q hold node loc = g*128 + 4q + c (x columns
                # host-permuted): h_my_a row g*96 + 3q + c, h_my_b row g*32+q
                hq = NG // 4
                for q8 in range(4):
                    a = q8 * hq
                    z = (q8 + 1) * hq if q8 < 3 else NG
                    va = h_my_a[a * 96 : z * 96, :].rearrange(
                        "(g q c) f -> q g c f", q=32, c=3
                    )
                    for c_ in range(3):
                        nc.sync.dma_start(
                            va[:, :, c_, :],
                            hstage[c_ * 32 : (c_ + 1) * 32, a:z, :],
                        )
                    nc.sync.dma_start(
                        h_my_b[a * 32 : z * 32, :].rearrange(
                            "(g p) f -> p g f", p=32
                        ),
                        hstage[96:128, a:z, :],
                    )

            # ---- stage C: two AllGathers (3/4 part, then 1/4 part) ----
            for h_my_h, h_all_h, hr in (
                (h_my_a, h_all_a, HA),
                (h_my_b, h_all_b, HB),
            ):
                if cfg.get("NOCC"):
                    rep = (
                        h_my_h[:]
                        .rearrange("(o r) f -> o r f", o=1)
                        .to_broadcast((C, hr, OUT))
                    )
                    nc.sync.dma_start(
                        h_all_h[:].rearrange("(o r) f -> o r f", o=C), rep
                    )
                else:
                    nc.gpsimd.collective_compute(
                        "AllGather",
                        mybir.AluOpType.bypass,
                        replica_groups=[list(range(C))],
                        ins=[h_my_h[:]],
                        outs=[h_all_h[:]],
                    )

            # index tables are first needed by stage D's gathers - load
            # them after the x/B/AG chain is underway
            nc.sync.dma_start(gidx_t[:], gidx_d[:])
            nc.sync.dma_start(doff_t[:], doff_d[:])

            # interleaved bank views: banks 0..2 -> row 3j + b of part A,
            # bank 3 -> row j of part B; banks 4..7 -> same views over the
            # core's OWN h tables (usable before any collective completes)
            h_banks = [
                h_all_a[:].rearrange("(j k) f -> j (k f)", k=3)[
                    :, b * OUT : (b + 1) * OUT
                ]
                for b in range(3)
            ] + [h_all_b[:]] + [
                h_my_a[:].rearrange("(j k) f -> j (k f)", k=3)[
                    :, b * OUT : (b + 1) * OUT
                ]
                for b in range(3)
            ] + [h_my_b[:]]
            h_esteps = [3 * OUT, 3 * OUT, 3 * OUT, OUT] * 2

            # ---- stage D: gather + one-hot matmul segmented sum ----
            # ---- stage E (inline): relu(psum*ndst) + BN partial sums ----
            # Gathers are batched per (chunk, bank); groups are processed
            # sequentially (their 4 bank runs back-to-back) so each PSUM bank
            # holds at most one pending accumulation group.
            ps_stat = pspool.tile([1, 2 * OUT], F32, name="ps_stat")
            ps_sum = ps_stat[:, 0:OUT]
            ps_sq = ps_stat[:, OUT : 2 * OUT]
            ndone = [0]  # groups completed (for BN-sum start/stop flags)

            def finish_group(g, ps_g):
                """relu + BN-sum accumulation for a completed group psum."""
                if b_nonzero:
                    tmp = epool.tile([128, OUT], F32, tag="etmp")
                    nc.vector.scalar_tensor_tensor(
                        tmp[:],
                        ps_g[:],
                        ndst_t[:, g : g + 1],
                        btile_t[:],
                        op0=mybir.AluOpType.mult,
                        op1=mybir.AluOpType.add,
                    )
                    nc.scalar.activation(
                        agg_t[:, g, :], tmp[:], mybir.ActivationFunctionType.Relu
                    )
                else:
                    nc.scalar.activation(
                        agg_t[:, g, :],
                        ps_g[:],
                        mybir.ActivationFunctionType.Relu,
                        scale=ndst_t[:, g : g + 1],
                    )
                ones = onesc_t if g < NG - 1 else onest_t
                i0 = ndone[0]
                # ps_sum/ps_sq share one bank = ONE accumulation group:
                # start only on the very first matmul, stop on the very last.
                nc.tensor.matmul(
                    ps_sum,
                    ones[:],
                    agg_t[:, g, :],
                    start=(i0 == 0),
                    stop=False,
                )
                sq = epool.tile([128, OUT], BF16, tag="esq")
                nc.scalar.activation(
                    sq[:], agg_t[:, g, :], mybir.ActivationFunctionType.Square
                )
                nc.tensor.matmul(
                    ps_sq,
                    ones[:],
                    sq[:],
                    start=False,
                    stop=(i0 == NG - 1),
                )
                ndone[0] += 1

            if b_nonzero:
                # replicate b across partitions once (PE broadcast)
                ps_b = pbpool.tile([128, OUT], F32, tag="hps", name="ps_b")
                btile_t = cpool.tile([128, OUT], F32)
                nc.tensor.matmul(ps_b[:], onesr_t[:], bt_t[:], start=True, stop=True)
                nc.scalar.activation(
                    btile_t[:], ps_b[:], mybir.ActivationFunctionType.Copy
                )

            run_segs = meta["run_segs"]
            chunks = meta["chunks"]
            nbmax = max(nb for _, _, nb in units)
            dstack = tc.tile_pool(name="gath", bufs=8)
            gpool = dstack.__enter__()
            phases = [(4, 5, 6, 7), (0, 1, 2), (3,)]
            ui = 0
            for pi, ph in enumerate(phases):
                for ci, ch in enumerate(chunks):
                    gts = {}
                    for b in ph:
                        bank, t0, nblk = units[ui]
                        ui += 1
                        assert bank == b
                        Gt = gpool.tile(
                            [128, nbmax, OUT], BF16, tag="G", name=f"G{pi}_{ci}_{b}"
                        )
                        nc.gpsimd.dma_gather(
                            Gt[:, :nblk, :],
                            h_banks[b],
                            gidx_t[:, t0 * 8 : (t0 + nblk) * 8],
                            nblk * 128,
                            nblk * 128,
                            OUT,
                            elem_step=h_esteps[b],
                            single_packet=False,
                        )
                        gts[b] = (Gt, t0)
                    for g in ch:
                        ps_g = pgpool.tile(
                            [128, OUT], F32, tag="aggps", name=f"ps{pi}_{g}"
                        )
                        if pi > 0:
                            # re-inject previous phase's partial (bf16)
                            nc.tensor.matmul(
                                ps_g[:],
                                ident_t[:],
                                agg_t[:, g, :],
                                start=True,
                                stop=False,
                            )
                        for bi, b in enumerate(ph):
                            Gt, t0 = gts[b]
                            segs = run_segs[(b, g)]
                            for si, (t, col, lo, hi) in enumerate(segs):
                                Mt = mpool.tile([128, GRP], BF16, tag="M")
                                nc.vector.tensor_scalar(
                                    Mt[:],
                                    iota_t[:],
                                    doff_t[:, col : col + 1],
                                    None,
                                    op0=mybir.AluOpType.is_equal,
                                )
                                nc.tensor.matmul(
                                    ps_g[:],
                                    Mt[:],
                                    Gt[:, t - t0, :],
                                    start=(pi == 0 and bi == 0 and si == 0),
                                    stop=(
                                        bi == len(ph) - 1
                                        and si == len(segs) - 1
                                    ),
                                )
                        if pi < len(phases) - 1:
                            # spill partial sum to agg_t (bf16), no relu yet
                            nc.scalar.activation(
                                agg_t[:, g, :],
                                ps_g[:],
                                mybir.ActivationFunctionType.Copy,
                            )
                        else:
                            finish_group(g, ps_g)
            dstack.__exit__(None, None, None)
            assert ndone[0] == NG

            # ---- stage F: AllReduce BN stats; build affine S/T tiles ----
            st_sb = cpool.tile([1, 2 * OUT], F32)
            nc.scalar.activation(
                st_sb[:, 0:OUT], ps_sum, mybir.ActivationFunctionType.Copy
            )
            nc.scalar.activation(
                st_sb[:, OUT : 2 * OUT], ps_sq, mybir.ActivationFunctionType.Copy
            )
            nc.sync.dma_start(stats_in[:], st_sb[:])
            if cfg.get("NOCC"):
                rep = (
                    stats_in[:]
                    .rearrange("(o r) f -> o r f", o=1)
                    .to_broadcast((C, 1, 2 * OUT))
                )
                nc.sync.dma_start(
                    stats_out[:].rearrange("(o r) f -> o r f", o=C), rep
                )
            else:
                nc.gpsimd.collective_compute(
                    "AllGather",
                    mybir.AluOpType.bypass,
                    replica_groups=[list(range(C))],
                    ins=[stats_in[:]],
                    outs=[stats_out[:]],
                )
            # per-core partials land row-major; sum the C rows locally
            st_all = cpool.tile([1, C * 2 * OUT], F32)
            nc.sync.dma_start(
                st_all[:], stats_out[:].rearrange("(o c) f -> o (c f)", o=1)
            )
            st_rb = cpool.tile([1, 2 * OUT], F32)
            nc.vector.tensor_add(
                st_rb[:], st_all[:, 0 : 2 * OUT], st_all[:, 2 * OUT : 4 * OUT]
            )
            for c_ in range(2, C):
                nc.vector.tensor_add(
                    st_rb[:],
                    st_rb[:],
                    st_all[:, c_ * 2 * OUT : (c_ + 1) * 2 * OUT],
                )

            mu = cpool.tile([1, OUT], F32)
            ex2 = cpool.tile([1, OUT], F32)
            var = cpool.tile([1, OUT], F32)
            srow = cpool.tile([1, OUT], F32)
            trow = cpool.tile([1, OUT], F32)
            inv_n = 1.0 / float(N)
            nc.scalar.activation(
                mu[:], st_rb[:, 0:OUT], mybir.ActivationFunctionType.Copy, scale=inv_n
            )
            nc.scalar.activation(
                ex2[:],
                st_rb[:, OUT : 2 * OUT],
                mybir.ActivationFunctionType.Copy,
                scale=inv_n,
            )
            nc.scalar.activation(var[:], mu[:], mybir.ActivationFunctionType.Square)
            nc.vector.tensor_sub(var[:], ex2[:], var[:])
            # var <- rsqrt(var + eps) (ACT Rsqrt is banned for accuracy)
            nc.scalar.activation(
                var[:],
                var[:],
                mybir.ActivationFunctionType.Copy,
                bias=float(cfg["EPS"]),
            )
            nc.vector.reciprocal(var[:], var[:])
            nc.scalar.activation(var[:], var[:], mybir.ActivationFunctionType.Sqrt)
            nc.vector.tensor_mul(srow[:], gm_t[:], var[:])
            nc.vector.tensor_mul(trow[:], mu[:], srow[:])
            nc.vector.tensor_sub(trow[:], bb_t[:], trow[:])

            S_t = cpool.tile([128, OUT], BF16)
            T_t = cpool.tile([128, OUT], BF16)
            ps_S = pgpool.tile([128, OUT], F32, tag="aggps", name="ps_S")
            ps_T = pgpool.tile([128, OUT], F32, tag="aggps", name="ps_T")
            nc.tensor.matmul(ps_S[:], onesr_t[:], srow[:], start=True, stop=True)
            nc.tensor.matmul(ps_T[:], onesr_t[:], trow[:], start=True, stop=True)
            nc.scalar.activation(S_t[:], ps_S[:], mybir.ActivationFunctionType.Copy)
            nc.scalar.activation(T_t[:], ps_T[:], mybir.ActivationFunctionType.Copy)

            # ---- stage G: y = hrelu * S + T (bf16), cast f32 on DMA out ----
            with tc.tile_pool(name="gtmp", bufs=2) as gpool2:
                GB = 25  # groups per batched op
                S_bc = (
                    S_t[:]
                    .rearrange("p (o f) -> p o f", o=1)
                    .to_broadcast((128, GB, OUT))
                )
                T_bc = (
                    T_t[:]
                    .rearrange("p (o f) -> p o f", o=1)
                    .to_broadcast((128, GB, OUT))
                )
                ypad_view = ypad_d[:].rearrange("(g p) f -> p g f", p=128)
                for g0 in range(0, NG, GB):
                    gw = min(GB, NG - g0)
                    tmp = gpool2.tile([128, GB, OUT], BF16, tag="gtmp")
                    nc.vector.tensor_mul(
                        tmp[:, :gw, :],
                        agg_t[:, g0 : g0 + gw, :],
                        S_bc if gw == GB else S_t[:]
                        .rearrange("p (o f) -> p o f", o=1)
                        .to_broadcast((128, gw, OUT)),
                    )
                    nc.vector.tensor_add(
                        agg_t[:, g0 : g0 + gw, :],
                        tmp[:, :gw, :],
                        T_bc if gw == GB else T_t[:]
                        .rearrange("p (o f) -> p o f", o=1)
                        .to_broadcast((128, gw, OUT)),
                    )
                    nc.gpsimd.dma_start(
                        ypad_view[:, g0 : g0 + gw, :],
                        agg_t[:, g0 : g0 + gw, :],
                    )

    nc.compile()
    return nc


def kernel(x, src, dst, W, b, gamma, beta):
    global LAST_RESULTS
    cfg = CFG
    N, E, IN, OUT, C = cfg["N"], cfg["E"], cfg["IN"], cfg["OUT"], cfg["NCORES"]
    GRP = cfg["GRP"]
    assert x.shape == (N, IN) and W.shape == (IN, OUT)
    assert src.shape == (E,) and dst.shape == (E,)

    b = np.asarray(b, np.float32)
    b_nonzero = bool(np.any(b != 0.0))
    meta, gidx_cores, dstoff_cores = _preprocess(cfg, src, dst)
    NPC, NPCP, NG = meta["NPC"], meta["NPCP"], meta["NG"]
    XK = _ceil_div(IN, 128)
    last_w = NPC - (NG - 1) * GRP
    # node permutation: within each 128-node group, order by loc%4 class
    perm = np.concatenate([np.arange(c, 128, 4) for c in range(4)])
    g_ = np.arange(NPCP) // 128
    p_ = np.arange(NPCP) % 128
    permn = g_ * 128 + perm[p_]          # source node (local) per padded col
    valid = permn < NPC

    nc = _build_nc(cfg, meta, b_nonzero=b_nonzero)

    xT = np.ascontiguousarray(np.asarray(x, np.float32).T)  # [IN, N]
    Wn = np.asarray(W, np.float32)
    import ml_dtypes

    iota = np.tile(np.arange(GRP, dtype=np.float32)[None, :], (128, 1)).astype(
        ml_dtypes.bfloat16
    )
    onesc = np.ones((128, 1), np.float32)
    onest = np.zeros((128, 1), np.float32)
    onest[:last_w] = 1.0
    onesr = np.ones((1, 128), np.float32)
    gm = np.asarray(gamma, np.float32)[None, :]
    bb = np.asarray(beta, np.float32)[None, :]

    in_maps = []
    for k in range(C):
        im = {
            "gidx": gidx_cores[k],
            "doff": dstoff_cores[k],
            "dego": _tile_major(
                np.where(
                    valid,
                    meta["deg_out"][k * NPC + np.minimum(permn, NPC - 1)],
                    np.float32(1.0),
                ).astype(np.float32),
                NG,
                GRP,
                np.float32(1.0),
            ),
            "degi": _tile_major(
                meta["deg_in"][k * NPC : (k + 1) * NPC], NG, GRP, np.float32(1.0)
            ),
            "iota": iota,
            "gm": gm,
            "bb": bb,
            "onesc": onesc.astype(ml_dtypes.bfloat16),
            "onest": onest.astype(ml_dtypes.bfloat16),
            "onesr": onesr,
            "ident": np.eye(128, dtype=np.float32).astype(ml_dtypes.bfloat16),
        }
        if b_nonzero:
            im["bt"] = b[None, :]
        for j in range(XK):
            xcols = np.zeros((128, NPCP), np.float32)
            xcols[:, valid] = xT[
                j * 128 : (j + 1) * 128, k * NPC + permn[valid]
            ]
            im[f"xt{j}"] = xcols.astype(ml_dtypes.bfloat16)
            im[f"wt{j}"] = np.ascontiguousarray(
                Wn[j * 128 : (j + 1) * 128, :]
            ).astype(ml_dtypes.bfloat16)
        in_maps.append(im)

    if cfg.get("SIM"):
        from concourse.bass_interp import MultiCoreSim

        sim = MultiCoreSim(nc, num_cores=C)
        for k, core_sim in sim.cores.items():
            for name, val in in_maps[k].items():
                core_sim.tensor(name)[:] = val
        sim.simulate()
        y = np.empty((N, OUT), np.float32)
        for k in range(C):
            y[k * NPC : (k + 1) * NPC] = sim.cores[k].tensor("ypad")[:NPC]
        return y

    global LAST_NC, LAST_RUN_S
    LAST_NC = nc
    import time as _time

    _t0 = _time.time()
    res = bass_utils.run_bass_kernel_spmd(
        nc,
        in_maps,
        core_ids=list(range(C)),
        trace=cfg.get("TRACE", False),
    )
    LAST_RUN_S = _time.time() - _t0
    LAST_RESULTS = res

    y = np.empty((N, OUT), np.float32)
    for k in range(C):
        y[k * NPC : (k + 1) * NPC] = res.results[k]["ypad"][:NPC]
    return y


# revision 43
# speedup vs baseline: 1.2211x; 1.0185x over previous
"""GCN block (GraphConv + BatchNorm1d + ReLU) on 8 Trainium2 NeuronCores.

Strategy (per sharding hint): partition nodes (and incident edges) across the
8 cores; replicate W/b/gamma/beta; all-reduce BN batch statistics.

Per core k (owns dst nodes [k*NPC, (k+1)*NPC)):
  1. h_k = (x_k @ W) * rsqrt(clip(deg_out_k,1))  (PE matmul; x columns are
     host-permuted so nodes land on partitions grouped by loc%4 class).
  2. TWO AllGathers of h (bf16), split 3:1: nodes with loc%4 in {0,1,2},
     then loc%4 == 3. The int16-indexable "bank" tables are *interleaved
     strided views* of the AG outputs: bank b < 3 lives at row 3j + b of
     part A (elem_step=3), bank 3 at row j of part B, with
     j = owner*(NPCP/4) + loc//4. The linearity holds for any k-of-4
     interleave, so two big collectives serve four gather tables.
  3. Edges are processed in three phases: (0) edges whose source is owned
     by this core gather from the core's OWN h tables and run inside the
     first collective's window; (1) remote banks {0,1,2} after AG part A
     (75% of remote work, hiding AG part B); (2) remote bank 3. Gathers
     are batched per (phase, chunk-of-groups, bank); 32-granular shared
     run sizes (= max over cores). Segment sums use one-hot matmuls
     M^T @ G accumulated in PSUM; each group keeps ONE psum accumulation
     per phase, spilled to bf16 and re-injected via an identity matmul at
     the next phase. Runs straddling 128-slot block boundaries get one
     matmul per straddled block; out-of-segment slots carry doff 255 so
     their one-hot column is zero.
  4. relu(psum * rsqrt(clip(deg_in,1)) [+ b]) via ACT directly from PSUM
     (bf16 out); BN sums via ones-matmuls (single accumulation group per
     PSUM bank); stats combined via AllGather + local sum; y = h*S + T
     with S = gamma*rsqrt(var+eps), T = beta - mu*S (broadcast-AP DVE
     ops); y cast bf16->f32 during the output DMA (SWDGE), pipelined per
     25-group batch.

Host-side work is limited to integer index bookkeeping (bucketing edges by
(core, locality, src-bank, dst-group), degree counting) and layout
transforms (x^T permutation/padding, int16 gather indices). All
floating-point math runs on device.

Run sizes are padded to a structure shared by all 8 cores so a single SPMD
NEFF serves every core; pad slots re-gather the run's last row (HBM page
hit) and carry a dst offset of 255 -> contribute exactly 0. Edges are
sorted by gather row within each bucket for HBM locality.
"""
import math
import os
import sys

sys.path.insert(0, "/opt/trn_rl_repo")

import numpy as np

import concourse.bacc as bacc
import concourse.bass as bass
import concourse.mybir as mybir
import concourse.tile as tile
from concourse import bass_utils

F32 = mybir.dt.float32
BF16 = mybir.dt.bfloat16
I16 = mybir.dt.int16

CFG = dict(
    N=100000,
    E=1600000,
    IN=256,
    OUT=128,
    NCORES=8,
    GRP=128,          # dst nodes per segment group (= psum partition dim)
    NBANKS=4,         # interleaved src banks (bank rows must be < 32768)
    GCHUNK=12,        # groups per chunk (gather batch granularity)
    EPS=1e-5,
    TRACE=False,
)

LAST_RESULTS = None  # set by kernel() for test harness introspection
LAST_NC = None
LAST_RUN_S = None


def _ceil_div(a, b):
    return (a + b - 1) // b


def _wrap16(idx, ncols):
    """int16 idx list -> [128, ncols] tile: idx i at [i%16, i//16], replicated
    8x across the 16-partition groups (one copy per GpSimd Q7 core)."""
    n = idx.shape[0]
    assert n == ncols * 16
    w = np.ascontiguousarray(idx.reshape(ncols, 16).T)
    return np.tile(w, (8, 1))


def _preprocess(cfg, src, dst):
    """Bucket edges by (owner core, interleaved src bank, dst group); build
    per-core gather-index / dst-offset arrays and the shared run structure."""
    N, E = cfg["N"], cfg["E"]
    C, NBANKS, GRP, GC = cfg["NCORES"], cfg["NBANKS"], cfg["GRP"], cfg["GCHUNK"]
    NPC = N // C
    NG = _ceil_div(NPC, GRP)
    NPCP = NG * GRP                # padded nodes per core (x cols zero-padded)
    assert NPCP % NBANKS == 0
    QB = NPCP // NBANKS            # gather rows per owner per bank view
    BANKROWS = QB * C              # rows per bank view of one AG-half output
    assert BANKROWS < 32768

    src = src.astype(np.int64)
    dst = dst.astype(np.int64)
    deg_out = np.bincount(src, minlength=N).astype(np.float32)
    deg_in = np.bincount(dst, minlength=N).astype(np.float32)

    owner = dst // NPC
    loc = src % NPC
    src_owner = src // NPC
    is_local = src_owner == owner  # src row available before any collective
    # bank classes: 0-3 remote (gather from AG output), 4-7 local (from
    # the core's own h tables, no owner term in the row index)
    bank = loc % NBANKS + NBANKS * is_local
    grow = np.where(is_local, loc // NBANKS, src_owner * QB + loc // NBANKS)
    assert grow.max() < 32768
    grp = (dst % NPC) // GRP
    key = (owner * 2 * NBANKS + bank) * NG + grp
    # sort by bucket, then by gather row inside the bucket (HBM locality)
    order = np.lexsort((grow, key))
    s_grow = grow[order]
    s_dst = dst[order]
    s_key = key[order]

    counts = np.bincount(key, minlength=C * 2 * NBANKS * NG).reshape(
        C, 2 * NBANKS, NG
    )
    P = counts.max(axis=0)  # [NBANKS, NG] shared run sizes (32-granular)
    P = ((P + 31) // 32) * 32
    P = np.maximum(P, 32)   # every (b,g) run structurally exists

    # local banks first (overlap the big collective), then remote 3:1
    phases = [(4, 5, 6, 7), (0, 1, 2), (3,)]
    chunks = [list(range(c, min(c + GC, NG))) for c in range(0, NG, GC)]
    run_seq = [
        (b, g) for ph in phases for ch in chunks for b in ph for g in ch
    ]
    # lay out runs; pad each (phase, chunk, bank) unit to a 128 multiple
    run_off = np.zeros((2 * NBANKS, NG), np.int64)
    units = []  # (bank, first_block, n_blocks) in stream order
    pos = 0
    for ph in phases:
        for ch in chunks:
            for b in ph:
                u0 = pos
                for g in ch:
                    run_off[b, g] = pos
                    pos += P[b, g]
                pos = ((pos + 127) // 128) * 128  # unit pad
                units.append((b, u0 // 128, (pos - u0) // 128))
    nidx_tot = int(pos)
    nb_tot = nidx_tot // 128

    # segments: a run may straddle block boundaries; each (run, block)
    # intersection is one segment = one doff column + one full matmul
    # (out-of-segment slots carry doff 255 -> zero one-hot column).
    run_segs = {}  # (b, g) -> list of (block_t, doff_col, slot_lo, slot_hi)
    nseg = 0
    for b, g in run_seq:
        off = int(run_off[b, g])
        end = off + int(P[b, g])
        segs = []
        t = off // 128
        while t * 128 < end:
            lo = max(off, t * 128)
            hi = min(end, (t + 1) * 128)
            segs.append((t, nseg, lo, hi))
            nseg += 1
            t += 1
        run_segs[(b, g)] = segs

    # boundaries of each (k, b, g) bucket in the sorted edge stream
    bkeys = (
        np.arange(C)[:, None, None] * 2 * NBANKS
        + np.arange(2 * NBANKS)[None, :, None]
    ) * NG + np.arange(NG)[None, None, :]
    starts = np.searchsorted(s_key, bkeys.ravel()).reshape(C, 2 * NBANKS, NG)
    ends = np.searchsorted(s_key, bkeys.ravel(), side="right").reshape(
        C, 2 * NBANKS, NG
    )

    gidx_cores = []
    dstoff_cores = []
    for k in range(C):
        gidx = np.zeros(nidx_tot, np.int16)
        doff_cols = np.full((nseg, 128), 255.0, np.float32)
        for b in range(2 * NBANKS):
            for g in range(NG):
                s, e = starts[k, b, g], ends[k, b, g]
                cnt = e - s
                p0 = int(run_off[b, g])
                if cnt:
                    gidx[p0 : p0 + cnt] = s_grow[s:e].astype(np.int16)
                    # pad slots re-gather the last row (HBM page hit)
                    gidx[p0 + cnt : p0 + int(P[b, g])] = gidx[p0 + cnt - 1]
                    offs = ((s_dst[s:e] % NPC) - g * GRP).astype(np.float32)
                    for t, col, lo, hi in run_segs[(b, g)]:
                        a = max(lo, p0)
                        z = min(hi, p0 + cnt)
                        if z > a:
                            doff_cols[col, a - t * 128 : z - t * 128] = offs[
                                a - p0 : z - p0
                            ]
        # unit-pad slots gather row 0 (gidx stays 0) and have no segment
        gidx_cores.append(_wrap16(gidx, nidx_tot // 16))
        dstoff_cores.append(np.ascontiguousarray(doff_cols.T))

    meta = dict(
        NPC=NPC,
        NPCP=NPCP,
        NG=NG,
        QB=QB,
        BANKROWS=BANKROWS,
        nidx_tot=nidx_tot,
        nb_tot=nb_tot,
        nseg=nseg,
        run_segs=run_segs,
        units=units,
        chunks=chunks,
        run_seq=run_seq,
        deg_out=deg_out,
        deg_in=deg_in,
    )
    return meta, gidx_cores, dstoff_cores


def _tile_major(vec, NG, GRP, pad_val):
    """[NPC] -> [GRP, NG]: entry (p, m) = vec[m*GRP + p], padded."""
    out = np.full((NG * GRP,), pad_val, vec.dtype)
    out[: vec.shape[0]] = vec
    return np.ascontiguousarray(out.reshape(NG, GRP).T)


def _build_nc(cfg, meta, b_nonzero=False):
    N, IN, OUT, C = cfg["N"], cfg["IN"], cfg["OUT"], cfg["NCORES"]
    GRP, NBANKS = cfg["GRP"], cfg["NBANKS"]
    NPC, NPCP, NG = meta["NPC"], meta["NPCP"], meta["NG"]
    nidx_tot, nb_tot = meta["nidx_tot"], meta["nb_tot"]
    units = meta["units"]
    XK = _ceil_div(IN, 128)
    assert OUT == 128 and GRP == 128
    last_w = NPC - (NG - 1) * GRP  # valid rows in the last group
    HALF = NPCP // 2               # rows per AG-half input

    nc = bacc.Bacc(
        "TRN2", target_bir_lowering=False, debug=False, num_devices=C
    )

    # ---- external inputs ----
    NXQ = 8  # x DMA split for earlier stage-B start
    xq = NPCP // NXQ
    assert NPCP % NXQ == 0
    xt = [
        nc.dram_tensor(f"xt{j}", [128, NPCP], BF16, kind="ExternalInput")
        for j in range(XK)
    ]
    wt = [
        nc.dram_tensor(f"wt{j}", [128, OUT], BF16, kind="ExternalInput")
        for j in range(XK)
    ]
    gidx_d = nc.dram_tensor("gidx", [128, nidx_tot // 16], I16, kind="ExternalInput")
    doff_d = nc.dram_tensor("doff", [128, meta["nseg"]], F32, kind="ExternalInput")
    dego_d = nc.dram_tensor("dego", [128, NG], F32, kind="ExternalInput")
    degi_d = nc.dram_tensor("degi", [128, NG], F32, kind="ExternalInput")
    iota_d = nc.dram_tensor("iota", [128, GRP], BF16, kind="ExternalInput")
    gm_d = nc.dram_tensor("gm", [1, OUT], F32, kind="ExternalInput")
    bb_d = nc.dram_tensor("bb", [1, OUT], F32, kind="ExternalInput")
    onesc_d = nc.dram_tensor("onesc", [128, 1], BF16, kind="ExternalInput")
    onest_d = nc.dram_tensor("onest", [128, 1], BF16, kind="ExternalInput")
    onesr_d = nc.dram_tensor("onesr", [1, 128], F32, kind="ExternalInput")
    ident_d = nc.dram_tensor("ident", [128, 128], BF16, kind="ExternalInput")
    if b_nonzero:
        bt_d = nc.dram_tensor("bt", [1, OUT], F32, kind="ExternalInput")

    ypad_d = nc.dram_tensor("ypad", [NG * GRP, OUT], F32, kind="ExternalOutput")

    with tile.TileContext(nc) as tc:
        with (
            tc.tile_pool(name="const", bufs=1) as cpool,
            tc.tile_pool(name="dram", bufs=1, space="DRAM") as dpool,
            tc.tile_pool(name="agg", bufs=1) as apool,
            tc.tile_pool(name="mpool", bufs=16) as mpool,
            tc.tile_pool(name="etmp", bufs=4) as epool,
            tc.tile_pool(name="psg", bufs=4, space="PSUM") as pgpool,
            tc.tile_pool(name="psb", bufs=3, space="PSUM") as pbpool,
            tc.tile_pool(name="pstat", bufs=1, space="PSUM") as pspool,
        ):
            # ---- constants / small tiles ----
            iota_t = cpool.tile([128, GRP], BF16)
            dego_t = cpool.tile([128, NG], F32)
            degi_t = cpool.tile([128, NG], F32)
            nsrc_t = cpool.tile([128, NG], F32)
            ndst_t = cpool.tile([128, NG], F32)
            gm_t = cpool.tile([1, OUT], F32)
            bb_t = cpool.tile([1, OUT], F32)
            onesc_t = cpool.tile([128, 1], BF16)
            onest_t = cpool.tile([128, 1], BF16)
            onesr_t = cpool.tile([1, 128], F32)
            gidx_t = cpool.tile([128, nidx_tot // 16], I16)
            doff_t = cpool.tile([128, meta["nseg"]], F32)
            ident_t = cpool.tile([128, 128], BF16)
            nc.sync.dma_start(ident_t[:], ident_d[:])

            nc.sync.dma_start(iota_t[:], iota_d[:])
            nc.sync.dma_start(dego_t[:], dego_d[:])
            nc.sync.dma_start(degi_t[:], degi_d[:])
            nc.sync.dma_start(gm_t[:], gm_d[:])
            nc.sync.dma_start(bb_t[:], bb_d[:])
            nc.sync.dma_start(onesc_t[:], onesc_d[:])
            nc.sync.dma_start(onest_t[:], onest_d[:])
            nc.sync.dma_start(onesr_t[:], onesr_d[:])
            if b_nonzero:
                bt_t = cpool.tile([1, OUT], F32)
                nc.sync.dma_start(bt_t[:], bt_d[:])

            # norms: rsqrt(max(deg, 1))
            for deg_t, norm_t in ((dego_t, nsrc_t), (degi_t, ndst_t)):
                nc.vector.tensor_scalar(
                    norm_t[:], deg_t[:], 1.0, None, op0=mybir.AluOpType.max
                )
                nc.vector.reciprocal(norm_t[:], norm_t[:])
                nc.scalar.activation(
                    norm_t[:], norm_t[:], mybir.ActivationFunctionType.Sqrt
                )

            # internal DRAM for collectives (3:1 interleaved node split)
            _aspace = "Local" if cfg.get("NOCC") else "Shared"
            HA = 3 * NPCP // 4     # nodes with loc%4 in {0,1,2}
            HB = NPCP // 4         # nodes with loc%4 == 3
            h_my_a = dpool.tile([HA, OUT], BF16, name="h_my_a")
            h_my_b = dpool.tile([HB, OUT], BF16, name="h_my_b")
            h_all_a = dpool.tile(
                [C * HA, OUT], BF16, addr_space=_aspace, name="h_all_a"
            )
            h_all_b = dpool.tile(
                [C * HB, OUT], BF16, addr_space=_aspace, name="h_all_b"
            )
            stats_in = dpool.tile([1, 2 * OUT], F32)
            stats_out = dpool.tile([C, 2 * OUT], F32, addr_space=_aspace)

            # relu(norm*agg) output, bf16, [128, NG, OUT]
            agg_t = apool.tile([128, NG, OUT], BF16)

            # ---- stage B: h = (x @ W) * norm_src, cast bf16, store to HBM
            # (staged in SBUF; 2 large DMAs instead of 98 small ones)
            with tc.tile_pool(name="xw", bufs=1) as xwp:
                xts = []
                wts = []
                for j in range(XK):
                    xts.append(xwp.tile([128, NPCP], BF16, name=f"xt_s{j}"))
                    wts.append(xwp.tile([128, OUT], BF16, name=f"wt_s{j}"))
                for j in range(XK):
                    nc.sync.dma_start(wts[j][:], wt[j][:])
                for q in range(NXQ):
                    for j in range(XK):
                        nc.sync.dma_start(
                            xts[j][:, q * xq : (q + 1) * xq],
                            xt[j][:, q * xq : (q + 1) * xq],
                        )
                hstage = xwp.tile([128, NG, OUT], BF16, name="hstage")
                for m in range(NG):
                    ps = pbpool.tile([128, OUT], F32, tag="hps")
                    for j in range(XK):
                        nc.tensor.matmul(
                            ps[:, :],
                            xts[j][:, m * GRP : (m + 1) * GRP],
                            wts[j][:, :],
                            start=(j == 0),
                            stop=(j == XK - 1),
                        )
                    if m % 2 == 0:
                        nc.scalar.activation(
                            hstage[:, m, :],
                            ps[:, :],
                            mybir.ActivationFunctionType.Copy,
                            scale=nsrc_t[:, m : m + 1],
                        )
                    else:
                        nc.vector.tensor_scalar(
                            hstage[:, m, :],
                            ps[:, :],
                            nsrc_t[:, m : m + 1],
                            None,
                            op0=mybir.AluOpType.mult,
                        )
                # partitions c*32+q hold node loc = g*128 + 4q + c (x columns
                # host-permuted): h_my_a row g*96 + 3q + c, h_my_b row g*32+q
                hq = NG // 4
                for q8 in range(4):
                    a = q8 * hq
                    z = (q8 + 1) * hq if q8 < 3 else NG
                    va = h_my_a[a * 96 : z * 96, :].rearrange(
                        "(g q c) f -> q g c f", q=32, c=3
                    )
                    for c_ in range(3):
                        nc.sync.dma_start(
                            va[:, :, c_, :],
                            hstage[c_ * 32 : (c_ + 1) * 32, a:z, :],
                        )
                    nc.sync.dma_start(
                        h_my_b[a * 32 : z * 32, :].rearrange(
                            "(g p) f -> p g f", p=32
                        ),
                        hstage[96:128, a:z, :],
                    )

            # ---- stage C: two AllGathers (3/4 part, then 1/4 part) ----
            for h_my_h, h_all_h, hr in (
                (h_my_a, h_all_a, HA),
                (h_my_b, h_all_b, HB),
            ):
                if cfg.get("NOCC"):
                    rep = (
                        h_my_h[:]
                        .rearrange("(o r) f -> o r f", o=1)
                        .to_broadcast((C, hr, OUT))
                    )
                    nc.sync.dma_start(
                        h_all_h[:].rearrange("(o r) f -> o r f", o=C), rep
                    )
                else:
                    nc.gpsimd.collective_compute(
                        "AllGather",
                        mybir.AluOpType.bypass,
                        replica_groups=[list(range(C))],
                        ins=[h_my_h[:]],
                        outs=[h_all_h[:]],
                    )

            # index tables are first needed by stage D's gathers - load
            # them after the x/B/AG chain is underway
            nc.sync.dma_start(gidx_t[:], gidx_d[:])
            nc.sync.dma_start(doff_t[:], doff_d[:])

            # interleaved bank views: banks 0..2 -> row 3j + b of part A,
            # bank 3 -> row j of part B; banks 4..7 -> same views over the
            # core's OWN h tables (usable before any collective completes)
            h_banks = [
                h_all_a[:].rearrange("(j k) f -> j (k f)", k=3)[
                    :, b * OUT : (b + 1) * OUT
                ]
                for b in range(3)
            ] + [h_all_b[:]] + [
                h_my_a[:].rearrange("(j k) f -> j (k f)", k=3)[
                    :, b * OUT : (b + 1) * OUT
                ]
                for b in range(3)
            ] + [h_my_b[:]]
            h_esteps = [3 * OUT, 3 * OUT, 3 * OUT, OUT] * 2

            # ---- stage D: gather + one-hot matmul segmented sum ----
            # ---- stage E (inline): relu(psum*ndst) + BN partial sums ----
            # Gathers are batched per (chunk, bank); groups are processed
            # sequentially (their 4 bank runs back-to-back) so each PSUM bank
            # holds at most one pending accumulation group.
            ps_stat = pspool.tile([1, 2 * OUT], F32, name="ps_stat")
            ps_sum = ps_stat[:, 0:OUT]
            ps_sq = ps_stat[:, OUT : 2 * OUT]
            ndone = [0]  # groups completed (for BN-sum start/stop flags)

            def finish_group(g, ps_g):
                """relu + BN-sum accumulation for a completed group psum."""
                if b_nonzero:
                    tmp = epool.tile([128, OUT], F32, tag="etmp")
                    nc.vector.scalar_tensor_tensor(
                        tmp[:],
                        ps_g[:],
                        ndst_t[:, g : g + 1],
                        btile_t[:],
                        op0=mybir.AluOpType.mult,
                        op1=mybir.AluOpType.add,
                    )
                    nc.scalar.activation(
                        agg_t[:, g, :], tmp[:], mybir.ActivationFunctionType.Relu
                    )
                else:
                    nc.scalar.activation(
                        agg_t[:, g, :],
                        ps_g[:],
                        mybir.ActivationFunctionType.Relu,
                        scale=ndst_t[:, g : g + 1],
                    )
                ones = onesc_t if g < NG - 1 else onest_t
                i0 = ndone[0]
                # ps_sum/ps_sq share one bank = ONE accumulation group:
                # start only on the very first matmul, stop on the very last.
                nc.tensor.matmul(
                    ps_sum,
                    ones[:],
                    agg_t[:, g, :],
                    start=(i0 == 0),
                    stop=False,
                )
                sq = epool.tile([128, OUT], BF16, tag="esq")
                nc.scalar.activation(
                    sq[:], agg_t[:, g, :], mybir.ActivationFunctionType.Square
                )
                nc.tensor.matmul(
                    ps_sq,
                    ones[:],
                    sq[:],
                    start=False,
                    stop=(i0 == NG - 1),
                )
                ndone[0] += 1

            if b_nonzero:
                # replicate b across partitions once (PE broadcast)
                ps_b = pbpool.tile([128, OUT], F32, tag="hps", name="ps_b")
                btile_t = cpool.tile([128, OUT], F32)
                nc.tensor.matmul(ps_b[:], onesr_t[:], bt_t[:], start=True, stop=True)
                nc.scalar.activation(
                    btile_t[:], ps_b[:], mybir.ActivationFunctionType.Copy
                )

            run_segs = meta["run_segs"]
            chunks = meta["chunks"]
            nbmax = max(nb for _, _, nb in units)
            dstack = tc.tile_pool(name="gath", bufs=8)
            gpool = dstack.__enter__()
            phases = [(4, 5, 6, 7), (0, 1, 2), (3,)]
            ui = 0
            for pi, ph in enumerate(phases):
                for ci, ch in enumerate(chunks):
                    gts = {}
                    for b in ph:
                        bank, t0, nblk = units[ui]
                        ui += 1
                        assert bank == b
                        Gt = gpool.tile(
                            [128, nbmax, OUT], BF16, tag="G", name=f"G{pi}_{ci}_{b}"
                        )
                        nc.gpsimd.dma_gather(
                            Gt[:, :nblk, :],
                            h_banks[b],
                            gidx_t[:, t0 * 8 : (t0 + nblk) * 8],
                            nblk * 128,
                            nblk * 128,
                            OUT,
                            elem_step=h_esteps[b],
                            single_packet=False,
                        )
                        gts[b] = (Gt, t0)
                    for g in ch:
                        ps_g = pgpool.tile(
                            [128, OUT], F32, tag="aggps", name=f"ps{pi}_{g}"
                        )
                        if pi > 0:
                            # re-inject previous phase's partial (bf16)
                            nc.tensor.matmul(
                                ps_g[:],
                                ident_t[:],
                                agg_t[:, g, :],
                                start=True,
                                stop=False,
                            )
                        for bi, b in enumerate(ph):
                            Gt, t0 = gts[b]
                            segs = run_segs[(b, g)]
                            for si, (t, col, lo, hi) in enumerate(segs):
                                Mt = mpool.tile([128, GRP], BF16, tag="M")
                                nc.vector.tensor_scalar(
                                    Mt[:],
                                    iota_t[:],
                                    doff_t[:, col : col + 1],
                                    None,
                                    op0=mybir.AluOpType.is_equal,
                                )
                                nc.tensor.matmul(
                                    ps_g[:],
                                    Mt[:],
                                    Gt[:, t - t0, :],
                                    start=(pi == 0 and bi == 0 and si == 0),
                                    stop=(
                                        bi == len(ph) - 1
                                        and si == len(segs) - 1
                                    ),
                                )
                        if pi < len(phases) - 1:
                            # spill partial sum to agg_t (bf16), no relu yet
                            nc.scalar.activation(
                                agg_t[:, g, :],
                                ps_g[:],
                                mybir.ActivationFunctionType.Copy,
                            )
                        else:
                            finish_group(g, ps_g)
            dstack.__exit__(None, None, None)
            assert ndone[0] == NG

            # ---- stage F: AllReduce BN stats; build affine S/T tiles ----
            st_sb = cpool.tile([1, 2 * OUT], F32)
            nc.scalar.activation(
                st_sb[:, 0:OUT], ps_sum, mybir.ActivationFunctionType.Copy
            )
            nc.scalar.activation(
                st_sb[:, OUT : 2 * OUT], ps_sq, mybir.ActivationFunctionType.Copy
            )
            nc.sync.dma_start(stats_in[:], st_sb[:])
            if cfg.get("NOCC"):
                rep = (
                    stats_in[:]
                    .rearrange("(o r) f -> o r f", o=1)
                    .to_broadcast((C, 1, 2 * OUT))
                )
                nc.sync.dma_start(
                    stats_out[:].rearrange("(o r) f -> o r f", o=C), rep
                )
            else:
                nc.gpsimd.collective_compute(
                    "AllGather",
                    mybir.AluOpType.bypass,
                    replica_groups=[list(range(C))],
                    ins=[stats_in[:]],
                    outs=[stats_out[:]],
                )
            # per-core partials land row-major; sum the C rows locally
            st_all = cpool.tile([1, C * 2 * OUT], F32)
            nc.sync.dma_start(
                st_all[:], stats_out[:].rearrange("(o c) f -> o (c f)", o=1)
            )
            st_rb = cpool.tile([1, 2 * OUT], F32)
            nc.vector.tensor_add(
                st_rb[:], st_all[:, 0 : 2 * OUT], st_all[:, 2 * OUT : 4 * OUT]
            )
            for c_ in range(2, C):
                nc.vector.tensor_add(
                    st_rb[:],
                    st_rb[:],
                    st_all[:, c_ * 2 * OUT : (c_ + 1) * 2 * OUT],
                )

            mu = cpool.tile([1, OUT], F32)
            ex2 = cpool.tile([1, OUT], F32)
            var = cpool.tile([1, OUT], F32)
            srow = cpool.tile([1, OUT], F32)
            trow = cpool.tile([1, OUT], F32)
            inv_n = 1.0 / float(N)
            nc.scalar.activation(
                mu[:], st_rb[:, 0:OUT], mybir.ActivationFunctionType.Copy, scale=inv_n
            )
            nc.scalar.activation(
                ex2[:],
                st_rb[:, OUT : 2 * OUT],
                mybir.ActivationFunctionType.Copy,
                scale=inv_n,
            )
            nc.scalar.activation(var[:], mu[:], mybir.ActivationFunctionType.Square)
            nc.vector.tensor_sub(var[:], ex2[:], var[:])
            # var <- rsqrt(var + eps) (ACT Rsqrt is banned for accuracy)
            nc.scalar.activation(
                var[:],
                var[:],
                mybir.ActivationFunctionType.Copy,
                bias=float(cfg["EPS"]),
            )
            nc.vector.reciprocal(var[:], var[:])
            nc.scalar.activation(var[:], var[:], mybir.ActivationFunctionType.Sqrt)
            nc.vector.tensor_mul(srow[:], gm_t[:], var[:])
            nc.vector.tensor_mul(trow[:], mu[:], srow[:])
            nc.vector.tensor_sub(trow[:], bb_t[:], trow[:])

            S_t = cpool.tile([128, OUT], BF16)
            T_t = cpool.tile([128, OUT], BF16)
            ps_S = pgpool.tile([128, OUT], F32, tag="aggps", name="ps_S")
            ps_T = pgpool.tile([128, OUT], F32, tag="aggps", name="ps_T")
            nc.tensor.matmul(ps_S[:], onesr_t[:], srow[:], start=True, stop=True)
            nc.tensor.matmul(ps_T[:], onesr_t[:], trow[:], start=True, stop=True)
            nc.scalar.activation(S_t[:], ps_S[:], mybir.ActivationFunctionType.Copy)
            nc.scalar.activation(T_t[:], ps_T[:], mybir.ActivationFunctionType.Copy)

            # ---- stage G: y = hrelu * S + T (bf16), cast f32 on DMA out ----
            with tc.tile_pool(name="gtmp", bufs=2) as gpool2:
                GB = 25  # groups per batched op
                S_bc = (
                    S_t[:]
                    .rearrange("p (o f) -> p o f", o=1)
                    .to_broadcast((128, GB, OUT))
                )
                T_bc = (
                    T_t[:]
                    .rearrange("p (o f) -> p o f", o=1)
                    .to_broadcast((128, GB, OUT))
                )
                ypad_view = ypad_d[:].rearrange("(g p) f -> p g f", p=128)
                for g0 in range(0, NG, GB):
                    gw = min(GB, NG - g0)
                    tmp = gpool2.tile([128, GB, OUT], BF16, tag="gtmp")
                    nc.vector.tensor_mul(
                        tmp[:, :gw, :],
                        agg_t[:, g0 : g0 + gw, :],
                        S_bc if gw == GB else S_t[:]
                        .rearrange("p (o f) -> p o f", o=1)
                        .to_broadcast((128, gw, OUT)),
                    )
                    nc.vector.tensor_add(
                        agg_t[:, g0 : g0 + gw, :],
                        tmp[:, :gw, :],
                        T_bc if gw == GB else T_t[:]
                        .rearrange("p (o f) -> p o f", o=1)
                        .to_broadcast((128, gw, OUT)),
                    )
                    nc.gpsimd.dma_start(
                        ypad_view[:, g0 : g0 + gw, :],
                        agg_t[:, g0 : g0 + gw, :],
                    )

    nc.compile()
    return nc


def kernel(x, src, dst, W, b, gamma, beta):
    global LAST_RESULTS
    cfg = CFG
    N, E, IN, OUT, C = cfg["N"], cfg["E"], cfg["IN"], cfg["OUT"], cfg["NCORES"]
    GRP = cfg["GRP"]
    assert x.shape == (N, IN) and W.shape == (IN, OUT)
    assert src.shape == (E,) and dst.shape == (E,)

    b = np.asarray(b, np.float32)
    b_nonzero = bool(np.any(b != 0.0))
    meta, gidx_cores, dstoff_cores = _preprocess(cfg, src, dst)
    NPC, NPCP, NG = meta["NPC"], meta["NPCP"], meta["NG"]
    XK = _ceil_div(IN, 128)
    last_w = NPC - (NG - 1) * GRP
    # node permutation: within each 128-node group, order by loc%4 class
    perm = np.concatenate([np.arange(c, 128, 4) for c in range(4)])
    g_ = np.arange(NPCP) // 128
    p_ = np.arange(NPCP) % 128
    permn = g_ * 128 + perm[p_]          # source node (local) per padded col
    valid = permn < NPC

    nc = _build_nc(cfg, meta, b_nonzero=b_nonzero)

    xT = np.ascontiguousarray(np.asarray(x, np.float32).T)  # [IN, N]
    Wn = np.asarray(W, np.float32)
    import ml_dtypes

    iota = np.tile(np.arange(GRP, dtype=np.float32)[None, :], (128, 1)).astype(
        ml_dtypes.bfloat16
    )
    onesc = np.ones((128, 1), np.float32)
    onest = np.zeros((128, 1), np.float32)
    onest[:last_w] = 1.0
    onesr = np.ones((1, 128), np.float32)
    gm = np.asarray(gamma, np.float32)[None, :]
    bb = np.asarray(beta, np.float32)[None, :]

    in_maps = []
    for k in range(C):
        im = {
            "gidx": gidx_cores[k],
            "doff": dstoff_cores[k],
            "dego": _tile_major(
                np.where(
                    valid,
                    meta["deg_out"][k * NPC + np.minimum(permn, NPC - 1)],
                    np.float32(1.0),
                ).astype(np.float32),
                NG,
                GRP,
                np.float32(1.0),
            ),
            "degi": _tile_major(
                meta["deg_in"][k * NPC : (k + 1) * NPC], NG, GRP, np.float32(1.0)
            ),
            "iota": iota,
            "gm": gm,
            "bb": bb,
            "onesc": onesc.astype(ml_dtypes.bfloat16),
            "onest": onest.astype(ml_dtypes.bfloat16),
            "onesr": onesr,
            "ident": np.eye(128, dtype=np.float32).astype(ml_dtypes.bfloat16),
        }
        if b_nonzero:
            im["bt"] = b[None, :]
        for j in range(XK):
            xcols = np.zeros((128, NPCP), np.float32)
            xcols[:, valid] = xT[
                j * 128 : (j + 1) * 128, k * NPC + permn[valid]
            ]
            im[f"xt{j}"] = xcols.astype(ml_dtypes.bfloat16)
            im[f"wt{j}"] = np.ascontiguousarray(
                Wn[j * 128 : (j + 1) * 128, :]
            ).astype(ml_dtypes.bfloat16)
        in_maps.append(im)

    if cfg.get("SIM"):
        from concourse.bass_interp import MultiCoreSim

        sim = MultiCoreSim(nc, num_cores=C)
        for k, core_sim in sim.cores.items():
            for name, val in in_maps[k].items():
                core_sim.tensor(name)[:] = val
        sim.simulate()
        y = np.empty((N, OUT), np.float32)
        for k in range(C):
            y[k * NPC : (k + 1) * NPC] = sim.cores[k].tensor("ypad")[:NPC]
        return y

    global LAST_NC, LAST_RUN_S
    LAST_NC = nc
    import time as _time

    _t0 = _time.time()
    res = bass_utils.run_bass_kernel_spmd(
        nc,
        in_maps,
        core_ids=list(range(C)),
        trace=cfg.get("TRACE", False),
    )
    LAST_RUN_S = _time.time() - _t0
    LAST_RESULTS = res

    y = np.empty((N, OUT), np.float32)
    for k in range(C):
        y[k * NPC : (k + 1) * NPC] = res.results[k]["ypad"][:NPC]
    return y


# revision 52
# speedup vs baseline: 1.2528x; 1.0259x over previous
"""GCN block (GraphConv + BatchNorm1d + ReLU) on 8 Trainium2 NeuronCores.

Strategy (per sharding hint): partition nodes (and incident edges) across the
8 cores; replicate W/b/gamma/beta; all-reduce BN batch statistics.

Per core k (owns dst nodes [k*NPC, (k+1)*NPC)):
  1. h_k = (x_k @ W) * rsqrt(clip(deg_out_k,1))  (PE matmul; x columns are
     host-permuted so nodes land on partitions grouped by loc%4 class).
  2. TWO AllGathers of h (bf16), split 3:1: nodes with loc%4 in {0,1,2},
     then loc%4 == 3. The int16-indexable "bank" tables are *interleaved
     strided views* of the AG outputs: bank b < 3 lives at row 3j + b of
     part A (elem_step=3), bank 3 at row j of part B, with
     j = owner*(NPCP/4) + loc//4. The linearity holds for any k-of-4
     interleave, so two big collectives serve four gather tables.
  3. Edges are processed in three phases: (0) edges whose source is owned
     by this core gather from the core's OWN h tables and run inside the
     first collective's window; (1) remote banks {0,1,2} after AG part A
     (75% of remote work, hiding AG part B); (2) remote bank 3. Gathers
     are batched per (phase, chunk-of-groups, bank); 32-granular shared
     run sizes (= max over cores). Segment sums use one-hot matmuls
     M^T @ G accumulated in PSUM; each group keeps ONE psum accumulation
     per phase, spilled to bf16 and re-injected via an identity matmul at
     the next phase. Runs straddling 128-slot block boundaries get one
     matmul per straddled block; out-of-segment slots carry doff 255 so
     their one-hot column is zero.
  4. relu(psum * rsqrt(clip(deg_in,1)) [+ b]) via ACT directly from PSUM
     (bf16 out); BN sums via ones-matmuls (single accumulation group per
     PSUM bank); stats combined via AllGather + local sum; y = h*S + T
     with S = gamma*rsqrt(var+eps), T = beta - mu*S (broadcast-AP DVE
     ops); y cast bf16->f32 during the output DMA (SWDGE), pipelined per
     25-group batch.

Host-side work is limited to integer index bookkeeping (bucketing edges by
(core, locality, src-bank, dst-group), degree counting) and layout
transforms (x^T permutation/padding, int16 gather indices). All
floating-point math runs on device.

Run sizes are padded to a structure shared by all 8 cores so a single SPMD
NEFF serves every core; pad slots re-gather the run's last row (HBM page
hit) and carry a dst offset of 255 -> contribute exactly 0. Edges are
sorted by gather row within each bucket for HBM locality.
"""
import math
import os
import sys

sys.path.insert(0, "/opt/trn_rl_repo")

import numpy as np

import concourse.bacc as bacc
import concourse.bass as bass
import concourse.mybir as mybir
import concourse.tile as tile
from concourse import bass_utils

F32 = mybir.dt.float32
BF16 = mybir.dt.bfloat16
I16 = mybir.dt.int16

CFG = dict(
    N=100000,
    E=1600000,
    IN=256,
    OUT=128,
    NCORES=8,
    GRP=128,          # dst nodes per segment group (= psum partition dim)
    NBANKS=4,         # interleaved src banks (bank rows must be < 32768)
    GCHUNK=7,        # groups per chunk (gather batch granularity)
    EPS=1e-5,
    TRACE=False,
)

LAST_RESULTS = None  # set by kernel() for test harness introspection
LAST_NC = None
LAST_RUN_S = None


def _ceil_div(a, b):
    return (a + b - 1) // b


def _wrap16(idx, ncols):
    """int16 idx list -> [128, ncols] tile: idx i at [i%16, i//16], replicated
    8x across the 16-partition groups (one copy per GpSimd Q7 core)."""
    n = idx.shape[0]
    assert n == ncols * 16
    w = np.ascontiguousarray(idx.reshape(ncols, 16).T)
    return np.tile(w, (8, 1))


def _preprocess(cfg, src, dst):
    """Bucket edges by (owner core, interleaved src bank, dst group); build
    per-core gather-index / dst-offset arrays and the shared run structure."""
    N, E = cfg["N"], cfg["E"]
    C, NBANKS, GRP, GC = cfg["NCORES"], cfg["NBANKS"], cfg["GRP"], cfg["GCHUNK"]
    NPC = N // C
    NG = _ceil_div(NPC, GRP)
    NPCP = NG * GRP                # padded nodes per core (x cols zero-padded)
    assert NPCP % NBANKS == 0
    QB = NPCP // NBANKS            # gather rows per owner per bank view
    BANKROWS = QB * C              # rows per bank view of one AG-half output
    assert BANKROWS < 32768

    src = src.astype(np.int64)
    dst = dst.astype(np.int64)
    deg_out = np.bincount(src, minlength=N).astype(np.float32)
    deg_in = np.bincount(dst, minlength=N).astype(np.float32)

    owner = dst // NPC
    loc = src % NPC
    src_owner = src // NPC
    is_local = src_owner == owner  # src row available before any collective
    # bank classes: 0-3 remote (gather from AG output), 4-7 local (from
    # the core's own h tables, no owner term in the row index)
    bank = loc % NBANKS + NBANKS * is_local
    grow = np.where(is_local, loc // NBANKS, src_owner * QB + loc // NBANKS)
    assert grow.max() < 32768
    grp = (dst % NPC) // GRP
    key = (owner * 2 * NBANKS + bank) * NG + grp
    # sort by bucket, then by gather row inside the bucket (HBM locality)
    order = np.lexsort((grow, key))
    s_grow = grow[order]
    s_dst = dst[order]
    s_key = key[order]

    counts = np.bincount(key, minlength=C * 2 * NBANKS * NG).reshape(
        C, 2 * NBANKS, NG
    )
    P = counts.max(axis=0)  # [NBANKS, NG] shared run sizes (32-granular)
    P = ((P + 31) // 32) * 32
    P = np.maximum(P, 32)   # every (b,g) run structurally exists

    # local banks first (overlap the big collective), then remote 3:1
    phases = [(4, 5, 6, 7), (0, 1, 2), (3,)]
    chunks = [list(range(c, min(c + GC, NG))) for c in range(0, NG, GC)]
    run_seq = [
        (b, g) for ph in phases for ch in chunks for b in ph for g in ch
    ]
    # lay out runs; pad each (phase, chunk, bank) unit to a 128 multiple
    run_off = np.zeros((2 * NBANKS, NG), np.int64)
    units = []  # (bank, first_block, n_blocks) in stream order
    pos = 0
    for ph in phases:
        for ch in chunks:
            for b in ph:
                u0 = pos
                for g in ch:
                    run_off[b, g] = pos
                    pos += P[b, g]
                pos = ((pos + 127) // 128) * 128  # unit pad
                units.append((b, u0 // 128, (pos - u0) // 128))
    nidx_tot = int(pos)
    nb_tot = nidx_tot // 128

    # segments: a run may straddle block boundaries; each (run, block)
    # intersection is one segment = one doff column + one full matmul
    # (out-of-segment slots carry doff 255 -> zero one-hot column).
    run_segs = {}  # (b, g) -> list of (block_t, doff_col, slot_lo, slot_hi)
    nseg = 0
    for b, g in run_seq:
        off = int(run_off[b, g])
        end = off + int(P[b, g])
        segs = []
        t = off // 128
        while t * 128 < end:
            lo = max(off, t * 128)
            hi = min(end, (t + 1) * 128)
            segs.append((t, nseg, lo, hi))
            nseg += 1
            t += 1
        run_segs[(b, g)] = segs

    # boundaries of each (k, b, g) bucket in the sorted edge stream
    bkeys = (
        np.arange(C)[:, None, None] * 2 * NBANKS
        + np.arange(2 * NBANKS)[None, :, None]
    ) * NG + np.arange(NG)[None, None, :]
    starts = np.searchsorted(s_key, bkeys.ravel()).reshape(C, 2 * NBANKS, NG)
    ends = np.searchsorted(s_key, bkeys.ravel(), side="right").reshape(
        C, 2 * NBANKS, NG
    )

    gidx_cores = []
    dstoff_cores = []
    for k in range(C):
        gidx = np.zeros(nidx_tot, np.int16)
        doff_cols = np.full((nseg, 128), 255.0, np.float32)
        for b in range(2 * NBANKS):
            for g in range(NG):
                s, e = starts[k, b, g], ends[k, b, g]
                cnt = e - s
                p0 = int(run_off[b, g])
                if cnt:
                    gidx[p0 : p0 + cnt] = s_grow[s:e].astype(np.int16)
                    # pad slots re-gather the last row (HBM page hit)
                    gidx[p0 + cnt : p0 + int(P[b, g])] = gidx[p0 + cnt - 1]
                    offs = ((s_dst[s:e] % NPC) - g * GRP).astype(np.float32)
                    for t, col, lo, hi in run_segs[(b, g)]:
                        a = max(lo, p0)
                        z = min(hi, p0 + cnt)
                        if z > a:
                            doff_cols[col, a - t * 128 : z - t * 128] = offs[
                                a - p0 : z - p0
                            ]
        # unit-pad slots gather row 0 (gidx stays 0) and have no segment
        gidx_cores.append(_wrap16(gidx, nidx_tot // 16))
        dstoff_cores.append(np.ascontiguousarray(doff_cols.T))

    meta = dict(
        NPC=NPC,
        NPCP=NPCP,
        NG=NG,
        QB=QB,
        BANKROWS=BANKROWS,
        nidx_tot=nidx_tot,
        nb_tot=nb_tot,
        nseg=nseg,
        run_segs=run_segs,
        units=units,
        chunks=chunks,
        run_seq=run_seq,
        deg_out=deg_out,
        deg_in=deg_in,
    )
    return meta, gidx_cores, dstoff_cores


def _tile_major(vec, NG, GRP, pad_val):
    """[NPC] -> [GRP, NG]: entry (p, m) = vec[m*GRP + p], padded."""
    out = np.full((NG * GRP,), pad_val, vec.dtype)
    out[: vec.shape[0]] = vec
    return np.ascontiguousarray(out.reshape(NG, GRP).T)


def _build_nc(cfg, meta, b_nonzero=False):
    N, IN, OUT, C = cfg["N"], cfg["IN"], cfg["OUT"], cfg["NCORES"]
    GRP, NBANKS = cfg["GRP"], cfg["NBANKS"]
    NPC, NPCP, NG = meta["NPC"], meta["NPCP"], meta["NG"]
    nidx_tot, nb_tot = meta["nidx_tot"], meta["nb_tot"]
    units = meta["units"]
    XK = _ceil_div(IN, 128)
    assert OUT == 128 and GRP == 128
    last_w = NPC - (NG - 1) * GRP  # valid rows in the last group
    HALF = NPCP // 2               # rows per AG-half input

    nc = bacc.Bacc(
        "TRN2", target_bir_lowering=False, debug=False, num_devices=C
    )

    # ---- external inputs ----
    NXQ = 8  # x DMA split for earlier stage-B start
    xq = NPCP // NXQ
    assert NPCP % NXQ == 0
    xt = [
        nc.dram_tensor(f"xt{j}", [128, NPCP], BF16, kind="ExternalInput")
        for j in range(XK)
    ]
    wt = [
        nc.dram_tensor(f"wt{j}", [128, OUT], BF16, kind="ExternalInput")
        for j in range(XK)
    ]
    gidx_d = nc.dram_tensor("gidx", [128, nidx_tot // 16], I16, kind="ExternalInput")
    doff_d = nc.dram_tensor("doff", [128, meta["nseg"]], F32, kind="ExternalInput")
    dego_d = nc.dram_tensor("dego", [128, NG], F32, kind="ExternalInput")
    degi_d = nc.dram_tensor("degi", [128, NG], F32, kind="ExternalInput")
    iota_d = nc.dram_tensor("iota", [128, GRP], BF16, kind="ExternalInput")
    gm_d = nc.dram_tensor("gm", [1, OUT], F32, kind="ExternalInput")
    bb_d = nc.dram_tensor("bb", [1, OUT], F32, kind="ExternalInput")
    onesc_d = nc.dram_tensor("onesc", [128, 1], BF16, kind="ExternalInput")
    onest_d = nc.dram_tensor("onest", [128, 1], BF16, kind="ExternalInput")
    onesr_d = nc.dram_tensor("onesr", [1, 128], F32, kind="ExternalInput")
    ident_d = nc.dram_tensor("ident", [128, 128], BF16, kind="ExternalInput")
    if b_nonzero:
        bt_d = nc.dram_tensor("bt", [1, OUT], F32, kind="ExternalInput")

    ypad_d = nc.dram_tensor("ypad", [NG * GRP, OUT], F32, kind="ExternalOutput")

    with tile.TileContext(nc) as tc:
        with (
            tc.tile_pool(name="const", bufs=1) as cpool,
            tc.tile_pool(name="dram", bufs=1, space="DRAM") as dpool,
            tc.tile_pool(name="agg", bufs=1) as apool,
            tc.tile_pool(name="mpool", bufs=16) as mpool,
            tc.tile_pool(name="etmp", bufs=4) as epool,
            tc.tile_pool(name="psg", bufs=4, space="PSUM") as pgpool,
            tc.tile_pool(name="psb", bufs=3, space="PSUM") as pbpool,
            tc.tile_pool(name="pstat", bufs=1, space="PSUM") as pspool,
        ):
            # ---- constants / small tiles ----
            iota_t = cpool.tile([128, GRP], BF16)
            dego_t = cpool.tile([128, NG], F32)
            degi_t = cpool.tile([128, NG], F32)
            nsrc_t = cpool.tile([128, NG], F32)
            ndst_t = cpool.tile([128, NG], F32)
            gm_t = cpool.tile([1, OUT], F32)
            bb_t = cpool.tile([1, OUT], F32)
            onesc_t = cpool.tile([128, 1], BF16)
            onest_t = cpool.tile([128, 1], BF16)
            onesr_t = cpool.tile([1, 128], F32)
            gidx_t = cpool.tile([128, nidx_tot // 16], I16)
            doff_t = cpool.tile([128, meta["nseg"]], F32)
            ident_t = cpool.tile([128, 128], BF16)
            nc.sync.dma_start(ident_t[:], ident_d[:])

            nc.sync.dma_start(iota_t[:], iota_d[:])
            nc.sync.dma_start(dego_t[:], dego_d[:])
            nc.sync.dma_start(degi_t[:], degi_d[:])
            nc.sync.dma_start(gm_t[:], gm_d[:])
            nc.sync.dma_start(bb_t[:], bb_d[:])
            nc.sync.dma_start(onesc_t[:], onesc_d[:])
            nc.sync.dma_start(onest_t[:], onest_d[:])
            nc.sync.dma_start(onesr_t[:], onesr_d[:])
            if b_nonzero:
                bt_t = cpool.tile([1, OUT], F32)
                nc.sync.dma_start(bt_t[:], bt_d[:])

            # norms: rsqrt(max(deg, 1))
            for deg_t, norm_t in ((dego_t, nsrc_t), (degi_t, ndst_t)):
                nc.vector.tensor_scalar(
                    norm_t[:], deg_t[:], 1.0, None, op0=mybir.AluOpType.max
                )
                nc.vector.reciprocal(norm_t[:], norm_t[:])
                nc.scalar.activation(
                    norm_t[:], norm_t[:], mybir.ActivationFunctionType.Sqrt
                )

            # internal DRAM for collectives (3:1 interleaved node split)
            _aspace = "Local" if cfg.get("NOCC") else "Shared"
            HA = 3 * NPCP // 4     # nodes with loc%4 in {0,1,2}
            HB = NPCP // 4         # nodes with loc%4 == 3
            h_my_a = dpool.tile([HA, OUT], BF16, name="h_my_a")
            h_my_b = dpool.tile([HB, OUT], BF16, name="h_my_b")
            h_all_a = dpool.tile(
                [C * HA, OUT], BF16, addr_space=_aspace, name="h_all_a"
            )
            h_all_b = dpool.tile(
                [C * HB, OUT], BF16, addr_space=_aspace, name="h_all_b"
            )
            stats_in = dpool.tile([1, 2 * OUT], F32)
            stats_out = dpool.tile([C, 2 * OUT], F32, addr_space=_aspace)

            # relu(norm*agg) output, bf16, [128, NG, OUT]
            agg_t = apool.tile([128, NG, OUT], BF16)

            # ---- stage B: h = (x @ W) * norm_src, cast bf16, store to HBM
            # (staged in SBUF; 2 large DMAs instead of 98 small ones)
            with tc.tile_pool(name="xw", bufs=1) as xwp:
                xts = []
                wts = []
                for j in range(XK):
                    xts.append(xwp.tile([128, NPCP], BF16, name=f"xt_s{j}"))
                    wts.append(xwp.tile([128, OUT], BF16, name=f"wt_s{j}"))
                for j in range(XK):
                    nc.sync.dma_start(wts[j][:], wt[j][:])
                for q in range(NXQ):
                    for j in range(XK):
                        nc.sync.dma_start(
                            xts[j][:, q * xq : (q + 1) * xq],
                            xt[j][:, q * xq : (q + 1) * xq],
                        )
                hstage = xwp.tile([128, NG, OUT], BF16, name="hstage")
                for m in range(NG):
                    ps = pbpool.tile([128, OUT], F32, tag="hps")
                    for j in range(XK):
                        nc.tensor.matmul(
                            ps[:, :],
                            xts[j][:, m * GRP : (m + 1) * GRP],
                            wts[j][:, :],
                            start=(j == 0),
                            stop=(j == XK - 1),
                        )
                    if m % 2 == 0:
                        nc.scalar.activation(
                            hstage[:, m, :],
                            ps[:, :],
                            mybir.ActivationFunctionType.Copy,
                            scale=nsrc_t[:, m : m + 1],
                        )
                    else:
                        nc.vector.tensor_scalar(
                            hstage[:, m, :],
                            ps[:, :],
                            nsrc_t[:, m : m + 1],
                            None,
                            op0=mybir.AluOpType.mult,
                        )
                # partitions c*32+q hold node loc = g*128 + 4q + c (x columns
                # host-permuted): h_my_a row g*96 + 3q + c, h_my_b row g*32+q
                hq = NG // 4
                for q8 in range(4):
                    a = q8 * hq
                    z = (q8 + 1) * hq if q8 < 3 else NG
                    va = h_my_a[a * 96 : z * 96, :].rearrange(
                        "(g q c) f -> q g c f", q=32, c=3
                    )
                    for c_ in range(3):
                        nc.sync.dma_start(
                            va[:, :, c_, :],
                            hstage[c_ * 32 : (c_ + 1) * 32, a:z, :],
                        )
                    nc.sync.dma_start(
                        h_my_b[a * 32 : z * 32, :].rearrange(
                            "(g p) f -> p g f", p=32
                        ),
                        hstage[96:128, a:z, :],
                    )

            # ---- stage C: two AllGathers (3/4 part, then 1/4 part) ----
            for h_my_h, h_all_h, hr in (
                (h_my_a, h_all_a, HA),
                (h_my_b, h_all_b, HB),
            ):
                if cfg.get("NOCC"):
                    rep = (
                        h_my_h[:]
                        .rearrange("(o r) f -> o r f", o=1)
                        .to_broadcast((C, hr, OUT))
                    )
                    nc.sync.dma_start(
                        h_all_h[:].rearrange("(o r) f -> o r f", o=C), rep
                    )
                else:
                    nc.gpsimd.collective_compute(
                        "AllGather",
                        mybir.AluOpType.bypass,
                        replica_groups=[list(range(C))],
                        ins=[h_my_h[:]],
                        outs=[h_all_h[:]],
                    )

            # index tables are first needed by stage D's gathers - load
            # them after the x/B/AG chain is underway
            nc.sync.dma_start(gidx_t[:], gidx_d[:])
            nc.sync.dma_start(doff_t[:], doff_d[:])

            # interleaved bank views: banks 0..2 -> row 3j + b of part A,
            # bank 3 -> row j of part B; banks 4..7 -> same views over the
            # core's OWN h tables (usable before any collective completes)
            h_banks = [
                h_all_a[:].rearrange("(j k) f -> j (k f)", k=3)[
                    :, b * OUT : (b + 1) * OUT
                ]
                for b in range(3)
            ] + [h_all_b[:]] + [
                h_my_a[:].rearrange("(j k) f -> j (k f)", k=3)[
                    :, b * OUT : (b + 1) * OUT
                ]
                for b in range(3)
            ] + [h_my_b[:]]
            h_esteps = [3 * OUT, 3 * OUT, 3 * OUT, OUT] * 2

            # ---- stage D: gather + one-hot matmul segmented sum ----
            # ---- stage E (inline): relu(psum*ndst) + BN partial sums ----
            # Gathers are batched per (chunk, bank); groups are processed
            # sequentially (their 4 bank runs back-to-back) so each PSUM bank
            # holds at most one pending accumulation group.
            ps_stat = pspool.tile([1, 2 * OUT], F32, name="ps_stat")
            ps_sum = ps_stat[:, 0:OUT]
            ps_sq = ps_stat[:, OUT : 2 * OUT]
            ndone = [0]  # groups completed (for BN-sum start/stop flags)

            def finish_group(g, ps_g):
                """relu + BN-sum accumulation for a completed group psum."""
                if b_nonzero:
                    tmp = epool.tile([128, OUT], F32, tag="etmp")
                    nc.vector.scalar_tensor_tensor(
                        tmp[:],
                        ps_g[:],
                        ndst_t[:, g : g + 1],
                        btile_t[:],
                        op0=mybir.AluOpType.mult,
                        op1=mybir.AluOpType.add,
                    )
                    nc.scalar.activation(
                        agg_t[:, g, :], tmp[:], mybir.ActivationFunctionType.Relu
                    )
                else:
                    nc.scalar.activation(
                        agg_t[:, g, :],
                        ps_g[:],
                        mybir.ActivationFunctionType.Relu,
                        scale=ndst_t[:, g : g + 1],
                    )
                ones = onesc_t if g < NG - 1 else onest_t
                i0 = ndone[0]
                # ps_sum/ps_sq share one bank = ONE accumulation group:
                # start only on the very first matmul, stop on the very last.
                nc.tensor.matmul(
                    ps_sum,
                    ones[:],
                    agg_t[:, g, :],
                    start=(i0 == 0),
                    stop=False,
                )
                sq = epool.tile([128, OUT], BF16, tag="esq")
                nc.scalar.activation(
                    sq[:], agg_t[:, g, :], mybir.ActivationFunctionType.Square
                )
                nc.tensor.matmul(
                    ps_sq,
                    ones[:],
                    sq[:],
                    start=False,
                    stop=(i0 == NG - 1),
                )
                ndone[0] += 1

            if b_nonzero:
                # replicate b across partitions once (PE broadcast)
                ps_b = pbpool.tile([128, OUT], F32, tag="hps", name="ps_b")
                btile_t = cpool.tile([128, OUT], F32)
                nc.tensor.matmul(ps_b[:], onesr_t[:], bt_t[:], start=True, stop=True)
                nc.scalar.activation(
                    btile_t[:], ps_b[:], mybir.ActivationFunctionType.Copy
                )

            run_segs = meta["run_segs"]
            chunks = meta["chunks"]
            nbmax = max(nb for _, _, nb in units)
            dstack = tc.tile_pool(name="gath", bufs=8)
            gpool = dstack.__enter__()
            phases = [(4, 5, 6, 7), (0, 1, 2), (3,)]
            ui = 0
            for pi, ph in enumerate(phases):
                for ci, ch in enumerate(chunks):
                    gts = {}
                    for b in ph:
                        bank, t0, nblk = units[ui]
                        ui += 1
                        assert bank == b
                        Gt = gpool.tile(
                            [128, nbmax, OUT], BF16, tag="G", name=f"G{pi}_{ci}_{b}"
                        )
                        nc.gpsimd.dma_gather(
                            Gt[:, :nblk, :],
                            h_banks[b],
                            gidx_t[:, t0 * 8 : (t0 + nblk) * 8],
                            nblk * 128,
                            nblk * 128,
                            OUT,
                            elem_step=h_esteps[b],
                            single_packet=False,
                        )
                        gts[b] = (Gt, t0)
                    for g in ch:
                        ps_g = pgpool.tile(
                            [128, OUT], F32, tag="aggps", name=f"ps{pi}_{g}"
                        )
                        if pi > 0:
                            # re-inject previous phase's partial (bf16)
                            nc.tensor.matmul(
                                ps_g[:],
                                ident_t[:],
                                agg_t[:, g, :],
                                start=True,
                                stop=False,
                            )
                        for bi, b in enumerate(ph):
                            Gt, t0 = gts[b]
                            segs = run_segs[(b, g)]
                            for si, (t, col, lo, hi) in enumerate(segs):
                                Mt = mpool.tile([128, GRP], BF16, tag="M")
                                nc.vector.tensor_scalar(
                                    Mt[:],
                                    iota_t[:],
                                    doff_t[:, col : col + 1],
                                    None,
                                    op0=mybir.AluOpType.is_equal,
                                )
                                nc.tensor.matmul(
                                    ps_g[:],
                                    Mt[:],
                                    Gt[:, t - t0, :],
                                    start=(pi == 0 and bi == 0 and si == 0),
                                    stop=(
                                        bi == len(ph) - 1
                                        and si == len(segs) - 1
                                    ),
                                )
                        if pi < len(phases) - 1:
                            # spill partial sum to agg_t (bf16), no relu yet
                            nc.scalar.activation(
                                agg_t[:, g, :],
                                ps_g[:],
                                mybir.ActivationFunctionType.Copy,
                            )
                        else:
                            finish_group(g, ps_g)
            dstack.__exit__(None, None, None)
            assert ndone[0] == NG

            # ---- stage F: AllReduce BN stats; build affine S/T tiles ----
            st_sb = cpool.tile([1, 2 * OUT], F32)
            nc.scalar.activation(
                st_sb[:, 0:OUT], ps_sum, mybir.ActivationFunctionType.Copy
            )
            nc.scalar.activation(
                st_sb[:, OUT : 2 * OUT], ps_sq, mybir.ActivationFunctionType.Copy
            )
            nc.sync.dma_start(stats_in[:], st_sb[:])
            if cfg.get("NOCC"):
                rep = (
                    stats_in[:]
                    .rearrange("(o r) f -> o r f", o=1)
                    .to_broadcast((C, 1, 2 * OUT))
                )
                nc.sync.dma_start(
                    stats_out[:].rearrange("(o r) f -> o r f", o=C), rep
                )
            else:
                nc.gpsimd.collective_compute(
                    "AllGather",
                    mybir.AluOpType.bypass,
                    replica_groups=[list(range(C))],
                    ins=[stats_in[:]],
                    outs=[stats_out[:]],
                )
            # per-core partials land row-major; sum the C rows locally
            st_all = cpool.tile([1, C * 2 * OUT], F32)
            nc.sync.dma_start(
                st_all[:], stats_out[:].rearrange("(o c) f -> o (c f)", o=1)
            )
            st_rb = cpool.tile([1, 2 * OUT], F32)
            nc.vector.tensor_add(
                st_rb[:], st_all[:, 0 : 2 * OUT], st_all[:, 2 * OUT : 4 * OUT]
            )
            for c_ in range(2, C):
                nc.vector.tensor_add(
                    st_rb[:],
                    st_rb[:],
                    st_all[:, c_ * 2 * OUT : (c_ + 1) * 2 * OUT],
                )

            mu = cpool.tile([1, OUT], F32)
            ex2 = cpool.tile([1, OUT], F32)
            var = cpool.tile([1, OUT], F32)
            srow = cpool.tile([1, OUT], F32)
            trow = cpool.tile([1, OUT], F32)
            inv_n = 1.0 / float(N)
            nc.scalar.activation(
                mu[:], st_rb[:, 0:OUT], mybir.ActivationFunctionType.Copy, scale=inv_n
            )
            nc.scalar.activation(
                ex2[:],
                st_rb[:, OUT : 2 * OUT],
                mybir.ActivationFunctionType.Copy,
                scale=inv_n,
            )
            nc.scalar.activation(var[:], mu[:], mybir.ActivationFunctionType.Square)
            nc.vector.tensor_sub(var[:], ex2[:], var[:])
            # var <- rsqrt(var + eps) (ACT Rsqrt is banned for accuracy)
            nc.scalar.activation(
                var[:],
                var[:],
                mybir.ActivationFunctionType.Copy,
                bias=float(cfg["EPS"]),
            )
            nc.vector.reciprocal(var[:], var[:])
            nc.scalar.activation(var[:], var[:], mybir.ActivationFunctionType.Sqrt)
            nc.vector.tensor_mul(srow[:], gm_t[:], var[:])
            nc.vector.tensor_mul(trow[:], mu[:], srow[:])
            nc.vector.tensor_sub(trow[:], bb_t[:], trow[:])

            S_t = cpool.tile([128, OUT], BF16)
            T_t = cpool.tile([128, OUT], BF16)
            ps_S = pgpool.tile([128, OUT], F32, tag="aggps", name="ps_S")
            ps_T = pgpool.tile([128, OUT], F32, tag="aggps", name="ps_T")
            nc.tensor.matmul(ps_S[:], onesr_t[:], srow[:], start=True, stop=True)
            nc.tensor.matmul(ps_T[:], onesr_t[:], trow[:], start=True, stop=True)
            nc.scalar.activation(S_t[:], ps_S[:], mybir.ActivationFunctionType.Copy)
            nc.scalar.activation(T_t[:], ps_T[:], mybir.ActivationFunctionType.Copy)

            # ---- stage G: y = hrelu * S + T (bf16), cast f32 on DMA out ----
            with tc.tile_pool(name="gtmp", bufs=2) as gpool2:
                GB = 8  # groups per batched op
                S_bc = (
                    S_t[:]
                    .rearrange("p (o f) -> p o f", o=1)
                    .to_broadcast((128, GB, OUT))
                )
                T_bc = (
                    T_t[:]
                    .rearrange("p (o f) -> p o f", o=1)
                    .to_broadcast((128, GB, OUT))
                )
                ypad_view = ypad_d[:].rearrange("(g p) f -> p g f", p=128)
                for g0 in range(0, NG, GB):
                    gw = min(GB, NG - g0)
                    tmp = gpool2.tile([128, GB, OUT], BF16, tag="gtmp")
                    nc.vector.tensor_mul(
                        tmp[:, :gw, :],
                        agg_t[:, g0 : g0 + gw, :],
                        S_bc if gw == GB else S_t[:]
                        .rearrange("p (o f) -> p o f", o=1)
                        .to_broadcast((128, gw, OUT)),
                    )
                    nc.vector.tensor_add(
                        agg_t[:, g0 : g0 + gw, :],
                        tmp[:, :gw, :],
                        T_bc if gw == GB else T_t[:]
                        .rearrange("p (o f) -> p o f", o=1)
                        .to_broadcast((128, gw, OUT)),
                    )
                    nc.gpsimd.dma_start(
                        ypad_view[:, g0 : g0 + gw, :],
                        agg_t[:, g0 : g0 + gw, :],
                    )

    nc.compile()
    return nc


def kernel(x, src, dst, W, b, gamma, beta):
    global LAST_RESULTS
    cfg = CFG
    N, E, IN, OUT, C = cfg["N"], cfg["E"], cfg["IN"], cfg["OUT"], cfg["NCORES"]
    GRP = cfg["GRP"]
    assert x.shape == (N, IN) and W.shape == (IN, OUT)
    assert src.shape == (E,) and dst.shape == (E,)

    b = np.asarray(b, np.float32)
    b_nonzero = bool(np.any(b != 0.0))
    meta, gidx_cores, dstoff_cores = _preprocess(cfg, src, dst)
    NPC, NPCP, NG = meta["NPC"], meta["NPCP"], meta["NG"]
    XK = _ceil_div(IN, 128)
    last_w = NPC - (NG - 1) * GRP
    # node permutation: within each 128-node group, order by loc%4 class
    perm = np.concatenate([np.arange(c, 128, 4) for c in range(4)])
    g_ = np.arange(NPCP) // 128
    p_ = np.arange(NPCP) % 128
    permn = g_ * 128 + perm[p_]          # source node (local) per padded col
    valid = permn < NPC

    nc = _build_nc(cfg, meta, b_nonzero=b_nonzero)

    xT = np.ascontiguousarray(np.asarray(x, np.float32).T)  # [IN, N]
    Wn = np.asarray(W, np.float32)
    import ml_dtypes

    iota = np.tile(np.arange(GRP, dtype=np.float32)[None, :], (128, 1)).astype(
        ml_dtypes.bfloat16
    )
    onesc = np.ones((128, 1), np.float32)
    onest = np.zeros((128, 1), np.float32)
    onest[:last_w] = 1.0
    onesr = np.ones((1, 128), np.float32)
    gm = np.asarray(gamma, np.float32)[None, :]
    bb = np.asarray(beta, np.float32)[None, :]

    in_maps = []
    for k in range(C):
        im = {
            "gidx": gidx_cores[k],
            "doff": dstoff_cores[k],
            "dego": _tile_major(
                np.where(
                    valid,
                    meta["deg_out"][k * NPC + np.minimum(permn, NPC - 1)],
                    np.float32(1.0),
                ).astype(np.float32),
                NG,
                GRP,
                np.float32(1.0),
            ),
            "degi": _tile_major(
                meta["deg_in"][k * NPC : (k + 1) * NPC], NG, GRP, np.float32(1.0)
            ),
            "iota": iota,
            "gm": gm,
            "bb": bb,
            "onesc": onesc.astype(ml_dtypes.bfloat16),
            "onest": onest.astype(ml_dtypes.bfloat16),
            "onesr": onesr,
            "ident": np.eye(128, dtype=np.float32).astype(ml_dtypes.bfloat16),
        }
        if b_nonzero:
            im["bt"] = b[None, :]
        for j in range(XK):
            xcols = np.zeros((128, NPCP), np.float32)
            xcols[:, valid] = xT[
                j * 128 : (j + 1) * 128, k * NPC + permn[valid]
            ]
            im[f"xt{j}"] = xcols.astype(ml_dtypes.bfloat16)
            im[f"wt{j}"] = np.ascontiguousarray(
                Wn[j * 128 : (j + 1) * 128, :]
            ).astype(ml_dtypes.bfloat16)
        in_maps.append(im)

    if cfg.get("SIM"):
        from concourse.bass_interp import MultiCoreSim

        sim = MultiCoreSim(nc, num_cores=C)
        for k, core_sim in sim.cores.items():
            for name, val in in_maps[k].items():
                core_sim.tensor(name)[:] = val
        sim.simulate()
        y = np.empty((N, OUT), np.float32)
        for k in range(C):
            y[k * NPC : (k + 1) * NPC] = sim.cores[k].tensor("ypad")[:NPC]
        return y

    global LAST_NC, LAST_RUN_S
    LAST_NC = nc
    import time as _time

    _t0 = _time.time()
    res = bass_utils.run_bass_kernel_spmd(
        nc,
        in_maps,
        core_ids=list(range(C)),
        trace=cfg.get("TRACE", False),
    )
    LAST_RUN_S = _time.time() - _t0
    LAST_RESULTS = res

    y = np.empty((N, OUT), np.float32)
    for k in range(C):
        y[k * NPC : (k + 1) * NPC] = res.results[k]["ypad"][:NPC]
    return y


# revision 54
# speedup vs baseline: 1.2557x; 1.0023x over previous
"""GCN block (GraphConv + BatchNorm1d + ReLU) on 8 Trainium2 NeuronCores.

Strategy (per sharding hint): partition nodes (and incident edges) across the
8 cores; replicate W/b/gamma/beta; all-reduce BN batch statistics.

Per core k (owns dst nodes [k*NPC, (k+1)*NPC)):
  1. h_k = (x_k @ W) * rsqrt(clip(deg_out_k,1))  (PE matmul; x columns are
     host-permuted so nodes land on partitions grouped by loc%4 class).
  2. TWO AllGathers of h (bf16), split 3:1: nodes with loc%4 in {0,1,2},
     then loc%4 == 3. The int16-indexable "bank" tables are *interleaved
     strided views* of the AG outputs: bank b < 3 lives at row 3j + b of
     part A (elem_step=3), bank 3 at row j of part B, with
     j = owner*(NPCP/4) + loc//4. The linearity holds for any k-of-4
     interleave, so two big collectives serve four gather tables.
  3. Edges are processed in three phases: (0) edges whose source is owned
     by this core gather from the core's OWN h tables and run inside the
     first collective's window; (1) remote banks {0,1,2} after AG part A
     (75% of remote work, hiding AG part B); (2) remote bank 3. Gathers
     are batched per (phase, chunk-of-groups, bank); 32-granular shared
     run sizes (= max over cores). Segment sums use one-hot matmuls
     M^T @ G accumulated in PSUM; each group keeps ONE psum accumulation
     per phase, spilled to bf16 and re-injected via an identity matmul at
     the next phase. Runs straddling 128-slot block boundaries get one
     matmul per straddled block; out-of-segment slots carry doff 255 so
     their one-hot column is zero.
  4. relu(psum * rsqrt(clip(deg_in,1)) [+ b]) via ACT directly from PSUM
     (bf16 out); BN sums via ones-matmuls (single accumulation group per
     PSUM bank); stats combined via AllGather + local sum; y = h*S + T
     with S = gamma*rsqrt(var+eps), T = beta - mu*S (broadcast-AP DVE
     ops); y cast bf16->f32 during the output DMA (SWDGE), pipelined per
     25-group batch.

Host-side work is limited to integer index bookkeeping (bucketing edges by
(core, locality, src-bank, dst-group), degree counting) and layout
transforms (x^T permutation/padding, int16 gather indices). All
floating-point math runs on device.

Run sizes are padded to a structure shared by all 8 cores so a single SPMD
NEFF serves every core; pad slots re-gather the run's last row (HBM page
hit) and carry a dst offset of 255 -> contribute exactly 0. Edges are
sorted by gather row within each bucket for HBM locality.
"""
import math
import os
import sys

sys.path.insert(0, "/opt/trn_rl_repo")

import numpy as np

import concourse.bacc as bacc
import concourse.bass as bass
import concourse.mybir as mybir
import concourse.tile as tile
from concourse import bass_utils

F32 = mybir.dt.float32
BF16 = mybir.dt.bfloat16
I16 = mybir.dt.int16

CFG = dict(
    N=100000,
    E=1600000,
    IN=256,
    OUT=128,
    NCORES=8,
    GRP=128,          # dst nodes per segment group (= psum partition dim)
    NBANKS=4,         # interleaved src banks (bank rows must be < 32768)
    GCHUNK=7,        # groups per chunk (gather batch granularity)
    EPS=1e-5,
    TRACE=False,
)

LAST_RESULTS = None  # set by kernel() for test harness introspection
LAST_NC = None
LAST_RUN_S = None


def _ceil_div(a, b):
    return (a + b - 1) // b


def _wrap16(idx, ncols):
    """int16 idx list -> [128, ncols] tile: idx i at [i%16, i//16], replicated
    8x across the 16-partition groups (one copy per GpSimd Q7 core)."""
    n = idx.shape[0]
    assert n == ncols * 16
    w = np.ascontiguousarray(idx.reshape(ncols, 16).T)
    return np.tile(w, (8, 1))


def _preprocess(cfg, src, dst):
    """Bucket edges by (owner core, interleaved src bank, dst group); build
    per-core gather-index / dst-offset arrays and the shared run structure."""
    N, E = cfg["N"], cfg["E"]
    C, NBANKS, GRP, GC = cfg["NCORES"], cfg["NBANKS"], cfg["GRP"], cfg["GCHUNK"]
    NPC = N // C
    NG = _ceil_div(NPC, GRP)
    NPCP = NG * GRP                # padded nodes per core (x cols zero-padded)
    assert NPCP % NBANKS == 0
    QB = NPCP // NBANKS            # gather rows per owner per bank view
    BANKROWS = QB * C              # rows per bank view of one AG-half output
    assert BANKROWS < 32768

    src = src.astype(np.int64)
    dst = dst.astype(np.int64)
    deg_out = np.bincount(src, minlength=N).astype(np.float32)
    deg_in = np.bincount(dst, minlength=N).astype(np.float32)

    owner = dst // NPC
    loc = src % NPC
    src_owner = src // NPC
    is_local = src_owner == owner  # src row available before any collective
    # bank classes: 0-3 remote (gather from AG output), 4-7 local (from
    # the core's own h tables, no owner term in the row index)
    bank = loc % NBANKS + NBANKS * is_local
    grow = np.where(is_local, loc // NBANKS, src_owner * QB + loc // NBANKS)
    assert grow.max() < 32768
    grp = (dst % NPC) // GRP
    key = (owner * 2 * NBANKS + bank) * NG + grp
    # sort by bucket, then by gather row inside the bucket (HBM locality)
    order = np.lexsort((grow, key))
    s_grow = grow[order]
    s_dst = dst[order]
    s_key = key[order]

    counts = np.bincount(key, minlength=C * 2 * NBANKS * NG).reshape(
        C, 2 * NBANKS, NG
    )
    P = counts.max(axis=0)  # [NBANKS, NG] shared run sizes (32-granular)
    P = ((P + 31) // 32) * 32
    P = np.maximum(P, 32)   # every (b,g) run structurally exists

    # local banks first (overlap the big collective), then remote 3:1
    phases = [(4, 5, 6, 7), (0, 1, 2), (3,)]
    chunks = [list(range(c, min(c + GC, NG))) for c in range(0, NG, GC)]
    run_seq = [
        (b, g) for ph in phases for ch in chunks for b in ph for g in ch
    ]
    # lay out runs; pad each (phase, chunk, bank) unit to a 128 multiple
    run_off = np.zeros((2 * NBANKS, NG), np.int64)
    units = []  # (bank, first_block, n_blocks) in stream order
    pos = 0
    for ph in phases:
        for ch in chunks:
            for b in ph:
                u0 = pos
                for g in ch:
                    run_off[b, g] = pos
                    pos += P[b, g]
                pos = ((pos + 127) // 128) * 128  # unit pad
                units.append((b, u0 // 128, (pos - u0) // 128))
    nidx_tot = int(pos)
    nb_tot = nidx_tot // 128

    # segments: a run may straddle block boundaries; each (run, block)
    # intersection is one segment = one doff column + one full matmul
    # (out-of-segment slots carry doff 255 -> zero one-hot column).
    run_segs = {}  # (b, g) -> list of (block_t, doff_col, slot_lo, slot_hi)
    nseg = 0
    for b, g in run_seq:
        off = int(run_off[b, g])
        end = off + int(P[b, g])
        segs = []
        t = off // 128
        while t * 128 < end:
            lo = max(off, t * 128)
            hi = min(end, (t + 1) * 128)
            segs.append((t, nseg, lo, hi))
            nseg += 1
            t += 1
        run_segs[(b, g)] = segs

    # boundaries of each (k, b, g) bucket in the sorted edge stream
    bkeys = (
        np.arange(C)[:, None, None] * 2 * NBANKS
        + np.arange(2 * NBANKS)[None, :, None]
    ) * NG + np.arange(NG)[None, None, :]
    starts = np.searchsorted(s_key, bkeys.ravel()).reshape(C, 2 * NBANKS, NG)
    ends = np.searchsorted(s_key, bkeys.ravel(), side="right").reshape(
        C, 2 * NBANKS, NG
    )

    gidx_cores = []
    dstoff_cores = []
    for k in range(C):
        gidx = np.zeros(nidx_tot, np.int16)
        doff_cols = np.full((nseg, 128), 255.0, np.float32)
        for b in range(2 * NBANKS):
            for g in range(NG):
                s, e = starts[k, b, g], ends[k, b, g]
                cnt = e - s
                p0 = int(run_off[b, g])
                if cnt:
                    gidx[p0 : p0 + cnt] = s_grow[s:e].astype(np.int16)
                    # pad slots re-gather the last row (HBM page hit)
                    gidx[p0 + cnt : p0 + int(P[b, g])] = gidx[p0 + cnt - 1]
                    offs = ((s_dst[s:e] % NPC) - g * GRP).astype(np.float32)
                    for t, col, lo, hi in run_segs[(b, g)]:
                        a = max(lo, p0)
                        z = min(hi, p0 + cnt)
                        if z > a:
                            doff_cols[col, a - t * 128 : z - t * 128] = offs[
                                a - p0 : z - p0
                            ]
        # unit-pad slots gather row 0 (gidx stays 0) and have no segment
        gidx_cores.append(_wrap16(gidx, nidx_tot // 16))
        dstoff_cores.append(np.ascontiguousarray(doff_cols.T))

    meta = dict(
        NPC=NPC,
        NPCP=NPCP,
        NG=NG,
        QB=QB,
        BANKROWS=BANKROWS,
        nidx_tot=nidx_tot,
        nb_tot=nb_tot,
        nseg=nseg,
        run_segs=run_segs,
        units=units,
        chunks=chunks,
        run_seq=run_seq,
        deg_out=deg_out,
        deg_in=deg_in,
    )
    return meta, gidx_cores, dstoff_cores


def _tile_major(vec, NG, GRP, pad_val):
    """[NPC] -> [GRP, NG]: entry (p, m) = vec[m*GRP + p], padded."""
    out = np.full((NG * GRP,), pad_val, vec.dtype)
    out[: vec.shape[0]] = vec
    return np.ascontiguousarray(out.reshape(NG, GRP).T)


def _build_nc(cfg, meta, b_nonzero=False):
    N, IN, OUT, C = cfg["N"], cfg["IN"], cfg["OUT"], cfg["NCORES"]
    GRP, NBANKS = cfg["GRP"], cfg["NBANKS"]
    NPC, NPCP, NG = meta["NPC"], meta["NPCP"], meta["NG"]
    nidx_tot, nb_tot = meta["nidx_tot"], meta["nb_tot"]
    units = meta["units"]
    XK = _ceil_div(IN, 128)
    assert OUT == 128 and GRP == 128
    last_w = NPC - (NG - 1) * GRP  # valid rows in the last group
    HALF = NPCP // 2               # rows per AG-half input

    nc = bacc.Bacc(
        "TRN2", target_bir_lowering=False, debug=False, num_devices=C
    )

    # ---- external inputs ----
    NXQ = 8  # x DMA split for earlier stage-B start
    xq = NPCP // NXQ
    assert NPCP % NXQ == 0
    xt = [
        nc.dram_tensor(f"xt{j}", [128, NPCP], BF16, kind="ExternalInput")
        for j in range(XK)
    ]
    wt = [
        nc.dram_tensor(f"wt{j}", [128, OUT], BF16, kind="ExternalInput")
        for j in range(XK)
    ]
    gidx_d = nc.dram_tensor("gidx", [128, nidx_tot // 16], I16, kind="ExternalInput")
    doff_d = nc.dram_tensor("doff", [128, meta["nseg"]], F32, kind="ExternalInput")
    dego_d = nc.dram_tensor("dego", [128, NG], F32, kind="ExternalInput")
    degi_d = nc.dram_tensor("degi", [128, NG], F32, kind="ExternalInput")
    iota_d = nc.dram_tensor("iota", [128, GRP], BF16, kind="ExternalInput")
    gm_d = nc.dram_tensor("gm", [1, OUT], F32, kind="ExternalInput")
    bb_d = nc.dram_tensor("bb", [1, OUT], F32, kind="ExternalInput")
    onesc_d = nc.dram_tensor("onesc", [128, 1], BF16, kind="ExternalInput")
    onest_d = nc.dram_tensor("onest", [128, 1], BF16, kind="ExternalInput")
    onesr_d = nc.dram_tensor("onesr", [1, 128], F32, kind="ExternalInput")
    ident_d = nc.dram_tensor("ident", [128, 128], BF16, kind="ExternalInput")
    if b_nonzero:
        bt_d = nc.dram_tensor("bt", [1, OUT], F32, kind="ExternalInput")

    ypad_d = nc.dram_tensor("ypad", [NG * GRP, OUT], F32, kind="ExternalOutput")

    with tile.TileContext(nc) as tc:
        with (
            tc.tile_pool(name="const", bufs=1) as cpool,
            tc.tile_pool(name="dram", bufs=1, space="DRAM") as dpool,
            tc.tile_pool(name="agg", bufs=1) as apool,
            tc.tile_pool(name="mpool", bufs=16) as mpool,
            tc.tile_pool(name="etmp", bufs=4) as epool,
            tc.tile_pool(name="psg", bufs=4, space="PSUM") as pgpool,
            tc.tile_pool(name="psb", bufs=3, space="PSUM") as pbpool,
            tc.tile_pool(name="pstat", bufs=1, space="PSUM") as pspool,
        ):
            # ---- constants / small tiles ----
            iota_t = cpool.tile([128, GRP], BF16)
            dego_t = cpool.tile([128, NG], F32)
            degi_t = cpool.tile([128, NG], F32)
            nsrc_t = cpool.tile([128, NG], F32)
            ndst_t = cpool.tile([128, NG], F32)
            gm_t = cpool.tile([1, OUT], F32)
            bb_t = cpool.tile([1, OUT], F32)
            onesc_t = cpool.tile([128, 1], BF16)
            onest_t = cpool.tile([128, 1], BF16)
            onesr_t = cpool.tile([1, 128], F32)
            gidx_t = cpool.tile([128, nidx_tot // 16], I16)
            doff_t = cpool.tile([128, meta["nseg"]], F32)
            ident_t = cpool.tile([128, 128], BF16)
            nc.sync.dma_start(ident_t[:], ident_d[:])

            nc.sync.dma_start(iota_t[:], iota_d[:])
            nc.sync.dma_start(dego_t[:], dego_d[:])
            nc.sync.dma_start(degi_t[:], degi_d[:])
            nc.sync.dma_start(gm_t[:], gm_d[:])
            nc.sync.dma_start(bb_t[:], bb_d[:])
            nc.sync.dma_start(onesc_t[:], onesc_d[:])
            nc.sync.dma_start(onest_t[:], onest_d[:])
            nc.sync.dma_start(onesr_t[:], onesr_d[:])
            if b_nonzero:
                bt_t = cpool.tile([1, OUT], F32)
                nc.sync.dma_start(bt_t[:], bt_d[:])

            # norms: rsqrt(max(deg, 1))
            for deg_t, norm_t in ((dego_t, nsrc_t), (degi_t, ndst_t)):
                nc.vector.tensor_scalar(
                    norm_t[:], deg_t[:], 1.0, None, op0=mybir.AluOpType.max
                )
                nc.vector.reciprocal(norm_t[:], norm_t[:])
                nc.scalar.activation(
                    norm_t[:], norm_t[:], mybir.ActivationFunctionType.Sqrt
                )

            # internal DRAM for collectives (3:1 interleaved node split)
            _aspace = "Local" if cfg.get("NOCC") else "Shared"
            HA = 3 * NPCP // 4     # nodes with loc%4 in {0,1,2}
            HB = NPCP // 4         # nodes with loc%4 == 3
            h_my_a = dpool.tile([HA, OUT], BF16, name="h_my_a")
            h_my_b = dpool.tile([HB, OUT], BF16, name="h_my_b")
            h_all_a = dpool.tile(
                [C * HA, OUT], BF16, addr_space=_aspace, name="h_all_a"
            )
            h_all_b = dpool.tile(
                [C * HB, OUT], BF16, addr_space=_aspace, name="h_all_b"
            )
            stats_in = dpool.tile([1, 2 * OUT], F32)
            stats_out = dpool.tile([C, 2 * OUT], F32, addr_space=_aspace)

            # relu(norm*agg) output, bf16, [128, NG, OUT]
            agg_t = apool.tile([128, NG, OUT], BF16)

            # ---- stage B: h = (x @ W) * norm_src, cast bf16, store to HBM
            # (staged in SBUF; 2 large DMAs instead of 98 small ones)
            with tc.tile_pool(name="xw", bufs=1) as xwp:
                xts = []
                wts = []
                for j in range(XK):
                    xts.append(xwp.tile([128, NPCP], BF16, name=f"xt_s{j}"))
                    wts.append(xwp.tile([128, OUT], BF16, name=f"wt_s{j}"))
                for j in range(XK):
                    nc.sync.dma_start(wts[j][:], wt[j][:])
                for q in range(NXQ):
                    for j in range(XK):
                        nc.sync.dma_start(
                            xts[j][:, q * xq : (q + 1) * xq],
                            xt[j][:, q * xq : (q + 1) * xq],
                        )
                hstage = xwp.tile([128, NG, OUT], BF16, name="hstage")
                for m in range(NG):
                    ps = pbpool.tile([128, OUT], F32, tag="hps")
                    for j in range(XK):
                        nc.tensor.matmul(
                            ps[:, :],
                            xts[j][:, m * GRP : (m + 1) * GRP],
                            wts[j][:, :],
                            start=(j == 0),
                            stop=(j == XK - 1),
                        )
                    if m % 2 == 0:
                        nc.scalar.activation(
                            hstage[:, m, :],
                            ps[:, :],
                            mybir.ActivationFunctionType.Copy,
                            scale=nsrc_t[:, m : m + 1],
                        )
                    else:
                        nc.vector.tensor_scalar(
                            hstage[:, m, :],
                            ps[:, :],
                            nsrc_t[:, m : m + 1],
                            None,
                            op0=mybir.AluOpType.mult,
                        )
                # partitions c*32+q hold node loc = g*128 + 4q + c (x columns
                # host-permuted): h_my_a row g*96 + 3q + c, h_my_b row g*32+q
                hq = NG // 4
                qr = [
                    (q8 * hq, (q8 + 1) * hq if q8 < 3 else NG)
                    for q8 in range(4)
                ]
                # part-A staging first: AG part A waits only on these
                for a, z in qr:
                    va = h_my_a[a * 96 : z * 96, :].rearrange(
                        "(g q c) f -> q g c f", q=32, c=3
                    )
                    for c_ in range(3):
                        nc.sync.dma_start(
                            va[:, :, c_, :],
                            hstage[c_ * 32 : (c_ + 1) * 32, a:z, :],
                        )
                for a, z in qr:
                    nc.sync.dma_start(
                        h_my_b[a * 32 : z * 32, :].rearrange(
                            "(g p) f -> p g f", p=32
                        ),
                        hstage[96:128, a:z, :],
                    )

            # ---- stage C: two AllGathers (3/4 part, then 1/4 part) ----
            for h_my_h, h_all_h, hr in (
                (h_my_a, h_all_a, HA),
                (h_my_b, h_all_b, HB),
            ):
                if cfg.get("NOCC"):
                    rep = (
                        h_my_h[:]
                        .rearrange("(o r) f -> o r f", o=1)
                        .to_broadcast((C, hr, OUT))
                    )
                    nc.sync.dma_start(
                        h_all_h[:].rearrange("(o r) f -> o r f", o=C), rep
                    )
                else:
                    nc.gpsimd.collective_compute(
                        "AllGather",
                        mybir.AluOpType.bypass,
                        replica_groups=[list(range(C))],
                        ins=[h_my_h[:]],
                        outs=[h_all_h[:]],
                    )

            # index tables are first needed by stage D's gathers - load
            # them after the x/B/AG chain is underway
            nc.sync.dma_start(gidx_t[:], gidx_d[:])
            nc.sync.dma_start(doff_t[:], doff_d[:])

            # interleaved bank views: banks 0..2 -> row 3j + b of part A,
            # bank 3 -> row j of part B; banks 4..7 -> same views over the
            # core's OWN h tables (usable before any collective completes)
            h_banks = [
                h_all_a[:].rearrange("(j k) f -> j (k f)", k=3)[
                    :, b * OUT : (b + 1) * OUT
                ]
                for b in range(3)
            ] + [h_all_b[:]] + [
                h_my_a[:].rearrange("(j k) f -> j (k f)", k=3)[
                    :, b * OUT : (b + 1) * OUT
                ]
                for b in range(3)
            ] + [h_my_b[:]]
            h_esteps = [3 * OUT, 3 * OUT, 3 * OUT, OUT] * 2

            # ---- stage D: gather + one-hot matmul segmented sum ----
            # ---- stage E (inline): relu(psum*ndst) + BN partial sums ----
            # Gathers are batched per (chunk, bank); groups are processed
            # sequentially (their 4 bank runs back-to-back) so each PSUM bank
            # holds at most one pending accumulation group.
            ps_stat = pspool.tile([1, 2 * OUT], F32, name="ps_stat")
            ps_sum = ps_stat[:, 0:OUT]
            ps_sq = ps_stat[:, OUT : 2 * OUT]
            ndone = [0]  # groups completed (for BN-sum start/stop flags)

            def finish_group(g, ps_g):
                """relu + BN-sum accumulation for a completed group psum."""
                if b_nonzero:
                    tmp = epool.tile([128, OUT], F32, tag="etmp")
                    nc.vector.scalar_tensor_tensor(
                        tmp[:],
                        ps_g[:],
                        ndst_t[:, g : g + 1],
                        btile_t[:],
                        op0=mybir.AluOpType.mult,
                        op1=mybir.AluOpType.add,
                    )
                    nc.scalar.activation(
                        agg_t[:, g, :], tmp[:], mybir.ActivationFunctionType.Relu
                    )
                else:
                    nc.scalar.activation(
                        agg_t[:, g, :],
                        ps_g[:],
                        mybir.ActivationFunctionType.Relu,
                        scale=ndst_t[:, g : g + 1],
                    )
                ones = onesc_t if g < NG - 1 else onest_t
                i0 = ndone[0]
                # ps_sum/ps_sq share one bank = ONE accumulation group:
                # start only on the very first matmul, stop on the very last.
                nc.tensor.matmul(
                    ps_sum,
                    ones[:],
                    agg_t[:, g, :],
                    start=(i0 == 0),
                    stop=False,
                )
                sq = epool.tile([128, OUT], BF16, tag="esq")
                nc.scalar.activation(
                    sq[:], agg_t[:, g, :], mybir.ActivationFunctionType.Square
                )
                nc.tensor.matmul(
                    ps_sq,
                    ones[:],
                    sq[:],
                    start=False,
                    stop=(i0 == NG - 1),
                )
                ndone[0] += 1

            if b_nonzero:
                # replicate b across partitions once (PE broadcast)
                ps_b = pbpool.tile([128, OUT], F32, tag="hps", name="ps_b")
                btile_t = cpool.tile([128, OUT], F32)
                nc.tensor.matmul(ps_b[:], onesr_t[:], bt_t[:], start=True, stop=True)
                nc.scalar.activation(
                    btile_t[:], ps_b[:], mybir.ActivationFunctionType.Copy
                )

            run_segs = meta["run_segs"]
            chunks = meta["chunks"]
            nbmax = max(nb for _, _, nb in units)
            dstack = tc.tile_pool(name="gath", bufs=8)
            gpool = dstack.__enter__()
            phases = [(4, 5, 6, 7), (0, 1, 2), (3,)]
            ui = 0
            for pi, ph in enumerate(phases):
                for ci, ch in enumerate(chunks):
                    gts = {}
                    for b in ph:
                        bank, t0, nblk = units[ui]
                        ui += 1
                        assert bank == b
                        Gt = gpool.tile(
                            [128, nbmax, OUT], BF16, tag="G", name=f"G{pi}_{ci}_{b}"
                        )
                        nc.gpsimd.dma_gather(
                            Gt[:, :nblk, :],
                            h_banks[b],
                            gidx_t[:, t0 * 8 : (t0 + nblk) * 8],
                            nblk * 128,
                            nblk * 128,
                            OUT,
                            elem_step=h_esteps[b],
                            single_packet=False,
                        )
                        gts[b] = (Gt, t0)
                    for g in ch:
                        ps_g = pgpool.tile(
                            [128, OUT], F32, tag="aggps", name=f"ps{pi}_{g}"
                        )
                        if pi > 0:
                            # re-inject previous phase's partial (bf16)
                            nc.tensor.matmul(
                                ps_g[:],
                                ident_t[:],
                                agg_t[:, g, :],
                                start=True,
                                stop=False,
                            )
                        for bi, b in enumerate(ph):
                            Gt, t0 = gts[b]
                            segs = run_segs[(b, g)]
                            for si, (t, col, lo, hi) in enumerate(segs):
                                Mt = mpool.tile([128, GRP], BF16, tag="M")
                                nc.vector.tensor_scalar(
                                    Mt[:],
                                    iota_t[:],
                                    doff_t[:, col : col + 1],
                                    None,
                                    op0=mybir.AluOpType.is_equal,
                                )
                                nc.tensor.matmul(
                                    ps_g[:],
                                    Mt[:],
                                    Gt[:, t - t0, :],
                                    start=(pi == 0 and bi == 0 and si == 0),
                                    stop=(
                                        bi == len(ph) - 1
                                        and si == len(segs) - 1
                                    ),
                                )
                        if pi < len(phases) - 1:
                            # spill partial sum to agg_t (bf16), no relu yet
                            nc.scalar.activation(
                                agg_t[:, g, :],
                                ps_g[:],
                                mybir.ActivationFunctionType.Copy,
                            )
                        else:
                            finish_group(g, ps_g)
            dstack.__exit__(None, None, None)
            assert ndone[0] == NG

            # ---- stage F: AllReduce BN stats; build affine S/T tiles ----
            st_sb = cpool.tile([1, 2 * OUT], F32)
            nc.scalar.activation(
                st_sb[:, 0:OUT], ps_sum, mybir.ActivationFunctionType.Copy
            )
            nc.scalar.activation(
                st_sb[:, OUT : 2 * OUT], ps_sq, mybir.ActivationFunctionType.Copy
            )
            nc.sync.dma_start(stats_in[:], st_sb[:])
            if cfg.get("NOCC"):
                rep = (
                    stats_in[:]
                    .rearrange("(o r) f -> o r f", o=1)
                    .to_broadcast((C, 1, 2 * OUT))
                )
                nc.sync.dma_start(
                    stats_out[:].rearrange("(o r) f -> o r f", o=C), rep
                )
            else:
                nc.gpsimd.collective_compute(
                    "AllGather",
                    mybir.AluOpType.bypass,
                    replica_groups=[list(range(C))],
                    ins=[stats_in[:]],
                    outs=[stats_out[:]],
                )
            # per-core partials land row-major; sum the C rows with one
            # ones-matmul (beats C-1 serial DVE adds with per-op drains)
            st8 = cpool.tile([C, 2 * OUT], F32)
            ones8 = cpool.tile([C, 1], F32)
            nc.gpsimd.memset(ones8[:], 1.0)
            nc.sync.dma_start(st8[:], stats_out[:])
            nc.tensor.matmul(ps_stat[:], ones8[:], st8[:], start=True, stop=True)
            st_rb = cpool.tile([1, 2 * OUT], F32)
            nc.scalar.activation(
                st_rb[:], ps_stat[:], mybir.ActivationFunctionType.Copy
            )

            mu = cpool.tile([1, OUT], F32)
            ex2 = cpool.tile([1, OUT], F32)
            var = cpool.tile([1, OUT], F32)
            srow = cpool.tile([1, OUT], F32)
            trow = cpool.tile([1, OUT], F32)
            inv_n = 1.0 / float(N)
            nc.scalar.activation(
                mu[:], st_rb[:, 0:OUT], mybir.ActivationFunctionType.Copy, scale=inv_n
            )
            nc.scalar.activation(
                ex2[:],
                st_rb[:, OUT : 2 * OUT],
                mybir.ActivationFunctionType.Copy,
                scale=inv_n,
            )
            nc.scalar.activation(var[:], mu[:], mybir.ActivationFunctionType.Square)
            nc.vector.tensor_sub(var[:], ex2[:], var[:])
            # var <- rsqrt(var + eps) (ACT Rsqrt is banned for accuracy)
            nc.scalar.activation(
                var[:],
                var[:],
                mybir.ActivationFunctionType.Copy,
                bias=float(cfg["EPS"]),
            )
            nc.vector.reciprocal(var[:], var[:])
            nc.scalar.activation(var[:], var[:], mybir.ActivationFunctionType.Sqrt)
            nc.vector.tensor_mul(srow[:], gm_t[:], var[:])
            nc.vector.tensor_mul(trow[:], mu[:], srow[:])
            nc.vector.tensor_sub(trow[:], bb_t[:], trow[:])

            S_t = cpool.tile([128, OUT], BF16)
            T_t = cpool.tile([128, OUT], BF16)
            ps_S = pgpool.tile([128, OUT], F32, tag="aggps", name="ps_S")
            ps_T = pgpool.tile([128, OUT], F32, tag="aggps", name="ps_T")
            nc.tensor.matmul(ps_S[:], onesr_t[:], srow[:], start=True, stop=True)
            nc.tensor.matmul(ps_T[:], onesr_t[:], trow[:], start=True, stop=True)
            nc.scalar.activation(S_t[:], ps_S[:], mybir.ActivationFunctionType.Copy)
            nc.scalar.activation(T_t[:], ps_T[:], mybir.ActivationFunctionType.Copy)

            # ---- stage G: y = hrelu * S + T (bf16), cast f32 on DMA out ----
            with tc.tile_pool(name="gtmp", bufs=2) as gpool2:
                GB = 8  # groups per batched op
                S_bc = (
                    S_t[:]
                    .rearrange("p (o f) -> p o f", o=1)
                    .to_broadcast((128, GB, OUT))
                )
                T_bc = (
                    T_t[:]
                    .rearrange("p (o f) -> p o f", o=1)
                    .to_broadcast((128, GB, OUT))
                )
                ypad_view = ypad_d[:].rearrange("(g p) f -> p g f", p=128)
                for g0 in range(0, NG, GB):
                    gw = min(GB, NG - g0)
                    tmp = gpool2.tile([128, GB, OUT], BF16, tag="gtmp")
                    nc.vector.tensor_mul(
                        tmp[:, :gw, :],
                        agg_t[:, g0 : g0 + gw, :],
                        S_bc if gw == GB else S_t[:]
                        .rearrange("p (o f) -> p o f", o=1)
                        .to_broadcast((128, gw, OUT)),
                    )
                    nc.vector.tensor_add(
                        agg_t[:, g0 : g0 + gw, :],
                        tmp[:, :gw, :],
                        T_bc if gw == GB else T_t[:]
                        .rearrange("p (o f) -> p o f", o=1)
                        .to_broadcast((128, gw, OUT)),
                    )
                    nc.gpsimd.dma_start(
                        ypad_view[:, g0 : g0 + gw, :],
                        agg_t[:, g0 : g0 + gw, :],
                    )

    nc.compile()
    return nc


def kernel(x, src, dst, W, b, gamma, beta):
    global LAST_RESULTS
    cfg = CFG
    N, E, IN, OUT, C = cfg["N"], cfg["E"], cfg["IN"], cfg["OUT"], cfg["NCORES"]
    GRP = cfg["GRP"]
    assert x.shape == (N, IN) and W.shape == (IN, OUT)
    assert src.shape == (E,) and dst.shape == (E,)

    b = np.asarray(b, np.float32)
    b_nonzero = bool(np.any(b != 0.0))
    meta, gidx_cores, dstoff_cores = _preprocess(cfg, src, dst)
    NPC, NPCP, NG = meta["NPC"], meta["NPCP"], meta["NG"]
    XK = _ceil_div(IN, 128)
    last_w = NPC - (NG - 1) * GRP
    # node permutation: within each 128-node group, order by loc%4 class
    perm = np.concatenate([np.arange(c, 128, 4) for c in range(4)])
    g_ = np.arange(NPCP) // 128
    p_ = np.arange(NPCP) % 128
    permn = g_ * 128 + perm[p_]          # source node (local) per padded col
    valid = permn < NPC

    nc = _build_nc(cfg, meta, b_nonzero=b_nonzero)

    xT = np.ascontiguousarray(np.asarray(x, np.float32).T)  # [IN, N]
    Wn = np.asarray(W, np.float32)
    import ml_dtypes

    iota = np.tile(np.arange(GRP, dtype=np.float32)[None, :], (128, 1)).astype(
        ml_dtypes.bfloat16
    )
    onesc = np.ones((128, 1), np.float32)
    onest = np.zeros((128, 1), np.float32)
    onest[:last_w] = 1.0
    onesr = np.ones((1, 128), np.float32)
    gm = np.asarray(gamma, np.float32)[None, :]
    bb = np.asarray(beta, np.float32)[None, :]

    in_maps = []
    for k in range(C):
        im = {
            "gidx": gidx_cores[k],
            "doff": dstoff_cores[k],
            "dego": _tile_major(
                np.where(
                    valid,
                    meta["deg_out"][k * NPC + np.minimum(permn, NPC - 1)],
                    np.float32(1.0),
                ).astype(np.float32),
                NG,
                GRP,
                np.float32(1.0),
            ),
            "degi": _tile_major(
                meta["deg_in"][k * NPC : (k + 1) * NPC], NG, GRP, np.float32(1.0)
            ),
            "iota": iota,
            "gm": gm,
            "bb": bb,
            "onesc": onesc.astype(ml_dtypes.bfloat16),
            "onest": onest.astype(ml_dtypes.bfloat16),
            "onesr": onesr,
            "ident": np.eye(128, dtype=np.float32).astype(ml_dtypes.bfloat16),
        }
        if b_nonzero:
            im["bt"] = b[None, :]
        for j in range(XK):
            xcols = np.zeros((128, NPCP), np.float32)
            xcols[:, valid] = xT[
                j * 128 : (j + 1) * 128, k * NPC + permn[valid]
            ]
            im[f"xt{j}"] = xcols.astype(ml_dtypes.bfloat16)
            im[f"wt{j}"] = np.ascontiguousarray(
                Wn[j * 128 : (j + 1) * 128, :]
            ).astype(ml_dtypes.bfloat16)
        in_maps.append(im)

    if cfg.get("SIM"):
        from concourse.bass_interp import MultiCoreSim

        sim = MultiCoreSim(nc, num_cores=C)
        for k, core_sim in sim.cores.items():
            for name, val in in_maps[k].items():
                core_sim.tensor(name)[:] = val
        sim.simulate()
        y = np.empty((N, OUT), np.float32)
        for k in range(C):
            y[k * NPC : (k + 1) * NPC] = sim.cores[k].tensor("ypad")[:NPC]
        return y

    global LAST_NC, LAST_RUN_S
    LAST_NC = nc
    import time as _time

    _t0 = _time.time()
    res = bass_utils.run_bass_kernel_spmd(
        nc,
        in_maps,
        core_ids=list(range(C)),
        trace=cfg.get("TRACE", False),
    )
    LAST_RUN_S = _time.time() - _t0
    LAST_RESULTS = res

    y = np.empty((N, OUT), np.float32)
    for k in range(C):
        y[k * NPC : (k + 1) * NPC] = res.results[k]["ypad"][:NPC]
    return y


# revision 58
# speedup vs baseline: 1.2563x; 1.0005x over previous
"""GCN block (GraphConv + BatchNorm1d + ReLU) on 8 Trainium2 NeuronCores.

Strategy (per sharding hint): partition nodes (and incident edges) across the
8 cores; replicate W/b/gamma/beta; all-reduce BN batch statistics.

Per core k (owns dst nodes [k*NPC, (k+1)*NPC)):
  1. h_k = (x_k @ W) * rsqrt(clip(deg_out_k,1))  (PE matmul; x columns are
     host-permuted so nodes land on partitions grouped by loc%4 class).
  2. TWO AllGathers of h (bf16), split 3:1: nodes with loc%4 in {0,1,2},
     then loc%4 == 3. The int16-indexable "bank" tables are *interleaved
     strided views* of the AG outputs: bank b < 3 lives at row 3j + b of
     part A (elem_step=3), bank 3 at row j of part B, with
     j = owner*(NPCP/4) + loc//4. The linearity holds for any k-of-4
     interleave, so two big collectives serve four gather tables.
  3. Edges are processed in three phases: (0) edges whose source is owned
     by this core gather from the core's OWN h tables and run inside the
     first collective's window; (1) remote banks {0,1,2} after AG part A
     (75% of remote work, hiding AG part B); (2) remote bank 3. Gathers
     are batched per (phase, chunk-of-groups, bank); 32-granular shared
     run sizes (= max over cores). Segment sums use one-hot matmuls
     M^T @ G accumulated in PSUM; each group keeps ONE psum accumulation
     per phase, spilled to bf16 and re-injected via an identity matmul at
     the next phase. Runs straddling 128-slot block boundaries get one
     matmul per straddled block; out-of-segment slots carry doff 255 so
     their one-hot column is zero.
  4. relu(psum * rsqrt(clip(deg_in,1)) [+ b]) via ACT directly from PSUM
     (bf16 out); BN sums via ones-matmuls (single accumulation group per
     PSUM bank); stats combined via AllGather + local sum; y = h*S + T
     with S = gamma*rsqrt(var+eps), T = beta - mu*S (broadcast-AP DVE
     ops); y cast bf16->f32 during the output DMA (SWDGE), pipelined per
     25-group batch.

Host-side work is limited to integer index bookkeeping (bucketing edges by
(core, locality, src-bank, dst-group), degree counting) and layout
transforms (x^T permutation/padding, int16 gather indices). All
floating-point math runs on device.

Run sizes are padded to a structure shared by all 8 cores so a single SPMD
NEFF serves every core; pad slots re-gather the run's last row (HBM page
hit) and carry a dst offset of 255 -> contribute exactly 0. Edges are
sorted by gather row within each bucket for HBM locality.
"""
import math
import os
import sys

sys.path.insert(0, "/opt/trn_rl_repo")

import numpy as np

import concourse.bacc as bacc
import concourse.bass as bass
import concourse.mybir as mybir
import concourse.tile as tile
from concourse import bass_utils

F32 = mybir.dt.float32
BF16 = mybir.dt.bfloat16
I16 = mybir.dt.int16

CFG = dict(
    N=100000,
    E=1600000,
    IN=256,
    OUT=128,
    NCORES=8,
    GRP=128,          # dst nodes per segment group (= psum partition dim)
    NBANKS=4,         # interleaved src banks (bank rows must be < 32768)
    GCHUNK=7,        # groups per chunk (gather batch granularity)
    EPS=1e-5,
    TRACE=False,
)

LAST_RESULTS = None  # set by kernel() for test harness introspection
LAST_NC = None
LAST_RUN_S = None


def _ceil_div(a, b):
    return (a + b - 1) // b


def _wrap16(idx, ncols):
    """int16 idx list -> [128, ncols] tile: idx i at [i%16, i//16], replicated
    8x across the 16-partition groups (one copy per GpSimd Q7 core)."""
    n = idx.shape[0]
    assert n == ncols * 16
    w = np.ascontiguousarray(idx.reshape(ncols, 16).T)
    return np.tile(w, (8, 1))


def _preprocess(cfg, src, dst):
    """Bucket edges by (owner core, interleaved src bank, dst group); build
    per-core gather-index / dst-offset arrays and the shared run structure."""
    N, E = cfg["N"], cfg["E"]
    C, NBANKS, GRP, GC = cfg["NCORES"], cfg["NBANKS"], cfg["GRP"], cfg["GCHUNK"]
    NPC = N // C
    NG = _ceil_div(NPC, GRP)
    NPCP = NG * GRP                # padded nodes per core (x cols zero-padded)
    assert NPCP % NBANKS == 0
    QB = NPCP // NBANKS            # gather rows per owner per bank view
    BANKROWS = QB * C              # rows per bank view of one AG-half output
    assert BANKROWS < 32768

    src = src.astype(np.int64)
    dst = dst.astype(np.int64)
    deg_out = np.bincount(src, minlength=N).astype(np.float32)
    deg_in = np.bincount(dst, minlength=N).astype(np.float32)

    owner = dst // NPC
    loc = src % NPC
    src_owner = src // NPC
    is_local = src_owner == owner  # src row available before any collective
    # bank classes: 0-3 remote (gather from AG output), 4-7 local (from
    # the core's own h tables, no owner term in the row index)
    bank = loc % NBANKS + NBANKS * is_local
    grow = np.where(is_local, loc // NBANKS, src_owner * QB + loc // NBANKS)
    assert grow.max() < 32768
    grp = (dst % NPC) // GRP
    key = (owner * 2 * NBANKS + bank) * NG + grp
    # sort by bucket, then by gather row inside the bucket (HBM locality)
    order = np.lexsort((grow, key))
    s_grow = grow[order]
    s_dst = dst[order]
    s_key = key[order]

    counts = np.bincount(key, minlength=C * 2 * NBANKS * NG).reshape(
        C, 2 * NBANKS, NG
    )
    P = counts.max(axis=0)  # [NBANKS, NG] shared run sizes (32-granular)
    P = ((P + 31) // 32) * 32
    P = np.maximum(P, 32)   # every (b,g) run structurally exists

    # local banks first (overlap the big collective), then remote 3:1
    phases = [(4, 5, 6, 7), (0, 1, 2), (3,)]
    chunks = [list(range(c, min(c + GC, NG))) for c in range(0, NG, GC)]
    run_seq = [
        (b, g) for ph in phases for ch in chunks for b in ph for g in ch
    ]
    # lay out runs; pad each (phase, chunk, bank) unit to a 128 multiple
    run_off = np.zeros((2 * NBANKS, NG), np.int64)
    units = []  # (bank, first_block, n_blocks) in stream order
    pos = 0
    for ph in phases:
        for ch in chunks:
            for b in ph:
                u0 = pos
                for g in ch:
                    run_off[b, g] = pos
                    pos += P[b, g]
                pos = ((pos + 127) // 128) * 128  # unit pad
                units.append((b, u0 // 128, (pos - u0) // 128))
    nidx_tot = int(pos)
    nb_tot = nidx_tot // 128

    # segments: a run may straddle block boundaries; each (run, block)
    # intersection is one segment = one doff column + one full matmul
    # (out-of-segment slots carry doff 255 -> zero one-hot column).
    run_segs = {}  # (b, g) -> list of (block_t, doff_col, slot_lo, slot_hi)
    nseg = 0
    for b, g in run_seq:
        off = int(run_off[b, g])
        end = off + int(P[b, g])
        segs = []
        t = off // 128
        while t * 128 < end:
            lo = max(off, t * 128)
            hi = min(end, (t + 1) * 128)
            segs.append((t, nseg, lo, hi))
            nseg += 1
            t += 1
        run_segs[(b, g)] = segs

    # boundaries of each (k, b, g) bucket in the sorted edge stream
    bkeys = (
        np.arange(C)[:, None, None] * 2 * NBANKS
        + np.arange(2 * NBANKS)[None, :, None]
    ) * NG + np.arange(NG)[None, None, :]
    starts = np.searchsorted(s_key, bkeys.ravel()).reshape(C, 2 * NBANKS, NG)
    ends = np.searchsorted(s_key, bkeys.ravel(), side="right").reshape(
        C, 2 * NBANKS, NG
    )

    gidx_cores = []
    dstoff_cores = []
    for k in range(C):
        gidx = np.zeros(nidx_tot, np.int16)
        doff_cols = np.full((nseg, 128), 255.0, np.float32)
        for b in range(2 * NBANKS):
            for g in range(NG):
                s, e = starts[k, b, g], ends[k, b, g]
                cnt = e - s
                p0 = int(run_off[b, g])
                if cnt:
                    gidx[p0 : p0 + cnt] = s_grow[s:e].astype(np.int16)
                    # pad slots re-gather the last row (HBM page hit)
                    gidx[p0 + cnt : p0 + int(P[b, g])] = gidx[p0 + cnt - 1]
                    offs = ((s_dst[s:e] % NPC) - g * GRP).astype(np.float32)
                    for t, col, lo, hi in run_segs[(b, g)]:
                        a = max(lo, p0)
                        z = min(hi, p0 + cnt)
                        if z > a:
                            doff_cols[col, a - t * 128 : z - t * 128] = offs[
                                a - p0 : z - p0
                            ]
        # unit-pad slots gather row 0 (gidx stays 0) and have no segment
        gidx_cores.append(_wrap16(gidx, nidx_tot // 16))
        dstoff_cores.append(np.ascontiguousarray(doff_cols.T))

    meta = dict(
        NPC=NPC,
        NPCP=NPCP,
        NG=NG,
        QB=QB,
        BANKROWS=BANKROWS,
        nidx_tot=nidx_tot,
        nb_tot=nb_tot,
        nseg=nseg,
        run_segs=run_segs,
        units=units,
        chunks=chunks,
        run_seq=run_seq,
        deg_out=deg_out,
        deg_in=deg_in,
    )
    return meta, gidx_cores, dstoff_cores


def _tile_major(vec, NG, GRP, pad_val):
    """[NPC] -> [GRP, NG]: entry (p, m) = vec[m*GRP + p], padded."""
    out = np.full((NG * GRP,), pad_val, vec.dtype)
    out[: vec.shape[0]] = vec
    return np.ascontiguousarray(out.reshape(NG, GRP).T)


def _build_nc(cfg, meta, b_nonzero=False):
    N, IN, OUT, C = cfg["N"], cfg["IN"], cfg["OUT"], cfg["NCORES"]
    GRP, NBANKS = cfg["GRP"], cfg["NBANKS"]
    NPC, NPCP, NG = meta["NPC"], meta["NPCP"], meta["NG"]
    nidx_tot, nb_tot = meta["nidx_tot"], meta["nb_tot"]
    units = meta["units"]
    XK = _ceil_div(IN, 128)
    assert OUT == 128 and GRP == 128
    last_w = NPC - (NG - 1) * GRP  # valid rows in the last group
    HALF = NPCP // 2               # rows per AG-half input

    nc = bacc.Bacc(
        "TRN2", target_bir_lowering=False, debug=False, num_devices=C
    )

    # ---- external inputs ----
    NXQ = 8  # x DMA split for earlier stage-B start
    xq = NPCP // NXQ
    assert NPCP % NXQ == 0
    xt = [
        nc.dram_tensor(f"xt{j}", [128, NPCP], BF16, kind="ExternalInput")
        for j in range(XK)
    ]
    wt = [
        nc.dram_tensor(f"wt{j}", [128, OUT], BF16, kind="ExternalInput")
        for j in range(XK)
    ]
    gidx_d = nc.dram_tensor("gidx", [128, nidx_tot // 16], I16, kind="ExternalInput")
    doff_d = nc.dram_tensor("doff", [128, meta["nseg"]], F32, kind="ExternalInput")
    dego_d = nc.dram_tensor("dego", [128, NG], F32, kind="ExternalInput")
    degi_d = nc.dram_tensor("degi", [128, NG], F32, kind="ExternalInput")
    iota_d = nc.dram_tensor("iota", [128, GRP], BF16, kind="ExternalInput")
    gm_d = nc.dram_tensor("gm", [1, OUT], F32, kind="ExternalInput")
    bb_d = nc.dram_tensor("bb", [1, OUT], F32, kind="ExternalInput")
    onesc_d = nc.dram_tensor("onesc", [128, 1], BF16, kind="ExternalInput")
    onest_d = nc.dram_tensor("onest", [128, 1], BF16, kind="ExternalInput")
    onesr_d = nc.dram_tensor("onesr", [1, 128], F32, kind="ExternalInput")
    ident_d = nc.dram_tensor("ident", [128, 128], BF16, kind="ExternalInput")
    if b_nonzero:
        bt_d = nc.dram_tensor("bt", [1, OUT], F32, kind="ExternalInput")

    ypad_d = nc.dram_tensor("ypad", [NG * GRP, OUT], F32, kind="ExternalOutput")

    with tile.TileContext(nc) as tc:
        with (
            tc.tile_pool(name="const", bufs=1) as cpool,
            tc.tile_pool(name="dram", bufs=1, space="DRAM") as dpool,
            tc.tile_pool(name="agg", bufs=1) as apool,
            tc.tile_pool(name="mpool", bufs=32) as mpool,
            tc.tile_pool(name="etmp", bufs=8) as epool,
            tc.tile_pool(name="psg", bufs=4, space="PSUM") as pgpool,
            tc.tile_pool(name="psb", bufs=3, space="PSUM") as pbpool,
            tc.tile_pool(name="pstat", bufs=1, space="PSUM") as pspool,
        ):
            # ---- constants / small tiles ----
            iota_t = cpool.tile([128, GRP], BF16)
            dego_t = cpool.tile([128, NG], F32)
            degi_t = cpool.tile([128, NG], F32)
            nsrc_t = cpool.tile([128, NG], F32)
            ndst_t = cpool.tile([128, NG], F32)
            gm_t = cpool.tile([1, OUT], F32)
            bb_t = cpool.tile([1, OUT], F32)
            onesc_t = cpool.tile([128, 1], BF16)
            onest_t = cpool.tile([128, 1], BF16)
            onesr_t = cpool.tile([1, 128], F32)
            gidx_t = cpool.tile([128, nidx_tot // 16], I16)
            doff_t = cpool.tile([128, meta["nseg"]], F32)
            ident_t = cpool.tile([128, 128], BF16)
            nc.sync.dma_start(ident_t[:], ident_d[:])

            nc.sync.dma_start(iota_t[:], iota_d[:])
            nc.sync.dma_start(dego_t[:], dego_d[:])
            nc.sync.dma_start(degi_t[:], degi_d[:])
            nc.sync.dma_start(gm_t[:], gm_d[:])
            nc.sync.dma_start(bb_t[:], bb_d[:])
            nc.sync.dma_start(onesc_t[:], onesc_d[:])
            nc.sync.dma_start(onest_t[:], onest_d[:])
            nc.sync.dma_start(onesr_t[:], onesr_d[:])
            if b_nonzero:
                bt_t = cpool.tile([1, OUT], F32)
                nc.sync.dma_start(bt_t[:], bt_d[:])

            # norms: rsqrt(max(deg, 1))
            for deg_t, norm_t in ((dego_t, nsrc_t), (degi_t, ndst_t)):
                nc.vector.tensor_scalar(
                    norm_t[:], deg_t[:], 1.0, None, op0=mybir.AluOpType.max
                )
                nc.vector.reciprocal(norm_t[:], norm_t[:])
                nc.scalar.activation(
                    norm_t[:], norm_t[:], mybir.ActivationFunctionType.Sqrt
                )

            # internal DRAM for collectives (3:1 interleaved node split)
            _aspace = "Local" if cfg.get("NOCC") else "Shared"
            HA = 3 * NPCP // 4     # nodes with loc%4 in {0,1,2}
            HB = NPCP // 4         # nodes with loc%4 == 3
            h_my_a = dpool.tile([HA, OUT], BF16, name="h_my_a")
            h_my_b = dpool.tile([HB, OUT], BF16, name="h_my_b")
            h_all_a = dpool.tile(
                [C * HA, OUT], BF16, addr_space=_aspace, name="h_all_a"
            )
            h_all_b = dpool.tile(
                [C * HB, OUT], BF16, addr_space=_aspace, name="h_all_b"
            )
            stats_in = dpool.tile([1, 2 * OUT], F32)
            stats_out = dpool.tile([C, 2 * OUT], F32, addr_space=_aspace)

            # relu(norm*agg) output, bf16, [128, NG, OUT]
            agg_t = apool.tile([128, NG, OUT], BF16)

            # ---- stage B: h = (x @ W) * norm_src, cast bf16, store to HBM
            # (staged in SBUF; 2 large DMAs instead of 98 small ones)
            with tc.tile_pool(name="xw", bufs=1) as xwp:
                xts = []
                wts = []
                for j in range(XK):
                    xts.append(xwp.tile([128, NPCP], BF16, name=f"xt_s{j}"))
                    wts.append(xwp.tile([128, OUT], BF16, name=f"wt_s{j}"))
                for j in range(XK):
                    nc.sync.dma_start(wts[j][:], wt[j][:])
                for q in range(NXQ):
                    for j in range(XK):
                        nc.sync.dma_start(
                            xts[j][:, q * xq : (q + 1) * xq],
                            xt[j][:, q * xq : (q + 1) * xq],
                        )
                hstage = xwp.tile([128, NG, OUT], BF16, name="hstage")
                for m in range(NG):
                    ps = pbpool.tile([128, OUT], F32, tag="hps")
                    for j in range(XK):
                        nc.tensor.matmul(
                            ps[:, :],
                            xts[j][:, m * GRP : (m + 1) * GRP],
                            wts[j][:, :],
                            start=(j == 0),
                            stop=(j == XK - 1),
                        )
                    if m % 2 == 0:
                        nc.scalar.activation(
                            hstage[:, m, :],
                            ps[:, :],
                            mybir.ActivationFunctionType.Copy,
                            scale=nsrc_t[:, m : m + 1],
                        )
                    else:
                        nc.vector.tensor_scalar(
                            hstage[:, m, :],
                            ps[:, :],
                            nsrc_t[:, m : m + 1],
                            None,
                            op0=mybir.AluOpType.mult,
                        )
                # partitions c*32+q hold node loc = g*128 + 4q + c (x columns
                # host-permuted): h_my_a row g*96 + 3q + c, h_my_b row g*32+q
                hq = NG // 4
                qr = [
                    (q8 * hq, (q8 + 1) * hq if q8 < 3 else NG)
                    for q8 in range(4)
                ]
                # part-A staging first: AG part A waits only on these
                for a, z in qr:
                    va = h_my_a[a * 96 : z * 96, :].rearrange(
                        "(g q c) f -> q g c f", q=32, c=3
                    )
                    for c_ in range(3):
                        nc.sync.dma_start(
                            va[:, :, c_, :],
                            hstage[c_ * 32 : (c_ + 1) * 32, a:z, :],
                        )
                for a, z in qr:
                    nc.sync.dma_start(
                        h_my_b[a * 32 : z * 32, :].rearrange(
                            "(g p) f -> p g f", p=32
                        ),
                        hstage[96:128, a:z, :],
                    )

            # ---- stage C: two AllGathers (3/4 part, then 1/4 part) ----
            for h_my_h, h_all_h, hr in (
                (h_my_a, h_all_a, HA),
                (h_my_b, h_all_b, HB),
            ):
                if cfg.get("NOCC"):
                    rep = (
                        h_my_h[:]
                        .rearrange("(o r) f -> o r f", o=1)
                        .to_broadcast((C, hr, OUT))
                    )
                    nc.sync.dma_start(
                        h_all_h[:].rearrange("(o r) f -> o r f", o=C), rep
                    )
                else:
                    nc.gpsimd.collective_compute(
                        "AllGather",
                        mybir.AluOpType.bypass,
                        replica_groups=[list(range(C))],
                        ins=[h_my_h[:]],
                        outs=[h_all_h[:]],
                    )

            # index tables are first needed by stage D's gathers - load
            # them after the x/B/AG chain is underway
            nc.sync.dma_start(gidx_t[:], gidx_d[:])
            nc.sync.dma_start(doff_t[:], doff_d[:])

            # interleaved bank views: banks 0..2 -> row 3j + b of part A,
            # bank 3 -> row j of part B; banks 4..7 -> same views over the
            # core's OWN h tables (usable before any collective completes)
            h_banks = [
                h_all_a[:].rearrange("(j k) f -> j (k f)", k=3)[
                    :, b * OUT : (b + 1) * OUT
                ]
                for b in range(3)
            ] + [h_all_b[:]] + [
                h_my_a[:].rearrange("(j k) f -> j (k f)", k=3)[
                    :, b * OUT : (b + 1) * OUT
                ]
                for b in range(3)
            ] + [h_my_b[:]]
            h_esteps = [3 * OUT, 3 * OUT, 3 * OUT, OUT] * 2

            # ---- stage D: gather + one-hot matmul segmented sum ----
            # ---- stage E (inline): relu(psum*ndst) + BN partial sums ----
            # Gathers are batched per (chunk, bank); groups are processed
            # sequentially (their 4 bank runs back-to-back) so each PSUM bank
            # holds at most one pending accumulation group.
            ps_stat = pspool.tile([1, 2 * OUT], F32, name="ps_stat")
            ps_sum = ps_stat[:, 0:OUT]
            ps_sq = ps_stat[:, OUT : 2 * OUT]
            ndone = [0]  # groups completed (for BN-sum start/stop flags)

            def finish_group(g, ps_g):
                """relu + BN-sum accumulation for a completed group psum."""
                if b_nonzero:
                    tmp = epool.tile([128, OUT], F32, tag="etmp")
                    nc.vector.scalar_tensor_tensor(
                        tmp[:],
                        ps_g[:],
                        ndst_t[:, g : g + 1],
                        btile_t[:],
                        op0=mybir.AluOpType.mult,
                        op1=mybir.AluOpType.add,
                    )
                    nc.scalar.activation(
                        agg_t[:, g, :], tmp[:], mybir.ActivationFunctionType.Relu
                    )
                else:
                    nc.scalar.activation(
                        agg_t[:, g, :],
                        ps_g[:],
                        mybir.ActivationFunctionType.Relu,
                        scale=ndst_t[:, g : g + 1],
                    )
                ones = onesc_t if g < NG - 1 else onest_t
                i0 = ndone[0]
                # ps_sum/ps_sq share one bank = ONE accumulation group:
                # start only on the very first matmul, stop on the very last.
                nc.tensor.matmul(
                    ps_sum,
                    ones[:],
                    agg_t[:, g, :],
                    start=(i0 == 0),
                    stop=False,
                )
                sq = epool.tile([128, OUT], BF16, tag="esq")
                nc.scalar.activation(
                    sq[:], agg_t[:, g, :], mybir.ActivationFunctionType.Square
                )
                nc.tensor.matmul(
                    ps_sq,
                    ones[:],
                    sq[:],
                    start=False,
                    stop=(i0 == NG - 1),
                )
                ndone[0] += 1

            if b_nonzero:
                # replicate b across partitions once (PE broadcast)
                ps_b = pbpool.tile([128, OUT], F32, tag="hps", name="ps_b")
                btile_t = cpool.tile([128, OUT], F32)
                nc.tensor.matmul(ps_b[:], onesr_t[:], bt_t[:], start=True, stop=True)
                nc.scalar.activation(
                    btile_t[:], ps_b[:], mybir.ActivationFunctionType.Copy
                )

            run_segs = meta["run_segs"]
            chunks = meta["chunks"]
            nbmax = max(nb for _, _, nb in units)
            dstack = tc.tile_pool(name="gath", bufs=8)
            gpool = dstack.__enter__()
            phases = [(4, 5, 6, 7), (0, 1, 2), (3,)]
            ui = 0
            for pi, ph in enumerate(phases):
                for ci, ch in enumerate(chunks):
                    gts = {}
                    for b in ph:
                        bank, t0, nblk = units[ui]
                        ui += 1
                        assert bank == b
                        Gt = gpool.tile(
                            [128, nbmax, OUT], BF16, tag="G", name=f"G{pi}_{ci}_{b}"
                        )
                        nc.gpsimd.dma_gather(
                            Gt[:, :nblk, :],
                            h_banks[b],
                            gidx_t[:, t0 * 8 : (t0 + nblk) * 8],
                            nblk * 128,
                            nblk * 128,
                            OUT,
                            elem_step=h_esteps[b],
                            single_packet=False,
                        )
                        gts[b] = (Gt, t0)
                    for g in ch:
                        ps_g = pgpool.tile(
                            [128, OUT], F32, tag="aggps", name=f"ps{pi}_{g}"
                        )
                        if pi > 0:
                            # re-inject previous phase's partial (bf16)
                            nc.tensor.matmul(
                                ps_g[:],
                                ident_t[:],
                                agg_t[:, g, :],
                                start=True,
                                stop=False,
                            )
                        for bi, b in enumerate(ph):
                            Gt, t0 = gts[b]
                            segs = run_segs[(b, g)]
                            for si, (t, col, lo, hi) in enumerate(segs):
                                Mt = mpool.tile([128, GRP], BF16, tag="M")
                                nc.vector.tensor_scalar(
                                    Mt[:],
                                    iota_t[:],
                                    doff_t[:, col : col + 1],
                                    None,
                                    op0=mybir.AluOpType.is_equal,
                                )
                                nc.tensor.matmul(
                                    ps_g[:],
                                    Mt[:],
                                    Gt[:, t - t0, :],
                                    start=(pi == 0 and bi == 0 and si == 0),
                                    stop=(
                                        bi == len(ph) - 1
                                        and si == len(segs) - 1
                                    ),
                                )
                        if pi < len(phases) - 1:
                            # spill partial sum to agg_t (bf16), no relu yet
                            nc.scalar.activation(
                                agg_t[:, g, :],
                                ps_g[:],
                                mybir.ActivationFunctionType.Copy,
                            )
                        else:
                            finish_group(g, ps_g)
            dstack.__exit__(None, None, None)
            assert ndone[0] == NG

            # ---- stage F: AllReduce BN stats; build affine S/T tiles ----
            st_sb = cpool.tile([1, 2 * OUT], F32)
            nc.scalar.activation(
                st_sb[:, 0:OUT], ps_sum, mybir.ActivationFunctionType.Copy
            )
            nc.scalar.activation(
                st_sb[:, OUT : 2 * OUT], ps_sq, mybir.ActivationFunctionType.Copy
            )
            nc.sync.dma_start(stats_in[:], st_sb[:])
            if cfg.get("NOCC"):
                rep = (
                    stats_in[:]
                    .rearrange("(o r) f -> o r f", o=1)
                    .to_broadcast((C, 1, 2 * OUT))
                )
                nc.sync.dma_start(
                    stats_out[:].rearrange("(o r) f -> o r f", o=C), rep
                )
            else:
                nc.gpsimd.collective_compute(
                    "AllGather",
                    mybir.AluOpType.bypass,
                    replica_groups=[list(range(C))],
                    ins=[stats_in[:]],
                    outs=[stats_out[:]],
                )
            # per-core partials land row-major; sum the C rows with one
            # ones-matmul (beats C-1 serial DVE adds with per-op drains)
            st8 = cpool.tile([C, 2 * OUT], F32)
            ones8 = cpool.tile([C, 1], F32)
            nc.gpsimd.memset(ones8[:], 1.0)
            nc.sync.dma_start(st8[:], stats_out[:])
            nc.tensor.matmul(ps_stat[:], ones8[:], st8[:], start=True, stop=True)
            st_rb = cpool.tile([1, 2 * OUT], F32)
            nc.scalar.activation(
                st_rb[:], ps_stat[:], mybir.ActivationFunctionType.Copy
            )

            mu = cpool.tile([1, OUT], F32)
            ex2 = cpool.tile([1, OUT], F32)
            var = cpool.tile([1, OUT], F32)
            srow = cpool.tile([1, OUT], F32)
            trow = cpool.tile([1, OUT], F32)
            inv_n = 1.0 / float(N)
            nc.scalar.activation(
                mu[:], st_rb[:, 0:OUT], mybir.ActivationFunctionType.Copy, scale=inv_n
            )
            nc.scalar.activation(
                ex2[:],
                st_rb[:, OUT : 2 * OUT],
                mybir.ActivationFunctionType.Copy,
                scale=inv_n,
            )
            nc.scalar.activation(var[:], mu[:], mybir.ActivationFunctionType.Square)
            nc.vector.tensor_sub(var[:], ex2[:], var[:])
            # var <- rsqrt(var + eps) (ACT Rsqrt is banned for accuracy)
            nc.scalar.activation(
                var[:],
                var[:],
                mybir.ActivationFunctionType.Copy,
                bias=float(cfg["EPS"]),
            )
            nc.vector.reciprocal(var[:], var[:])
            nc.scalar.activation(var[:], var[:], mybir.ActivationFunctionType.Sqrt)
            nc.vector.tensor_mul(srow[:], gm_t[:], var[:])
            nc.vector.tensor_mul(trow[:], mu[:], srow[:])
            nc.vector.tensor_sub(trow[:], bb_t[:], trow[:])

            S_t = cpool.tile([128, OUT], BF16)
            T_t = cpool.tile([128, OUT], BF16)
            ps_S = pgpool.tile([128, OUT], F32, tag="aggps", name="ps_S")
            ps_T = pgpool.tile([128, OUT], F32, tag="aggps", name="ps_T")
            nc.tensor.matmul(ps_S[:], onesr_t[:], srow[:], start=True, stop=True)
            nc.tensor.matmul(ps_T[:], onesr_t[:], trow[:], start=True, stop=True)
            nc.scalar.activation(S_t[:], ps_S[:], mybir.ActivationFunctionType.Copy)
            nc.scalar.activation(T_t[:], ps_T[:], mybir.ActivationFunctionType.Copy)

            # ---- stage G: y = hrelu * S + T (bf16), cast f32 on DMA out ----
            with tc.tile_pool(name="gtmp", bufs=2) as gpool2:
                GB = 8  # groups per batched op
                S_bc = (
                    S_t[:]
                    .rearrange("p (o f) -> p o f", o=1)
                    .to_broadcast((128, GB, OUT))
                )
                T_bc = (
                    T_t[:]
                    .rearrange("p (o f) -> p o f", o=1)
                    .to_broadcast((128, GB, OUT))
                )
                ypad_view = ypad_d[:].rearrange("(g p) f -> p g f", p=128)
                for g0 in range(0, NG, GB):
                    gw = min(GB, NG - g0)
                    tmp = gpool2.tile([128, GB, OUT], BF16, tag="gtmp")
                    nc.vector.tensor_mul(
                        tmp[:, :gw, :],
                        agg_t[:, g0 : g0 + gw, :],
                        S_bc if gw == GB else S_t[:]
                        .rearrange("p (o f) -> p o f", o=1)
                        .to_broadcast((128, gw, OUT)),
                    )
                    nc.vector.tensor_add(
                        agg_t[:, g0 : g0 + gw, :],
                        tmp[:, :gw, :],
                        T_bc if gw == GB else T_t[:]
                        .rearrange("p (o f) -> p o f", o=1)
                        .to_broadcast((128, gw, OUT)),
                    )
                    nc.gpsimd.dma_start(
                        ypad_view[:, g0 : g0 + gw, :],
                        agg_t[:, g0 : g0 + gw, :],
                    )

    nc.compile()
    return nc


def kernel(x, src, dst, W, b, gamma, beta):
    global LAST_RESULTS
    cfg = CFG
    N, E, IN, OUT, C = cfg["N"], cfg["E"], cfg["IN"], cfg["OUT"], cfg["NCORES"]
    GRP = cfg["GRP"]
    assert x.shape == (N, IN) and W.shape == (IN, OUT)
    assert src.shape == (E,) and dst.shape == (E,)

    b = np.asarray(b, np.float32)
    b_nonzero = bool(np.any(b != 0.0))
    meta, gidx_cores, dstoff_cores = _preprocess(cfg, src, dst)
    NPC, NPCP, NG = meta["NPC"], meta["NPCP"], meta["NG"]
    XK = _ceil_div(IN, 128)
    last_w = NPC - (NG - 1) * GRP
    # node permutation: within each 128-node group, order by loc%4 class
    perm = np.concatenate([np.arange(c, 128, 4) for c in range(4)])
    g_ = np.arange(NPCP) // 128
    p_ = np.arange(NPCP) % 128
    permn = g_ * 128 + perm[p_]          # source node (local) per padded col
    valid = permn < NPC

    nc = _build_nc(cfg, meta, b_nonzero=b_nonzero)

    xT = np.ascontiguousarray(np.asarray(x, np.float32).T)  # [IN, N]
    Wn = np.asarray(W, np.float32)
    import ml_dtypes

    iota = np.tile(np.arange(GRP, dtype=np.float32)[None, :], (128, 1)).astype(
        ml_dtypes.bfloat16
    )
    onesc = np.ones((128, 1), np.float32)
    onest = np.zeros((128, 1), np.float32)
    onest[:last_w] = 1.0
    onesr = np.ones((1, 128), np.float32)
    gm = np.asarray(gamma, np.float32)[None, :]
    bb = np.asarray(beta, np.float32)[None, :]

    in_maps = []
    for k in range(C):
        im = {
            "gidx": gidx_cores[k],
            "doff": dstoff_cores[k],
            "dego": _tile_major(
                np.where(
                    valid,
                    meta["deg_out"][k * NPC + np.minimum(permn, NPC - 1)],
                    np.float32(1.0),
                ).astype(np.float32),
                NG,
                GRP,
                np.float32(1.0),
            ),
            "degi": _tile_major(
                meta["deg_in"][k * NPC : (k + 1) * NPC], NG, GRP, np.float32(1.0)
            ),
            "iota": iota,
            "gm": gm,
            "bb": bb,
            "onesc": onesc.astype(ml_dtypes.bfloat16),
            "onest": onest.astype(ml_dtypes.bfloat16),
            "onesr": onesr,
            "ident": np.eye(128, dtype=np.float32).astype(ml_dtypes.bfloat16),
        }
        if b_nonzero:
            im["bt"] = b[None, :]
        for j in range(XK):
            xcols = np.zeros((128, NPCP), np.float32)
            xcols[:, valid] = xT[
                j * 128 : (j + 1) * 128, k * NPC + permn[valid]
            ]
            im[f"xt{j}"] = xcols.astype(ml_dtypes.bfloat16)
            im[f"wt{j}"] = np.ascontiguousarray(
                Wn[j * 128 : (j + 1) * 128, :]
            ).astype(ml_dtypes.bfloat16)
        in_maps.append(im)

    if cfg.get("SIM"):
        from concourse.bass_interp import MultiCoreSim

        sim = MultiCoreSim(nc, num_cores=C)
        for k, core_sim in sim.cores.items():
            for name, val in in_maps[k].items():
                core_sim.tensor(name)[:] = val
        sim.simulate()
        y = np.empty((N, OUT), np.float32)
        for k in range(C):
            y[k * NPC : (k + 1) * NPC] = sim.cores[k].tensor("ypad")[:NPC]
        return y

    global LAST_NC, LAST_RUN_S
    LAST_NC = nc
    import time as _time

    _t0 = _time.time()
    res = bass_utils.run_bass_kernel_spmd(
        nc,
        in_maps,
        core_ids=list(range(C)),
        trace=cfg.get("TRACE", False),
    )
    LAST_RUN_S = _time.time() - _t0
    LAST_RESULTS = res

    y = np.empty((N, OUT), np.float32)
    for k in range(C):
        y[k * NPC : (k + 1) * NPC] = res.results[k]["ypad"][:NPC]
    return y


# revision 60
# speedup vs baseline: 1.2577x; 1.0012x over previous
"""GCN block (GraphConv + BatchNorm1d + ReLU) on 8 Trainium2 NeuronCores.

Strategy (per sharding hint): partition nodes (and incident edges) across the
8 cores; replicate W/b/gamma/beta; all-reduce BN batch statistics.

Per core k (owns dst nodes [k*NPC, (k+1)*NPC)):
  1. h_k = (x_k @ W) * rsqrt(clip(deg_out_k,1))  (PE matmul; x columns are
     host-permuted so nodes land on partitions grouped by loc%4 class).
  2. TWO AllGathers of h (bf16), split 3:1: nodes with loc%4 in {0,1,2},
     then loc%4 == 3. The int16-indexable "bank" tables are *interleaved
     strided views* of the AG outputs: bank b < 3 lives at row 3j + b of
     part A (elem_step=3), bank 3 at row j of part B, with
     j = owner*(NPCP/4) + loc//4. The linearity holds for any k-of-4
     interleave, so two big collectives serve four gather tables.
  3. Edges are processed in three phases: (0) edges whose source is owned
     by this core gather from the core's OWN h tables and run inside the
     first collective's window; (1) remote banks {0,1,2} after AG part A
     (75% of remote work, hiding AG part B); (2) remote bank 3. Gathers
     are batched per (phase, chunk-of-groups, bank); 32-granular shared
     run sizes (= max over cores). Segment sums use one-hot matmuls
     M^T @ G accumulated in PSUM; each group keeps ONE psum accumulation
     per phase, spilled to bf16 and re-injected via an identity matmul at
     the next phase. Runs straddling 128-slot block boundaries get one
     matmul per straddled block; out-of-segment slots carry doff 255 so
     their one-hot column is zero.
  4. relu(psum * rsqrt(clip(deg_in,1)) [+ b]) via ACT directly from PSUM
     (bf16 out); BN sums via ones-matmuls (single accumulation group per
     PSUM bank); stats combined via AllGather + local sum; y = h*S + T
     with S = gamma*rsqrt(var+eps), T = beta - mu*S (broadcast-AP DVE
     ops); y cast bf16->f32 during the output DMA (SWDGE), pipelined per
     25-group batch.

Host-side work is limited to integer index bookkeeping (bucketing edges by
(core, locality, src-bank, dst-group), degree counting) and layout
transforms (x^T permutation/padding, int16 gather indices). All
floating-point math runs on device.

Run sizes are padded to a structure shared by all 8 cores so a single SPMD
NEFF serves every core; pad slots re-gather the run's last row (HBM page
hit) and carry a dst offset of 255 -> contribute exactly 0. Edges are
sorted by gather row within each bucket for HBM locality.
"""
import math
import os
import sys

sys.path.insert(0, "/opt/trn_rl_repo")

import numpy as np

import concourse.bacc as bacc
import concourse.bass as bass
import concourse.mybir as mybir
import concourse.tile as tile
from concourse import bass_utils

F32 = mybir.dt.float32
BF16 = mybir.dt.bfloat16
I16 = mybir.dt.int16

CFG = dict(
    N=100000,
    E=1600000,
    IN=256,
    OUT=128,
    NCORES=8,
    GRP=128,          # dst nodes per segment group (= psum partition dim)
    NBANKS=4,         # interleaved src banks (bank rows must be < 32768)
    GCHUNK=7,        # groups per chunk (gather batch granularity)
    EPS=1e-5,
    TRACE=False,
)

LAST_RESULTS = None  # set by kernel() for test harness introspection
LAST_NC = None
LAST_RUN_S = None


def _ceil_div(a, b):
    return (a + b - 1) // b


def _wrap16(idx, ncols):
    """int16 idx list -> [128, ncols] tile: idx i at [i%16, i//16], replicated
    8x across the 16-partition groups (one copy per GpSimd Q7 core)."""
    n = idx.shape[0]
    assert n == ncols * 16
    w = np.ascontiguousarray(idx.reshape(ncols, 16).T)
    return np.tile(w, (8, 1))


def _preprocess(cfg, src, dst):
    """Bucket edges by (owner core, interleaved src bank, dst group); build
    per-core gather-index / dst-offset arrays and the shared run structure."""
    N, E = cfg["N"], cfg["E"]
    C, NBANKS, GRP, GC = cfg["NCORES"], cfg["NBANKS"], cfg["GRP"], cfg["GCHUNK"]
    NPC = N // C
    NG = _ceil_div(NPC, GRP)
    NPCP = NG * GRP                # padded nodes per core (x cols zero-padded)
    assert NPCP % NBANKS == 0
    QB = NPCP // NBANKS            # gather rows per owner per bank view
    BANKROWS = QB * C              # rows per bank view of one AG-half output
    assert BANKROWS < 32768

    src = src.astype(np.int64)
    dst = dst.astype(np.int64)
    deg_out = np.bincount(src, minlength=N).astype(np.float32)
    deg_in = np.bincount(dst, minlength=N).astype(np.float32)

    owner = dst // NPC
    loc = src % NPC
    src_owner = src // NPC
    is_local = src_owner == owner  # src row available before any collective
    # bank classes: 0-3 remote (gather from AG output), 4-7 local (from
    # the core's own h tables, no owner term in the row index)
    bank = loc % NBANKS + NBANKS * is_local
    grow = np.where(is_local, loc // NBANKS, src_owner * QB + loc // NBANKS)
    assert grow.max() < 32768
    grp = (dst % NPC) // GRP
    key = (owner * 2 * NBANKS + bank) * NG + grp
    # sort by bucket, then by gather row inside the bucket (HBM locality)
    order = np.lexsort((grow, key))
    s_grow = grow[order]
    s_dst = dst[order]
    s_key = key[order]

    counts = np.bincount(key, minlength=C * 2 * NBANKS * NG).reshape(
        C, 2 * NBANKS, NG
    )
    P = counts.max(axis=0)  # [NBANKS, NG] shared run sizes (32-granular)
    P = ((P + 31) // 32) * 32
    P = np.maximum(P, 32)   # every (b,g) run structurally exists

    # local banks first (overlap the big collective), then remote 3:1
    phases = [(4, 5, 6, 7), (0, 1, 2), (3,)]
    chunks = [list(range(c, min(c + GC, NG))) for c in range(0, NG, GC)]
    run_seq = [
        (b, g) for ph in phases for ch in chunks for b in ph for g in ch
    ]
    # lay out runs; pad each (phase, chunk, bank) unit to a 128 multiple
    run_off = np.zeros((2 * NBANKS, NG), np.int64)
    units = []  # (bank, first_block, n_blocks) in stream order
    pos = 0
    for ph in phases:
        for ch in chunks:
            for b in ph:
                u0 = pos
                for g in ch:
                    run_off[b, g] = pos
                    pos += P[b, g]
                pos = ((pos + 127) // 128) * 128  # unit pad
                units.append((b, u0 // 128, (pos - u0) // 128))
    nidx_tot = int(pos)
    nb_tot = nidx_tot // 128

    # segments: a run may straddle block boundaries; each (run, block)
    # intersection is one segment = one doff column + one full matmul
    # (out-of-segment slots carry doff 255 -> zero one-hot column).
    run_segs = {}  # (b, g) -> list of (block_t, doff_col, slot_lo, slot_hi)
    nseg = 0
    for b, g in run_seq:
        off = int(run_off[b, g])
        end = off + int(P[b, g])
        segs = []
        t = off // 128
        while t * 128 < end:
            lo = max(off, t * 128)
            hi = min(end, (t + 1) * 128)
            segs.append((t, nseg, lo, hi))
            nseg += 1
            t += 1
        run_segs[(b, g)] = segs

    # boundaries of each (k, b, g) bucket in the sorted edge stream
    bkeys = (
        np.arange(C)[:, None, None] * 2 * NBANKS
        + np.arange(2 * NBANKS)[None, :, None]
    ) * NG + np.arange(NG)[None, None, :]
    starts = np.searchsorted(s_key, bkeys.ravel()).reshape(C, 2 * NBANKS, NG)
    ends = np.searchsorted(s_key, bkeys.ravel(), side="right").reshape(
        C, 2 * NBANKS, NG
    )

    gidx_cores = []
    dstoff_cores = []
    for k in range(C):
        gidx = np.zeros(nidx_tot, np.int16)
        doff_cols = np.full((nseg, 128), 255.0, np.float32)
        for b in range(2 * NBANKS):
            for g in range(NG):
                s, e = starts[k, b, g], ends[k, b, g]
                cnt = e - s
                p0 = int(run_off[b, g])
                if cnt:
                    gidx[p0 : p0 + cnt] = s_grow[s:e].astype(np.int16)
                    # pad slots re-gather the last row (HBM page hit)
                    gidx[p0 + cnt : p0 + int(P[b, g])] = gidx[p0 + cnt - 1]
                    offs = ((s_dst[s:e] % NPC) - g * GRP).astype(np.float32)
                    for t, col, lo, hi in run_segs[(b, g)]:
                        a = max(lo, p0)
                        z = min(hi, p0 + cnt)
                        if z > a:
                            doff_cols[col, a - t * 128 : z - t * 128] = offs[
                                a - p0 : z - p0
                            ]
        # unit-pad slots gather row 0 (gidx stays 0) and have no segment
        gidx_cores.append(_wrap16(gidx, nidx_tot // 16))
        dstoff_cores.append(np.ascontiguousarray(doff_cols.T))

    meta = dict(
        NPC=NPC,
        NPCP=NPCP,
        NG=NG,
        QB=QB,
        BANKROWS=BANKROWS,
        nidx_tot=nidx_tot,
        nb_tot=nb_tot,
        nseg=nseg,
        run_segs=run_segs,
        units=units,
        chunks=chunks,
        run_seq=run_seq,
        deg_out=deg_out,
        deg_in=deg_in,
    )
    return meta, gidx_cores, dstoff_cores


def _tile_major(vec, NG, GRP, pad_val):
    """[NPC] -> [GRP, NG]: entry (p, m) = vec[m*GRP + p], padded."""
    out = np.full((NG * GRP,), pad_val, vec.dtype)
    out[: vec.shape[0]] = vec
    return np.ascontiguousarray(out.reshape(NG, GRP).T)


def _build_nc(cfg, meta, b_nonzero=False):
    N, IN, OUT, C = cfg["N"], cfg["IN"], cfg["OUT"], cfg["NCORES"]
    GRP, NBANKS = cfg["GRP"], cfg["NBANKS"]
    NPC, NPCP, NG = meta["NPC"], meta["NPCP"], meta["NG"]
    nidx_tot, nb_tot = meta["nidx_tot"], meta["nb_tot"]
    units = meta["units"]
    XK = _ceil_div(IN, 128)
    assert OUT == 128 and GRP == 128
    last_w = NPC - (NG - 1) * GRP  # valid rows in the last group
    HALF = NPCP // 2               # rows per AG-half input

    nc = bacc.Bacc(
        "TRN2", target_bir_lowering=False, debug=False, num_devices=C
    )

    # ---- external inputs ----
    NXQ = 8  # x DMA split for earlier stage-B start
    xq = NPCP // NXQ
    assert NPCP % NXQ == 0
    xt = [
        nc.dram_tensor(f"xt{j}", [128, NPCP], BF16, kind="ExternalInput")
        for j in range(XK)
    ]
    wt = [
        nc.dram_tensor(f"wt{j}", [128, OUT], BF16, kind="ExternalInput")
        for j in range(XK)
    ]
    gidx_d = nc.dram_tensor("gidx", [128, nidx_tot // 16], I16, kind="ExternalInput")
    doff_d = nc.dram_tensor("doff", [128, meta["nseg"]], F32, kind="ExternalInput")
    dego_d = nc.dram_tensor("dego", [128, NG], F32, kind="ExternalInput")
    degi_d = nc.dram_tensor("degi", [128, NG], F32, kind="ExternalInput")
    iota_d = nc.dram_tensor("iota", [128, GRP], BF16, kind="ExternalInput")
    gm_d = nc.dram_tensor("gm", [1, OUT], F32, kind="ExternalInput")
    bb_d = nc.dram_tensor("bb", [1, OUT], F32, kind="ExternalInput")
    onesc_d = nc.dram_tensor("onesc", [128, 1], BF16, kind="ExternalInput")
    onest_d = nc.dram_tensor("onest", [128, 1], BF16, kind="ExternalInput")
    onesr_d = nc.dram_tensor("onesr", [1, 128], F32, kind="ExternalInput")
    ident_d = nc.dram_tensor("ident", [128, 128], BF16, kind="ExternalInput")
    if b_nonzero:
        bt_d = nc.dram_tensor("bt", [1, OUT], F32, kind="ExternalInput")

    ypad_d = nc.dram_tensor("ypad", [NG * GRP, OUT], F32, kind="ExternalOutput")

    with tile.TileContext(nc) as tc:
        with (
            tc.tile_pool(name="const", bufs=1) as cpool,
            tc.tile_pool(name="dram", bufs=1, space="DRAM") as dpool,
            tc.tile_pool(name="agg", bufs=1) as apool,
            tc.tile_pool(name="mpool", bufs=32) as mpool,
            tc.tile_pool(name="etmp", bufs=8) as epool,
            tc.tile_pool(name="psg", bufs=4, space="PSUM") as pgpool,
            tc.tile_pool(name="psb", bufs=3, space="PSUM") as pbpool,
            tc.tile_pool(name="pstat", bufs=1, space="PSUM") as pspool,
        ):
            # ---- constants / small tiles ----
            iota_t = cpool.tile([128, GRP], BF16)
            dego_t = cpool.tile([128, NG], F32)
            degi_t = cpool.tile([128, NG], F32)
            nsrc_t = cpool.tile([128, NG], F32)
            ndst_t = cpool.tile([128, NG], F32)
            gm_t = cpool.tile([1, OUT], F32)
            bb_t = cpool.tile([1, OUT], F32)
            onesc_t = cpool.tile([128, 1], BF16)
            onest_t = cpool.tile([128, 1], BF16)
            onesr_t = cpool.tile([1, 128], F32)
            gidx_t = cpool.tile([128, nidx_tot // 16], I16)
            doff_t = cpool.tile([128, meta["nseg"]], F32)
            ident_t = cpool.tile([128, 128], BF16)
            nc.sync.dma_start(ident_t[:], ident_d[:])

            nc.sync.dma_start(iota_t[:], iota_d[:])
            nc.sync.dma_start(dego_t[:], dego_d[:])
            nc.sync.dma_start(degi_t[:], degi_d[:])
            nc.sync.dma_start(gm_t[:], gm_d[:])
            nc.sync.dma_start(bb_t[:], bb_d[:])
            nc.sync.dma_start(onesc_t[:], onesc_d[:])
            nc.sync.dma_start(onest_t[:], onest_d[:])
            nc.sync.dma_start(onesr_t[:], onesr_d[:])
            if b_nonzero:
                bt_t = cpool.tile([1, OUT], F32)
                nc.sync.dma_start(bt_t[:], bt_d[:])

            # norms: rsqrt(max(deg, 1))
            for deg_t, norm_t in ((dego_t, nsrc_t), (degi_t, ndst_t)):
                nc.vector.tensor_scalar(
                    norm_t[:], deg_t[:], 1.0, None, op0=mybir.AluOpType.max
                )
                nc.vector.reciprocal(norm_t[:], norm_t[:])
                nc.scalar.activation(
                    norm_t[:], norm_t[:], mybir.ActivationFunctionType.Sqrt
                )

            # internal DRAM for collectives (3:1 interleaved node split)
            _aspace = "Local" if cfg.get("NOCC") else "Shared"
            HA = 3 * NPCP // 4     # nodes with loc%4 in {0,1,2}
            HB = NPCP // 4         # nodes with loc%4 == 3
            h_my_a = dpool.tile([HA, OUT], BF16, name="h_my_a")
            h_my_b = dpool.tile([HB, OUT], BF16, name="h_my_b")
            h_all_a = dpool.tile(
                [C * HA, OUT], BF16, addr_space=_aspace, name="h_all_a"
            )
            h_all_b = dpool.tile(
                [C * HB, OUT], BF16, addr_space=_aspace, name="h_all_b"
            )
            stats_in = dpool.tile([1, 2 * OUT], F32)
            stats_out = dpool.tile([C, 2 * OUT], F32, addr_space=_aspace)

            # relu(norm*agg) output, bf16, [128, NG, OUT]
            agg_t = apool.tile([128, NG, OUT], BF16)

            # ---- stage B: h = (x @ W) * norm_src, cast bf16, store to HBM
            # (staged in SBUF; 2 large DMAs instead of 98 small ones)
            with tc.tile_pool(name="xw", bufs=1) as xwp:
                xts = []
                wts = []
                for j in range(XK):
                    xts.append(xwp.tile([128, NPCP], BF16, name=f"xt_s{j}"))
                    wts.append(xwp.tile([128, OUT], BF16, name=f"wt_s{j}"))
                for j in range(XK):
                    nc.sync.dma_start(wts[j][:], wt[j][:])
                for q in range(NXQ):
                    for j in range(XK):
                        nc.sync.dma_start(
                            xts[j][:, q * xq : (q + 1) * xq],
                            xt[j][:, q * xq : (q + 1) * xq],
                        )
                hstage = xwp.tile([128, NG, OUT], BF16, name="hstage")
                for m in range(NG):
                    ps = pbpool.tile([128, OUT], F32, tag="hps")
                    for j in range(XK):
                        nc.tensor.matmul(
                            ps[:, :],
                            xts[j][:, m * GRP : (m + 1) * GRP],
                            wts[j][:, :],
                            start=(j == 0),
                            stop=(j == XK - 1),
                        )
                    if m % 2 == 0:
                        nc.scalar.activation(
                            hstage[:, m, :],
                            ps[:, :],
                            mybir.ActivationFunctionType.Copy,
                            scale=nsrc_t[:, m : m + 1],
                        )
                    else:
                        nc.vector.tensor_scalar(
                            hstage[:, m, :],
                            ps[:, :],
                            nsrc_t[:, m : m + 1],
                            None,
                            op0=mybir.AluOpType.mult,
                        )
                # partitions c*32+q hold node loc = g*128 + 4q + c (x columns
                # host-permuted): h_my_a row g*96 + 3q + c, h_my_b row g*32+q
                hq = NG // 4
                qr = [
                    (q8 * hq, (q8 + 1) * hq if q8 < 3 else NG)
                    for q8 in range(4)
                ]
                # part-A staging first: AG part A waits only on these
                for a, z in qr:
                    va = h_my_a[a * 96 : z * 96, :].rearrange(
                        "(g q c) f -> q g c f", q=32, c=3
                    )
                    for c_ in range(3):
                        nc.sync.dma_start(
                            va[:, :, c_, :],
                            hstage[c_ * 32 : (c_ + 1) * 32, a:z, :],
                        )
                for a, z in qr:
                    nc.sync.dma_start(
                        h_my_b[a * 32 : z * 32, :].rearrange(
                            "(g p) f -> p g f", p=32
                        ),
                        hstage[96:128, a:z, :],
                    )

            # ---- stage C: two AllGathers (3/4 part, then 1/4 part) ----
            for h_my_h, h_all_h, hr in (
                (h_my_a, h_all_a, HA),
                (h_my_b, h_all_b, HB),
            ):
                if cfg.get("NOCC"):
                    rep = (
                        h_my_h[:]
                        .rearrange("(o r) f -> o r f", o=1)
                        .to_broadcast((C, hr, OUT))
                    )
                    nc.sync.dma_start(
                        h_all_h[:].rearrange("(o r) f -> o r f", o=C), rep
                    )
                else:
                    nc.gpsimd.collective_compute(
                        "AllGather",
                        mybir.AluOpType.bypass,
                        replica_groups=[list(range(C))],
                        ins=[h_my_h[:]],
                        outs=[h_all_h[:]],
                    )

            # index tables are first needed by stage D's gathers - load
            # them after the x/B/AG chain is underway
            nc.sync.dma_start(gidx_t[:], gidx_d[:])
            nc.sync.dma_start(doff_t[:], doff_d[:])

            # interleaved bank views: banks 0..2 -> row 3j + b of part A,
            # bank 3 -> row j of part B; banks 4..7 -> same views over the
            # core's OWN h tables (usable before any collective completes)
            h_banks = [
                h_all_a[:].rearrange("(j k) f -> j (k f)", k=3)[
                    :, b * OUT : (b + 1) * OUT
                ]
                for b in range(3)
            ] + [h_all_b[:]] + [
                h_my_a[:].rearrange("(j k) f -> j (k f)", k=3)[
                    :, b * OUT : (b + 1) * OUT
                ]
                for b in range(3)
            ] + [h_my_b[:]]
            h_esteps = [3 * OUT, 3 * OUT, 3 * OUT, OUT] * 2

            # ---- stage D: gather + one-hot matmul segmented sum ----
            # ---- stage E (inline): relu(psum*ndst) + BN partial sums ----
            # Gathers are batched per (chunk, bank); groups are processed
            # sequentially (their 4 bank runs back-to-back) so each PSUM bank
            # holds at most one pending accumulation group.
            ps_stat = pspool.tile([1, 2 * OUT], F32, name="ps_stat")
            ps_sum = ps_stat[:, 0:OUT]
            ps_sq = ps_stat[:, OUT : 2 * OUT]
            ndone = [0]  # groups completed (for BN-sum start/stop flags)

            def finish_group(g, ps_g):
                """relu + BN-sum accumulation for a completed group psum."""
                if b_nonzero:
                    tmp = epool.tile([128, OUT], F32, tag="etmp")
                    nc.vector.scalar_tensor_tensor(
                        tmp[:],
                        ps_g[:],
                        ndst_t[:, g : g + 1],
                        btile_t[:],
                        op0=mybir.AluOpType.mult,
                        op1=mybir.AluOpType.add,
                    )
                    nc.scalar.activation(
                        agg_t[:, g, :], tmp[:], mybir.ActivationFunctionType.Relu
                    )
                else:
                    nc.scalar.activation(
                        agg_t[:, g, :],
                        ps_g[:],
                        mybir.ActivationFunctionType.Relu,
                        scale=ndst_t[:, g : g + 1],
                    )
                ones = onesc_t if g < NG - 1 else onest_t
                i0 = ndone[0]
                # ps_sum/ps_sq share one bank = ONE accumulation group:
                # start only on the very first matmul, stop on the very last.
                nc.tensor.matmul(
                    ps_sum,
                    ones[:],
                    agg_t[:, g, :],
                    start=(i0 == 0),
                    stop=False,
                )
                sq = epool.tile([128, OUT], BF16, tag="esq")
                nc.scalar.activation(
                    sq[:], agg_t[:, g, :], mybir.ActivationFunctionType.Square
                )
                nc.tensor.matmul(
                    ps_sq,
                    ones[:],
                    sq[:],
                    start=False,
                    stop=(i0 == NG - 1),
                )
                ndone[0] += 1

            if b_nonzero:
                # replicate b across partitions once (PE broadcast)
                ps_b = pbpool.tile([128, OUT], F32, tag="hps", name="ps_b")
                btile_t = cpool.tile([128, OUT], F32)
                nc.tensor.matmul(ps_b[:], onesr_t[:], bt_t[:], start=True, stop=True)
                nc.scalar.activation(
                    btile_t[:], ps_b[:], mybir.ActivationFunctionType.Copy
                )

            run_segs = meta["run_segs"]
            chunks = meta["chunks"]
            nbmax = max(nb for _, _, nb in units)
            dstack = tc.tile_pool(name="gath", bufs=8)
            gpool = dstack.__enter__()
            phases = [(4, 5, 6, 7), (0, 1, 2), (3,)]
            ui = 0
            for pi, ph in enumerate(phases):
                for ci, ch in enumerate(chunks):
                    gts = {}
                    for b in ph:
                        bank, t0, nblk = units[ui]
                        ui += 1
                        assert bank == b
                        Gt = gpool.tile(
                            [128, nbmax, OUT], BF16, tag="G", name=f"G{pi}_{ci}_{b}"
                        )
                        nc.gpsimd.dma_gather(
                            Gt[:, :nblk, :],
                            h_banks[b],
                            gidx_t[:, t0 * 8 : (t0 + nblk) * 8],
                            nblk * 128,
                            nblk * 128,
                            OUT,
                            elem_step=h_esteps[b],
                            single_packet=False,
                        )
                        gts[b] = (Gt, t0)
                    for g in ch:
                        ps_g = pgpool.tile(
                            [128, OUT], F32, tag="aggps", name=f"ps{pi}_{g}"
                        )
                        if pi > 0:
                            # re-inject previous phase's partial (bf16)
                            nc.tensor.matmul(
                                ps_g[:],
                                ident_t[:],
                                agg_t[:, g, :],
                                start=True,
                                stop=False,
                            )
                        for bi, b in enumerate(ph):
                            Gt, t0 = gts[b]
                            segs = run_segs[(b, g)]
                            for si, (t, col, lo, hi) in enumerate(segs):
                                Mt = mpool.tile([128, GRP], BF16, tag="M")
                                nc.vector.tensor_scalar(
                                    Mt[:],
                                    iota_t[:],
                                    doff_t[:, col : col + 1],
                                    None,
                                    op0=mybir.AluOpType.is_equal,
                                )
                                nc.tensor.matmul(
                                    ps_g[:],
                                    Mt[:],
                                    Gt[:, t - t0, :],
                                    start=(pi == 0 and bi == 0 and si == 0),
                                    stop=(
                                        bi == len(ph) - 1
                                        and si == len(segs) - 1
                                    ),
                                )
                        if pi < len(phases) - 1:
                            # spill partial sum to agg_t (bf16), no relu yet
                            nc.scalar.activation(
                                agg_t[:, g, :],
                                ps_g[:],
                                mybir.ActivationFunctionType.Copy,
                            )
                        else:
                            finish_group(g, ps_g)
            dstack.__exit__(None, None, None)
            assert ndone[0] == NG

            # ---- stage F: AllReduce BN stats; build affine S/T tiles ----
            st_sb = cpool.tile([1, 2 * OUT], F32)
            nc.scalar.activation(
                st_sb[:, 0:OUT], ps_sum, mybir.ActivationFunctionType.Copy
            )
            nc.scalar.activation(
                st_sb[:, OUT : 2 * OUT], ps_sq, mybir.ActivationFunctionType.Copy
            )
            nc.sync.dma_start(stats_in[:], st_sb[:])
            if cfg.get("NOCC"):
                rep = (
                    stats_in[:]
                    .rearrange("(o r) f -> o r f", o=1)
                    .to_broadcast((C, 1, 2 * OUT))
                )
                nc.sync.dma_start(
                    stats_out[:].rearrange("(o r) f -> o r f", o=C), rep
                )
            else:
                nc.gpsimd.collective_compute(
                    "AllGather",
                    mybir.AluOpType.bypass,
                    replica_groups=[list(range(C))],
                    ins=[stats_in[:]],
                    outs=[stats_out[:]],
                )
            # per-core partials land row-major; sum the C rows with one
            # ones-matmul (beats C-1 serial DVE adds with per-op drains)
            st8 = cpool.tile([C, 2 * OUT], F32)
            ones8 = cpool.tile([C, 1], F32)
            nc.gpsimd.memset(ones8[:], 1.0)
            nc.sync.dma_start(st8[:], stats_out[:])
            nc.tensor.matmul(ps_stat[:], ones8[:], st8[:], start=True, stop=True)
            st_rb = cpool.tile([1, 2 * OUT], F32)
            nc.scalar.activation(
                st_rb[:], ps_stat[:], mybir.ActivationFunctionType.Copy
            )

            mu = cpool.tile([1, OUT], F32)
            ex2 = cpool.tile([1, OUT], F32)
            var = cpool.tile([1, OUT], F32)
            srow = cpool.tile([1, OUT], F32)
            trow = cpool.tile([1, OUT], F32)
            inv_n = 1.0 / float(N)
            nc.scalar.activation(
                mu[:], st_rb[:, 0:OUT], mybir.ActivationFunctionType.Copy, scale=inv_n
            )
            nc.scalar.activation(
                ex2[:],
                st_rb[:, OUT : 2 * OUT],
                mybir.ActivationFunctionType.Copy,
                scale=inv_n,
            )
            nc.scalar.activation(var[:], mu[:], mybir.ActivationFunctionType.Square)
            nc.vector.tensor_sub(var[:], ex2[:], var[:])
            # var <- rsqrt(var + eps) (ACT Rsqrt is banned for accuracy)
            nc.scalar.activation(
                var[:],
                var[:],
                mybir.ActivationFunctionType.Copy,
                bias=float(cfg["EPS"]),
            )
            nc.vector.reciprocal(var[:], var[:])
            nc.scalar.activation(var[:], var[:], mybir.ActivationFunctionType.Sqrt)
            nc.vector.tensor_mul(srow[:], gm_t[:], var[:])
            nc.vector.tensor_mul(trow[:], mu[:], srow[:])
            nc.vector.tensor_sub(trow[:], bb_t[:], trow[:])

            S_t = cpool.tile([128, OUT], BF16)
            T_t = cpool.tile([128, OUT], BF16)
            ps_S = pgpool.tile([128, OUT], F32, tag="aggps", name="ps_S")
            ps_T = pgpool.tile([128, OUT], F32, tag="aggps", name="ps_T")
            nc.tensor.matmul(ps_S[:], onesr_t[:], srow[:], start=True, stop=True)
            nc.tensor.matmul(ps_T[:], onesr_t[:], trow[:], start=True, stop=True)
            nc.scalar.activation(S_t[:], ps_S[:], mybir.ActivationFunctionType.Copy)
            nc.scalar.activation(T_t[:], ps_T[:], mybir.ActivationFunctionType.Copy)

            # ---- stage G: y = hrelu * S + T (bf16), cast f32 on DMA out ----
            with tc.tile_pool(name="gtmp", bufs=2) as gpool2:
                GB = 6  # groups per batched op
                S_bc = (
                    S_t[:]
                    .rearrange("p (o f) -> p o f", o=1)
                    .to_broadcast((128, GB, OUT))
                )
                T_bc = (
                    T_t[:]
                    .rearrange("p (o f) -> p o f", o=1)
                    .to_broadcast((128, GB, OUT))
                )
                ypad_view = ypad_d[:].rearrange("(g p) f -> p g f", p=128)
                for g0 in range(0, NG, GB):
                    gw = min(GB, NG - g0)
                    tmp = gpool2.tile([128, GB, OUT], BF16, tag="gtmp")
                    nc.vector.tensor_mul(
                        tmp[:, :gw, :],
                        agg_t[:, g0 : g0 + gw, :],
                        S_bc if gw == GB else S_t[:]
                        .rearrange("p (o f) -> p o f", o=1)
                        .to_broadcast((128, gw, OUT)),
                    )
                    nc.vector.tensor_add(
                        agg_t[:, g0 : g0 + gw, :],
                        tmp[:, :gw, :],
                        T_bc if gw == GB else T_t[:]
                        .rearrange("p (o f) -> p o f", o=1)
                        .to_broadcast((128, gw, OUT)),
                    )
                    nc.gpsimd.dma_start(
                        ypad_view[:, g0 : g0 + gw, :],
                        agg_t[:, g0 : g0 + gw, :],
                    )

    nc.compile()
    return nc


def kernel(x, src, dst, W, b, gamma, beta):
    global LAST_RESULTS
    cfg = CFG
    N, E, IN, OUT, C = cfg["N"], cfg["E"], cfg["IN"], cfg["OUT"], cfg["NCORES"]
    GRP = cfg["GRP"]
    assert x.shape == (N, IN) and W.shape == (IN, OUT)
    assert src.shape == (E,) and dst.shape == (E,)

    b = np.asarray(b, np.float32)
    b_nonzero = bool(np.any(b != 0.0))
    meta, gidx_cores, dstoff_cores = _preprocess(cfg, src, dst)
    NPC, NPCP, NG = meta["NPC"], meta["NPCP"], meta["NG"]
    XK = _ceil_div(IN, 128)
    last_w = NPC - (NG - 1) * GRP
    # node permutation: within each 128-node group, order by loc%4 class
    perm = np.concatenate([np.arange(c, 128, 4) for c in range(4)])
    g_ = np.arange(NPCP) // 128
    p_ = np.arange(NPCP) % 128
    permn = g_ * 128 + perm[p_]          # source node (local) per padded col
    valid = permn < NPC

    nc = _build_nc(cfg, meta, b_nonzero=b_nonzero)

    xT = np.ascontiguousarray(np.asarray(x, np.float32).T)  # [IN, N]
    Wn = np.asarray(W, np.float32)
    import ml_dtypes

    iota = np.tile(np.arange(GRP, dtype=np.float32)[None, :], (128, 1)).astype(
        ml_dtypes.bfloat16
    )
    onesc = np.ones((128, 1), np.float32)
    onest = np.zeros((128, 1), np.float32)
    onest[:last_w] = 1.0
    onesr = np.ones((1, 128), np.float32)
    gm = np.asarray(gamma, np.float32)[None, :]
    bb = np.asarray(beta, np.float32)[None, :]

    in_maps = []
    for k in range(C):
        im = {
            "gidx": gidx_cores[k],
            "doff": dstoff_cores[k],
            "dego": _tile_major(
                np.where(
                    valid,
                    meta["deg_out"][k * NPC + np.minimum(permn, NPC - 1)],
                    np.float32(1.0),
                ).astype(np.float32),
                NG,
                GRP,
                np.float32(1.0),
            ),
            "degi": _tile_major(
                meta["deg_in"][k * NPC : (k + 1) * NPC], NG, GRP, np.float32(1.0)
            ),
            "iota": iota,
            "gm": gm,
            "bb": bb,
            "onesc": onesc.astype(ml_dtypes.bfloat16),
            "onest": onest.astype(ml_dtypes.bfloat16),
            "onesr": onesr,
            "ident": np.eye(128, dtype=np.float32).astype(ml_dtypes.bfloat16),
        }
        if b_nonzero:
            im["bt"] = b[None, :]
        for j in range(XK):
            xcols = np.zeros((128, NPCP), np.float32)
            xcols[:, valid] = xT[
                j * 128 : (j + 1) * 128, k * NPC + permn[valid]
            ]
            im[f"xt{j}"] = xcols.astype(ml_dtypes.bfloat16)
            im[f"wt{j}"] = np.ascontiguousarray(
                Wn[j * 128 : (j + 1) * 128, :]
            ).astype(ml_dtypes.bfloat16)
        in_maps.append(im)

    if cfg.get("SIM"):
        from concourse.bass_interp import MultiCoreSim

        sim = MultiCoreSim(nc, num_cores=C)
        for k, core_sim in sim.cores.items():
            for name, val in in_maps[k].items():
                core_sim.tensor(name)[:] = val
        sim.simulate()
        y = np.empty((N, OUT), np.float32)
        for k in range(C):
            y[k * NPC : (k + 1) * NPC] = sim.cores[k].tensor("ypad")[:NPC]
        return y

    global LAST_NC, LAST_RUN_S
    LAST_NC = nc
    import time as _time

    _t0 = _time.time()
    res = bass_utils.run_bass_kernel_spmd(
        nc,
        in_maps,
        core_ids=list(range(C)),
        trace=cfg.get("TRACE", False),
    )
    LAST_RUN_S = _time.time() - _t0
    LAST_RESULTS = res

    y = np.empty((N, OUT), np.float32)
    for k in range(C):
        y[k * NPC : (k + 1) * NPC] = res.results[k]["ypad"][:NPC]
    return y
